# revision 1
# baseline (speedup 1.0000x reference)
"""DMV inside algorithm (Eisner chart DP, logsumexp semiring) on Trainium2.

Strategy
--------
Pure data parallelism over the batch: 4096 sentences -> 8 cores x 512.
Per core: 2 sequential "runs" of 256 sentences, each laid out as
[128 SBUF partitions] x [G=2 sentence groups in the free dim].

The DP runs in the *exp domain* (no per-split transcendentals): tables hold
exp(score) and each width-w update is one fused strided multiply + one fused
segmented reduce on VectorE, plus a handful of small fixup ops.

Tables are stored *diagonal-packed*: Xd[d*41 + i] = X[i, i+d], which makes
every gather in the width-w recurrence a regular (constant-stride) access
pattern. IR/IL are stored with row r holding width r+1 (IL additionally
column-shifted by +1) so that all four quantities' gathers share one AP.

Numerics: scale composes linearly in span width (every width-w entry contains
exactly w arcs), so on-device renormalization at w=14 and w=28 multiplies
row d by exp(delta*d) (and the per-arc constant tables by exp(delta)),
keeping everything in f32 range. The accumulated delta is returned per
sentence and undone on the host: LL = log(CR[0,len]) - dsum*len.
"""

import os

os.environ.setdefault("JAX_PLATFORMS", "cpu")

import numpy as np

import concourse.bass as bass  # noqa: F401  (registers engine classes)
import concourse.tile as tile
import bass_rust
from concourse import bacc, mybir

F32 = mybir.dt.float32
AF = mybir.ActivationFunctionType
OP = mybir.AluOpType
AX = mybir.AxisListType

N = 41              # fake_len (ROOT at 0)
D = 1681            # table pitch: N*N elements
G = 2               # sentence groups per partition
RUNS = 2            # runs per core (2 * 128 * G = 512 sentences)
NCORES = 8
B_CORE = RUNS * 128 * G
CONST_IN = 4 * D          # host sends 4 pre-exponentiated tables/sentence
STOP_IN = 8 * N           # host sends 8 exp'd stop/go vectors/sentence
RENORM_AT = (20,)

# banks tile: 8 diag-packed tables x 2 groups; slots arranged in 4 blocks of
# 4 so each big op's (q,g) gather is ONE fused AP dim (ISA: max 3 free dims):
#   opA in0: [KR_g0 KR_g1 CRa_g0 CRa_g1]  slots 0..3
#   opA in1: [CLb_g0 CLb_g1 KL_g0 KL_g1]  slots 4..7
#   opB in0: [IR_g0 IR_g1 CLa_g0 CLa_g1]  slots 8..11
#   opB in1: [CRb_g0 CRb_g1 IL_g0 IL_g1]  slots 12..15 (IL stored col+1)
S_KR, S_CRA, S_CLB, S_KL, S_IR, S_CLA, S_CRB, S_IL = (
    0, 2, 4, 6, 8, 10, 12, 14)
# consts tile: 4 per-arc tables, g-major: offset (4*g + C)*D
C_A1, C_B1, C_DA, C_DB = range(4)
# stops tile: 16 vectors of 41, offset (g*8 + v)*41
V_GL0, V_GL1, V_GR0, V_GR1, V_SLNO, V_SLHAS, V_SRNO, V_SRHAS = range(8)

# scratch tile element offsets
Z_P = 0          # 1680: products [qg,i,t]
Z_SSUM = 1680    # 164
Z_TMP1 = 1844    # 164
Z_TMP2 = 2008    # 164
Z_M2 = 2172      # 4
Z_MU = 2176      # 2
Z_LM = 2178      # 2 (reused for delta)
Z_M = 2180       # 84: renorm multiplier table [g, 42]
Z_CROUT = 2264   # 82
Z_DSUM = 2346    # 2
Z_IOTA = 2348    # 42
Z_TOTAL = 2390

LN2_32 = 32.0 * float(np.log(2.0))


def ap_of(t, offset, dims, lead=None):
    """Build a raw AP on tile/dram ap `t`: [lead or t.ap[0]] + dims."""
    ap = t.copy()
    first = list(t.ap[0]) if lead is None else list(lead)
    ap.ap = bass_rust.VecI64Pair([first] + [list(d) for d in dims])
    ap.offset = offset
    return ap


def build_nc():
    nc = bacc.Bacc("TRN2", target_bir_lowering=False, debug=False, num_devices=1)
    consts_in = nc.dram_tensor("consts", [B_CORE, CONST_IN], F32, kind="ExternalInput").ap()
    stops_in = nc.dram_tensor("stops", [B_CORE, STOP_IN], F32, kind="ExternalInput").ap()
    iota_d = nc.dram_tensor("iota", [42], F32, kind="ExternalInput").ap()
    logs_d = nc.dram_tensor("ecr", [B_CORE, N], F32, kind="ExternalOutput").ap()
    dsum_d = nc.dram_tensor("dsum", [B_CORE], F32, kind="ExternalOutput").ap()

    with tile.TileContext(nc) as tc:
        with tc.tile_pool(name="p", bufs=1) as pool:
            banks_t = pool.tile([128, 16 * D], F32)
            consts_t = pool.tile([128, 8 * D], F32)
            stops_t = pool.tile([128, 16 * N], F32)
            z_t = pool.tile([128, Z_TOTAL], F32)
            banks = banks_t[:]
            consts = consts_t[:]
            stops = stops_t[:]
            z = z_t[:]

            v = nc.vector
            sc = nc.scalar

            # iota constant (once)
            nc.sync.dma_start(
                ap_of(z, Z_IOTA, [[1, 42]]),
                ap_of(iota_d, 0, [[1, 42]], lead=[0, 128]),
            )

            for r in range(RUNS):
                base_s = r * 256  # first sentence of this run (per core)

                # ---- load host-precomputed exp-domain constants ----
                nc.sync.dma_start(
                    ap_of(stops, 0, [[STOP_IN, G], [1, STOP_IN]]),
                    ap_of(stops_in, base_s * STOP_IN,
                          [[STOP_IN, G], [1, STOP_IN]], lead=[G * STOP_IN, 128]),
                )
                # consts split by row range: step w reads row w only, so
                # later chunks' DMA hides under early DP steps
                for lo, hi in ((0, 2 * N), (2 * N, 8 * N), (8 * N, D)):
                    for g in range(G):
                        nc.sync.dma_start(
                            ap_of(consts, 4 * g * D + lo, [[D, 4], [1, hi - lo]]),
                            ap_of(consts_in, (base_s + g) * CONST_IN + lo,
                                  [[D, 4], [1, hi - lo]], lead=[G * CONST_IN, 128]),
                        )

                # ---- width-0 init ----
                # zero only rows the in-place renorm rescale can read before
                # the DP writes them (rows <= RENORM_AT[-1]+1); all gather
                # reads stay within written data by construction.
                nc.gpsimd.memset(
                    ap_of(banks, 0, [[D, 16], [1, (RENORM_AT[-1] + 2) * N]]), 0.0)
                v.memset(ap_of(z, Z_DSUM, [[1, 2]]), 0.0)
                # KR[0,:] = KL[0,:] = 1
                v.memset(ap_of(banks, S_KR * D, [[6 * D, 2], [D, 2], [1, N]]), 1.0)
                # CRa/CRb[0,i] = exp(stop[i,R,NO])
                v.tensor_copy(
                    ap_of(banks, S_CRA * D, [[10 * D, 2], [D, 2], [1, N]]),
                    ap_of(stops, V_SRNO * N, [[0, 2], [8 * N, 2], [1, N]]),
                )
                # CLa/CLb[0,i] = exp(stop[i,L,NO])
                v.tensor_copy(
                    ap_of(banks, S_CLA * D, [[-6 * D, 2], [D, 2], [1, N]]),
                    ap_of(stops, V_SLNO * N, [[0, 2], [8 * N, 2], [1, N]]),
                )

                # ---- chart DP ----
                for w in range(1, N):
                    s = N - w
                    row = (w - 1) * N + 1
                    # opA: P[qg,i,t] = {KR,CRa}[t,i] * {CLb,KL}[w-1-t, i+t+1]
                    pa = ap_of(z, Z_P, [[s * w, 4], [w, s], [1, w]])
                    v.tensor_tensor(
                        pa,
                        ap_of(banks, S_KR * D, [[D, 4], [1, s], [N, w]]),
                        ap_of(banks, S_CLB * D + row, [[D, 4], [1, s], [-40, w]]),
                        OP.mult,
                    )
                    v.reduce_sum(
                        ap_of(z, Z_SSUM, [[s, 4], [1, s]]), pa, axis=AX.X,
                    )
                    # tmp1 = Ssum * {A1,B1}[w,:]
                    v.tensor_tensor(
                        ap_of(z, Z_TMP1, [[2 * s, 2], [s, 2], [1, s]]),
                        ap_of(z, Z_SSUM, [[2 * s, 2], [s, 2], [1, s]]),
                        ap_of(consts, C_A1 * D + w * N, [[D, 2], [4 * D, 2], [1, s]]),
                        OP.mult,
                    )
                    # tmp2 = {CLb[w-1,1+i], CRa[w-1,i]} * {DA,DB}[w,:]
                    v.tensor_tensor(
                        ap_of(z, Z_TMP2, [[2 * s, 2], [s, 2], [1, s]]),
                        ap_of(banks, S_CLB * D + row, [[-2 * D - 1, 2], [D, 2], [1, s]]),
                        ap_of(consts, C_DA * D + w * N, [[D, 2], [4 * D, 2], [1, s]]),
                        OP.mult,
                    )
                    # IR[w-1, i] / IL[w-1, i+1] = tmp1 + tmp2
                    v.tensor_tensor(
                        ap_of(banks, S_IR * D + (w - 1) * N, [[6 * D + 1, 2], [D, 2], [1, s]]),
                        ap_of(z, Z_TMP1, [[2 * s, 2], [s, 2], [1, s]]),
                        ap_of(z, Z_TMP2, [[2 * s, 2], [s, 2], [1, s]]),
                        OP.add,
                    )
                    # opB: P[qg,i,t] = {IR,CLa}[t,i] * {CRb,IL}[w-1-t, i+t+1]
                    v.tensor_tensor(
                        pa,
                        ap_of(banks, S_IR * D, [[D, 4], [1, s], [N, w]]),
                        ap_of(banks, S_CRB * D + row, [[D, 4], [1, s], [-40, w]]),
                        OP.mult,
                    )
                    # KR[w,i], KL[w,i] = sum_t
                    v.reduce_sum(
                        ap_of(banks, S_KR * D + w * N, [[6 * D, 2], [D, 2], [1, s]]),
                        pa, axis=AX.X,
                    )
                    # CRa/CRb[w,i] = KR[w,i] * sRhas[i]
                    v.tensor_tensor(
                        ap_of(banks, S_CRA * D + w * N, [[10 * D, 2], [D, 2], [1, s]]),
                        ap_of(banks, S_KR * D + w * N, [[0, 2], [D, 2], [1, s]]),
                        ap_of(stops, V_SRHAS * N, [[0, 2], [8 * N, 2], [1, s]]),
                        OP.mult,
                    )
                    # CLa/CLb[w,i] = KL[w,i] * sLhas[i+w]
                    v.tensor_tensor(
                        ap_of(banks, S_CLA * D + w * N, [[-6 * D, 2], [D, 2], [1, s]]),
                        ap_of(banks, S_KL * D + w * N, [[0, 2], [D, 2], [1, s]]),
                        ap_of(stops, V_SLHAS * N + w, [[0, 2], [8 * N, 2], [1, s]]),
                        OP.mult,
                    )

                    if w in RENORM_AT:
                        s0 = N - w
                        # mu[g] = max_i max(KR[w,i], KL[w,i])
                        v.reduce_max(
                            ap_of(z, Z_M2, [[2, 2], [1, 2]]),
                            ap_of(banks, S_KR * D + w * N, [[6 * D, 2], [D, 2], [1, s0]]),
                            axis=AX.X,
                        )
                        v.tensor_tensor(
                            ap_of(z, Z_MU, [[1, 2]]),
                            ap_of(z, Z_M2, [[1, 2]]),
                            ap_of(z, Z_M2 + 2, [[1, 2]]),
                            OP.max,
                        )
                        # Ln range on ACT is +-2^64: compute via mu*2^-32
                        v.tensor_scalar_mul(
                            ap_of(z, Z_MU, [[1, 2]]), ap_of(z, Z_MU, [[1, 2]]), 2.0**-32
                        )
                        v.tensor_scalar_max(
                            ap_of(z, Z_MU, [[1, 2]]), ap_of(z, Z_MU, [[1, 2]]), 1e-36
                        )
                        sc.activation(
                            ap_of(z, Z_LM, [[1, 2]]), ap_of(z, Z_MU, [[1, 2]]), AF.Ln
                        )
                        # quantize the per-width shift to delta = -k*ln2 with
                        # k integer, so every rescale factor is an EXACT power
                        # of two (the ACT exp LUT would otherwise perturb all
                        # tables by its relative error).
                        # kf = round((log(mu*2^-32) + 32 ln2) / (w ln2))
                        v.tensor_scalar(
                            ap_of(z, Z_LM, [[1, 2]]), ap_of(z, Z_LM, [[1, 2]]),
                            LN2_32, 1.0 / (w * float(np.log(2.0))),
                            OP.add, OP.mult,
                        )
                        v.tensor_scalar(
                            ap_of(z, Z_LM, [[1, 2]]), ap_of(z, Z_LM, [[1, 2]]),
                            12582912.0, 12582912.0, OP.add, OP.subtract,
                        )
                        # dsum accumulates k (exact small integers)
                        v.tensor_tensor(
                            ap_of(z, Z_DSUM, [[1, 2]]),
                            ap_of(z, Z_DSUM, [[1, 2]]),
                            ap_of(z, Z_LM, [[1, 2]]),
                            OP.add,
                        )
                        # scale2 = 2^-k via exponent bits: (127 - k) << 23
                        v.tensor_scalar(
                            ap_of(z, Z_M2, [[1, 2]]), ap_of(z, Z_LM, [[1, 2]]),
                            -1.0, 127.0, OP.mult, OP.add,
                        )
                        zi = z.bitcast(mybir.dt.int32)
                        v.tensor_copy(
                            ap_of(zi, Z_M2 + 2, [[1, 2]]),
                            ap_of(z, Z_M2, [[1, 2]]),
                        )
                        v.tensor_scalar(
                            ap_of(zi, Z_M2 + 2, [[1, 2]]),
                            ap_of(zi, Z_M2 + 2, [[1, 2]]),
                            23, None, OP.arith_shift_left,
                        )
                        # M[g, d] = 2^(-k*d): d=0 -> 1, then multiplicative scan
                        v.memset(ap_of(z, Z_M, [[42, 2], [1, 1]]), 1.0)
                        for g in range(G):
                            sca = ap_of(z, Z_M2 + 2 + g, [[0, 41]])
                            v.tensor_tensor_scan(
                                ap_of(z, Z_M + g * 42 + 1, [[1, 41]]),
                                sca, sca, 1.0, OP.mult, OP.bypass,
                            )
                        for g in range(G):
                            # natural tables, rows d<=w: scale by exp(delta*d)
                            tA = ap_of(banks, g * D, [[2 * D, 4], [N, w + 1], [1, N]])
                            v.tensor_tensor(
                                tA, tA,
                                ap_of(z, Z_M + g * 42, [[0, 4], [1, w + 1], [0, N]]),
                                OP.mult,
                            )
                            tB = ap_of(banks, (10 + g) * D, [[2 * D, 2], [N, w + 1], [1, N]])
                            v.tensor_tensor(
                                tB, tB,
                                ap_of(z, Z_M + g * 42, [[0, 2], [1, w + 1], [0, N]]),
                                OP.mult,
                            )
                            # IR/IL rows r<=w-1 hold width r+1: exp(delta*(r+1))
                            tI = ap_of(banks, (8 + g) * D, [[6 * D, 2], [N, w], [1, N]])
                            v.tensor_tensor(
                                tI, tI,
                                ap_of(z, Z_M + g * 42 + 1, [[0, 2], [1, w], [0, N]]),
                                OP.mult,
                            )
                            # const rows > w: one extra arc factor exp(delta)
                            tC = ap_of(consts, 4 * g * D + (w + 1) * N,
                                       [[D, 4], [N, 40 - w], [1, N]])
                            v.tensor_tensor(
                                tC, tC,
                                ap_of(z, Z_M + g * 42 + 1, [[0, 4], [0, 40 - w], [0, N]]),
                                OP.mult,
                            )

                # ---- extract raw exp-domain CR[0, j] (log on host) ----
                v.tensor_copy(
                    ap_of(z, Z_CROUT, [[N, 2], [1, N]]),
                    ap_of(banks, S_CRA * D, [[D, 2], [N, N]]),
                )
                nc.sync.dma_start(
                    ap_of(logs_d, base_s * N, [[N, G], [1, N]], lead=[G * N, 128]),
                    ap_of(z, Z_CROUT, [[N, G], [1, N]]),
                )
                nc.sync.dma_start(
                    ap_of(dsum_d, base_s, [[1, G]], lead=[G, 128]),
                    ap_of(z, Z_DSUM, [[1, G]]),
                )

    nc.compile()
    return nc


_NC_CACHE = {}


def get_nc():
    if "nc" not in _NC_CACHE:
        _NC_CACHE["nc"] = build_nc()
    return _NC_CACHE["nc"]


def make_in_maps(trans_scores, dec_scores):
    t = np.asarray(trans_scores, dtype=np.float32)
    dec = np.asarray(dec_scores, dtype=np.float32)
    B = t.shape[0]
    go = dec[..., 0]                        # [B, n, dir, dv]
    # per-sentence linear pre-shift: each arc factor carries exp(-c0), so a
    # width-w entry is scaled exp(-c0*w); undone on the host at the end.
    tm = np.where(t < -1e8, -np.inf, t).max(axis=3)
    with np.errstate(invalid="ignore"):
        colmax = tm.max(axis=1)             # [B, n] best arc into each child
        proxy = np.nanmean(
            np.where(np.isfinite(colmax), colmax, np.nan)[:, 1:], axis=-1)
    c0 = (proxy + 0.5).astype(np.float32)
    c0 = np.clip(np.nan_to_num(c0), -20.0, 20.0)
    # one exp over trans (NEG -> 0 underflow is intended), then gather diags
    with np.errstate(under="ignore"):
        E = np.exp(t - c0[:, None, None, None])      # [B, n, n, 2]
        ego = np.exp(go)                             # [B, n, 2, 2]
    d_idx, i_idx = np.meshgrid(np.arange(N), np.arange(N), indexing="ij")
    j_idx = np.minimum(i_idx + d_idx, N - 1)
    valid = ((i_idx + d_idx) <= N - 1)[None].astype(np.float32)
    ea = E[:, i_idx, j_idx, :]              # [B, n, n, 2]  trans[i, i+d, v]
    eb = E[:, j_idx, i_idx, :]              # [B, n, n, 2]  trans[i+d, i, v]
    a1 = ea[..., 1] * ego[:, :, 1, 1][:, i_idx] * valid
    a0 = ea[..., 0] * ego[:, :, 1, 0][:, i_idx] * valid
    b1 = eb[..., 1] * ego[:, :, 0, 1][:, j_idx] * valid
    b0 = eb[..., 0] * ego[:, :, 0, 0][:, j_idx] * valid
    consts = np.empty((B, 4, N, N), dtype=np.float32)
    consts[:, 0] = a1
    consts[:, 1] = b1
    consts[:, 2] = a0 - a1
    consts[:, 3] = b0 - b1
    consts = consts.reshape(B, CONST_IN)
    est = np.exp(dec[..., 1])               # [B, n, dir, dv]
    stops = np.empty((B, 8, N), dtype=np.float32)
    stops[:, 0] = ego[:, :, 0, 0]; stops[:, 1] = ego[:, :, 0, 1]
    stops[:, 2] = ego[:, :, 1, 0]; stops[:, 3] = ego[:, :, 1, 1]
    stops[:, 4] = est[:, :, 0, 0]; stops[:, 5] = est[:, :, 0, 1]
    stops[:, 6] = est[:, :, 1, 0]; stops[:, 7] = est[:, :, 1, 1]
    stops = stops.reshape(B, STOP_IN)
    iota = np.arange(42, dtype=np.float32)
    in_maps = []
    for c in range(NCORES):
        sl = slice(c * B_CORE, (c + 1) * B_CORE)
        in_maps.append({
            "consts": consts[sl],
            "stops": stops[sl],
            "iota": iota,
        })
    return in_maps, c0


def assemble(results, len_array, c0):
    ln = np.asarray(len_array).astype(np.int64)
    c0 = np.asarray(c0).astype(np.float64)
    out = np.empty(len(ln), dtype=np.float32)
    for c, res in enumerate(results):
        ecr = res["ecr"].reshape(B_CORE, N).astype(np.float64)
        dsum = res["dsum"].reshape(B_CORE).astype(np.float64)
        lc = ln[c * B_CORE:(c + 1) * B_CORE]
        idx = np.arange(B_CORE)
        with np.errstate(divide="ignore"):
            out[c * B_CORE:(c + 1) * B_CORE] = (
                np.log(ecr[idx, lc]) + dsum * np.log(2.0) * lc
                + c0[c * B_CORE:(c + 1) * B_CORE] * lc
            ).astype(np.float32)
    return out


def kernel(trans_scores, dec_scores, len_array):
    from concourse.bass_utils import run_bass_kernel_spmd

    nc = get_nc()
    in_maps, c0 = make_in_maps(trans_scores, dec_scores)
    res = run_bass_kernel_spmd(nc, in_maps, core_ids=list(range(NCORES)))
    return assemble(res.results, len_array, c0)



# revision 4
# speedup vs baseline: 2.3294x; 2.3294x over previous
"""DMV inside algorithm (Eisner chart DP, logsumexp semiring) on Trainium2.

Strategy
--------
Data parallel over the batch: 4096 sentences -> 8 cores x 512, with
length-aware packing: sentences are sorted by length (desc) and dealt
round-robin to cores, then within a core split into 4 slots of 128
(one sentence per SBUF partition per slot). Slot q only runs chart
widths w <= L[q] (its max length), cutting DP work ~2.4x vs running
all sentences to width 40.

The DP runs in the *exp domain* (tables hold exp(score)); each width-w
update is a fused strided multiply + segmented reduce. Multiplies are
split between VectorE (DVE) and GpSimd (Pool) to use both engines;
segmented reduces only exist on DVE.

Per slot q the 6 diag-packed tables [R=L+1 rows x 41] live at stride
D=41*R in the order [KR, CR, IL, IR, CL, KL], chosen so every fused
operand pair is adjacent: opA in0={KR,CR}, in1={CL,KL}; opB in0={IR,CL},
in1={CR,IL}; outputs {IR,IL}, {KR,KL}, {CR,CL} all constant-stride.
IR/IL row r holds width r+1; IL/opB patterns are column-shifted so all
gathers are constant-stride (as in the classic diagonal packing).

The valence edge case (closest-child GO) is handled by excluding the
edge split (t=0 for right arcs, t=w-1 for left arcs) from the reduce --
the fused AP absorbs the per-slot offset difference into the pair
stride -- and adding edge*{A0,B0} separately. Per-arc constants
{A1,B1,A0,B0} are host-precomputed per width row (triangular pack).

Numerics: scale composes linearly in span width, so slots with L >= 25
renormalize once at w=20: row d of every table is multiplied by an
exact power of two 2^(-k*d) (k integer per sentence), k accumulated in
dsum and undone on the host: LL = log(CR[0,len]) + k*ln2*len + c0*len.
"""

import os

os.environ.setdefault("JAX_PLATFORMS", "cpu")

import numpy as np

import concourse.bass as bass  # noqa: F401  (registers engine classes)
import concourse.tile as tile
import bass_rust
from concourse import bacc, mybir

F32 = mybir.dt.float32
AF = mybir.ActivationFunctionType
OP = mybir.AluOpType
AX = mybir.AxisListType

N = 41              # fake_len (ROOT at 0)
NCORES = 8
NSLOT = 4
P128 = 128
B_CORE = NSLOT * P128
RENORM_W = 20       # renorm width (slots with L >= RENORM_MIN_L)
RENORM_MIN_L = 25

# table order within a slot (pairs used by the fused ops are adjacent)
T_KR, T_CR, T_IL, T_IR, T_CL, T_KL = range(6)

LN2 = float(np.log(2.0))


def ap_of(t, offset, dims, lead=None):
    """Build a raw AP on tile/dram ap `t`: [lead or t.ap[0]] + dims."""
    ap = t.copy()
    first = list(t.ap[0]) if lead is None else list(lead)
    ap.ap = bass_rust.VecI64Pair([first] + [list(d) for d in dims])
    ap.offset = offset
    return ap


def _layout(bounds):
    """Compute per-slot offsets for banks / consts / stops / scratch."""
    L = list(bounds)
    R = [l + 1 for l in L]
    D = [N * r for r in R]
    bank_base, acc = [], 0
    for q in range(NSLOT):
        bank_base.append(acc)
        acc += 6 * D[q]
    bank_total = acc
    # consts: per q, per w in 1..L[q]: [A1(s), B1(s), A0(s), B0(s)]
    cbase, coff, acc = [], [], 0
    for q in range(NSLOT):
        cbase.append(acc)
        offs = {}
        for w in range(1, L[q] + 1):
            offs[w] = acc
            acc += 4 * (R[q] - w)
        coff.append(offs)
    const_total = acc
    stop_total = NSLOT * 4 * N  # per q: [SRHAS, SLHAS, SRNO, SLNO] x 41
    # z scratch
    z = {}
    zacc = 0
    for q in range(NSLOT):
        pmax = max((2 * (R[q] - w) * w for w in range(1, L[q] + 1)),
                   default=2)
        z[("P", q)] = zacc; zacc += pmax
        z[("SS", q)] = zacc; zacc += 2 * N
        z[("T1", q)] = zacc; zacc += 2 * N
        z[("T2", q)] = zacc; zacc += 2 * N
    z["CROUT"] = zacc; zacc += NSLOT * N
    z["DSUM"] = zacc; zacc += NSLOT
    z["M2"] = zacc; zacc += 8
    z["MU"] = zacc; zacc += 2
    z["LM"] = zacc; zacc += 2
    z["M"] = zacc; zacc += NSLOT * (N + 1)  # renorm multiplier rows [q, 42]
    z_total = zacc
    return dict(L=L, R=R, D=D, bank_base=bank_base, bank_total=bank_total,
                cbase=cbase, coff=coff, const_total=const_total,
                stop_total=stop_total, z=z, z_total=z_total)


def build_nc(bounds):
    lay = _layout(bounds)
    L, R, D = lay["L"], lay["R"], lay["D"]
    bb = lay["bank_base"]
    coff = lay["coff"]
    zoff = lay["z"]
    renorm_qs = [q for q in range(NSLOT) if L[q] >= RENORM_MIN_L]
    # bounds are sorted desc, so renorm slots are the prefix [0..nq) and
    # dsum[j] lines up with slot j
    assert renorm_qs == list(range(len(renorm_qs)))

    nc = bacc.Bacc("TRN2", target_bir_lowering=False, debug=False,
                   num_devices=1)
    consts_in = nc.dram_tensor(
        "consts", [P128, lay["const_total"]], F32, kind="ExternalInput").ap()
    stops_in = nc.dram_tensor(
        "stops", [P128, lay["stop_total"]], F32, kind="ExternalInput").ap()
    ecr_d = nc.dram_tensor(
        "ecr", [P128, NSLOT * N], F32, kind="ExternalOutput").ap()
    dsum_d = nc.dram_tensor(
        "dsum", [P128, NSLOT], F32, kind="ExternalOutput").ap()

    with tile.TileContext(nc) as tc:
        with tc.tile_pool(name="p", bufs=1) as pool:
            banks_t = pool.tile([P128, lay["bank_total"]], F32)
            consts_t = pool.tile([P128, lay["const_total"]], F32)
            stops_t = pool.tile([P128, lay["stop_total"]], F32)
            z_t = pool.tile([P128, lay["z_total"]], F32)
            banks = banks_t[:]
            consts = consts_t[:]
            stops = stops_t[:]
            z = z_t[:]

            v = nc.vector
            g = nc.gpsimd
            sc = nc.scalar

            # ---- input DMA ----
            nc.sync.dma_start(
                ap_of(stops, 0, [[1, lay["stop_total"]]]),
                ap_of(stops_in, 0, [[1, lay["stop_total"]]],
                      lead=[lay["stop_total"], P128]),
            )
            # consts split at row 6 so the DP can start before the bulk lands
            for q in range(NSLOT):
                lo = coff[q][1]
                mid = coff[q][7] if L[q] >= 7 else (
                    coff[q][L[q]] + 4 * (R[q] - L[q]))
                hi = lay["cbase"][q + 1] if q + 1 < NSLOT else \
                    lay["const_total"]
                nc.sync.dma_start(
                    ap_of(consts, lo, [[1, mid - lo]]),
                    ap_of(consts_in, lo, [[1, mid - lo]],
                          lead=[lay["const_total"], P128]),
                )
                if hi > mid:
                    nc.sync.dma_start(
                        ap_of(consts, mid, [[1, hi - mid]]),
                        ap_of(consts_in, mid, [[1, hi - mid]],
                              lead=[lay["const_total"], P128]),
                    )

            # ---- init ----
            # zero rows 1..RENORM_W of all 6 tables for renorm slots (the
            # in-place rescale reads full 41-wide rows); staged in 3 chunks
            # so early DP widths don't wait on the full memset.
            for q in renorm_qs:
                for r0, r1 in ((1, 3), (3, 7), (7, RENORM_W + 1)):
                    g.memset(
                        ap_of(banks, bb[q] + r0 * N,
                              [[D[q], 6], [1, (r1 - r0) * N]]), 0.0)
            v.memset(ap_of(z, zoff["DSUM"], [[1, NSLOT]]), 0.0)
            v.memset(ap_of(z, zoff["CROUT"], [[1, NSLOT * N]]), 0.0)
            for q in range(NSLOT):
                # KR[0,:] = KL[0,:] = 1
                v.memset(
                    ap_of(banks, bb[q] + T_KR * D[q],
                          [[5 * D[q], 2], [1, N]]), 1.0)
                # CR[0,i] = exp(stop[i,R,NO]); CL[0,i] = exp(stop[i,L,NO])
                v.tensor_copy(
                    ap_of(banks, bb[q] + T_CR * D[q],
                          [[3 * D[q], 2], [1, N]]),
                    ap_of(stops, (4 * q + 2) * N, [[N, 2], [1, N]]),
                )

            # ---- chart DP ----
            def emit_w_q(w, q):
                s = R[q] - w
                Dq = D[q]
                base = bb[q]
                on_pool = q <= 1  # engine split for the big multiplies
                me = g if on_pool else v
                pa = ap_of(z, zoff[("P", q)], [[(w - 1) * s, 2],
                                               [s, w - 1], [1, s]])
                par = ap_of(z, zoff[("P", q)], [[(w - 1) * s, 2],
                                                [1, s], [s, w - 1]])
                pb = ap_of(z, zoff[("P", q)], [[w * s, 2], [s, w], [1, s]])
                pbr = ap_of(z, zoff[("P", q)], [[w * s, 2], [1, s], [s, w]])
                t1 = ap_of(z, zoff[("T1", q)], [[s, 2], [1, s]])
                t2 = ap_of(z, zoff[("T2", q)], [[s, 2], [1, s]])
                ss = ap_of(z, zoff[("SS", q)], [[s, 2], [1, s]])
                # edge operand {CL[w-1,i+1], CR[w-1,i]}
                edge = ap_of(banks, base + T_CL * Dq + (w - 1) * N + 1,
                             [[-3 * Dq - 1, 2], [1, s]])
                irout = ap_of(banks, base + T_IR * Dq + (w - 1) * N,
                              [[-Dq + 1, 2], [1, s]])
                if w == 1:
                    # IR[0,i] = edge * {A0,B0}
                    v.tensor_tensor(
                        irout, edge,
                        ap_of(consts, coff[q][w] + 2 * s, [[s, 2], [1, s]]),
                        OP.mult)
                else:
                    # opA: P[2,t,i] = {KR[t+1,i], CR[t,i]}
                    #               * {CL[w-2-t,i+t+2], KL[w-1-t,i+t+1]}
                    me.tensor_tensor(
                        pa,
                        ap_of(banks, base + T_KR * Dq + N,
                              [[Dq - N, 2], [N, w - 1], [1, s]]),
                        ap_of(banks, base + T_CL * Dq + (w - 2) * N + 2,
                              [[Dq + N - 1, 2], [-(N - 1), w - 1], [1, s]]),
                        OP.mult)
                    v.reduce_sum(ss, par, axis=AX.X)
                    v.tensor_tensor(
                        t1, ss,
                        ap_of(consts, coff[q][w], [[s, 2], [1, s]]),
                        OP.mult)
                    v.tensor_tensor(
                        t2, edge,
                        ap_of(consts, coff[q][w] + 2 * s, [[s, 2], [1, s]]),
                        OP.mult)
                    v.tensor_tensor(irout, t1, t2, OP.add)
                # opB: P[2,t,i] = {IR[t,i], CL[t,i]}
                #               * {CR[w-1-t,i+t+1], IL[w-1-t,i+t+1]}
                krout = ap_of(banks, base + T_KR * Dq + w * N,
                              [[5 * Dq, 2], [1, s]])
                in0b = ap_of(banks, base + T_IR * Dq,
                             [[Dq, 2], [N, w], [1, s]])
                in1b = ap_of(banks, base + T_CR * Dq + (w - 1) * N + 1,
                             [[Dq, 2], [-(N - 1), w], [1, s]])
                if w == 1:
                    v.tensor_tensor(
                        krout,
                        ap_of(banks, base + T_IR * Dq, [[Dq, 2], [1, s]]),
                        ap_of(banks, base + T_CR * Dq + 1,
                              [[Dq, 2], [1, s]]),
                        OP.mult)
                else:
                    me.tensor_tensor(pb, in0b, in1b, OP.mult)
                    v.reduce_sum(krout, pbr, axis=AX.X)
                # {CR[w,i], CL[w,i]} = {KR[w,i]*sRhas[i], KL[w,i]*sLhas[i+w]}
                v.tensor_tensor(
                    ap_of(banks, base + T_CR * Dq + w * N,
                          [[3 * Dq, 2], [1, s]]),
                    ap_of(banks, base + T_KR * Dq + w * N,
                          [[5 * Dq, 2], [1, s]]),
                    ap_of(stops, 4 * q * N, [[N + w, 2], [1, s]]),
                    OP.mult)

            def emit_renorm():
                # per renorm slot: k = round(log2(max KR/KL at w) / w);
                # rescale row d by exact 2^(-k*d); accumulate k into dsum.
                w = RENORM_W
                nq = len(renorm_qs)
                if not nq:
                    return
                for j, q in enumerate(renorm_qs):
                    s = R[q] - w
                    v.reduce_max(
                        ap_of(z, zoff["M2"] + 2 * j, [[1, 2]]),
                        ap_of(banks, bb[q] + T_KR * D[q] + w * N,
                              [[5 * D[q], 2], [1, s]]),
                        axis=AX.X)
                # mu[j] = max over the {KR,KL} pair
                v.tensor_tensor(
                    ap_of(z, zoff["MU"], [[1, nq]]),
                    ap_of(z, zoff["M2"], [[2, nq]]),
                    ap_of(z, zoff["M2"] + 1, [[2, nq]]),
                    OP.max)
                # Ln range on ACT is +-2^64: compute via mu*2^-32
                v.tensor_scalar_mul(
                    ap_of(z, zoff["MU"], [[1, nq]]),
                    ap_of(z, zoff["MU"], [[1, nq]]), 2.0 ** -32)
                v.tensor_scalar_max(
                    ap_of(z, zoff["MU"], [[1, nq]]),
                    ap_of(z, zoff["MU"], [[1, nq]]), 1e-36)
                sc.activation(
                    ap_of(z, zoff["LM"], [[1, nq]]),
                    ap_of(z, zoff["MU"], [[1, nq]]), AF.Ln)
                # k = round((ln(mu*2^-32) + 32 ln2)/(w ln2)); round via the
                # 1.5*2^23 trick so every factor is an exact power of two
                v.tensor_scalar(
                    ap_of(z, zoff["LM"], [[1, nq]]),
                    ap_of(z, zoff["LM"], [[1, nq]]),
                    32.0 * LN2, 1.0 / (w * LN2), OP.add, OP.mult)
                v.tensor_scalar(
                    ap_of(z, zoff["LM"], [[1, nq]]),
                    ap_of(z, zoff["LM"], [[1, nq]]),
                    12582912.0, 12582912.0, OP.add, OP.subtract)
                # dsum[q] += k  (host undoes k*ln2*len)
                v.tensor_tensor(
                    ap_of(z, zoff["DSUM"], [[1, nq]]),
                    ap_of(z, zoff["DSUM"], [[1, nq]]),
                    ap_of(z, zoff["LM"], [[1, nq]]),
                    OP.add)
                # scale2 = 2^-k via exponent bits: (127 - k) << 23
                v.tensor_scalar(
                    ap_of(z, zoff["M2"], [[1, nq]]),
                    ap_of(z, zoff["LM"], [[1, nq]]),
                    -1.0, 127.0, OP.mult, OP.add)
                zi = z.bitcast(mybir.dt.int32)
                v.tensor_copy(
                    ap_of(zi, zoff["M2"] + 4, [[1, nq]]),
                    ap_of(z, zoff["M2"], [[1, nq]]))
                v.tensor_scalar(
                    ap_of(zi, zoff["M2"] + 4, [[1, nq]]),
                    ap_of(zi, zoff["M2"] + 4, [[1, nq]]),
                    23, None, OP.arith_shift_left)
                # M[j, d] = 2^(-k*d): d=0 -> 1, then multiplicative scan
                v.memset(ap_of(z, zoff["M"], [[N + 1, nq], [1, 1]]), 1.0)
                for j in range(nq):
                    sca = ap_of(z, zoff["M2"] + 4 + j, [[0, N]])
                    v.tensor_tensor_scan(
                        ap_of(z, zoff["M"] + j * (N + 1) + 1, [[1, N]]),
                        sca, sca, 1.0, OP.mult, OP.bypass)
                for j, q in enumerate(renorm_qs):
                    Dq, base = D[q], bb[q]
                    mrow = zoff["M"] + j * (N + 1)
                    eng = g if q == 0 else v
                    # natural tables rows d<=w: *2^(-k*d)
                    for tb in (T_KR, T_CL):
                        eng.tensor_tensor(
                            ap_of(banks, base + tb * Dq,
                                  [[Dq, 2], [N, w + 1], [1, N]]),
                            ap_of(banks, base + tb * Dq,
                                  [[Dq, 2], [N, w + 1], [1, N]]),
                            ap_of(z, mrow, [[0, 2], [1, w + 1], [0, N]]),
                            OP.mult)
                    # IR/IL rows r<=w-1 hold width r+1: *2^(-k*(r+1))
                    eng.tensor_tensor(
                        ap_of(banks, base + T_IL * Dq,
                              [[Dq, 2], [N, w], [1, N]]),
                        ap_of(banks, base + T_IL * Dq,
                              [[Dq, 2], [N, w], [1, N]]),
                        ap_of(z, mrow + 1, [[0, 2], [1, w], [0, N]]),
                        OP.mult)
                    # const rows > w: one extra arc factor 2^(-k)
                    lo = coff[q][w + 1]
                    hi = lay["cbase"][q + 1] if q + 1 < NSLOT else \
                        lay["const_total"]
                    v.tensor_tensor(
                        ap_of(consts, lo, [[1, hi - lo]]),
                        ap_of(consts, lo, [[1, hi - lo]]),
                        ap_of(z, mrow + 1, [[0, hi - lo]]),
                        OP.mult)

            for w in range(1, max(L) + 1):
                for q in range(NSLOT):
                    if w <= L[q]:
                        emit_w_q(w, q)
                    elif w == L[q] + 1:
                        # extract exp CR[0, 0..L] (stride-41 diag gather)
                        v.tensor_copy(
                            ap_of(z, zoff["CROUT"] + q * N, [[1, R[q]]]),
                            ap_of(banks, bb[q] + T_CR * D[q], [[N, R[q]]]))
                if w == RENORM_W:
                    emit_renorm()
            for q in range(NSLOT):
                if L[q] == max(L):
                    v.tensor_copy(
                        ap_of(z, zoff["CROUT"] + q * N, [[1, R[q]]]),
                        ap_of(banks, bb[q] + T_CR * D[q], [[N, R[q]]]))

            # ---- outputs ----
            nc.sync.dma_start(
                ap_of(ecr_d, 0, [[1, NSLOT * N]], lead=[NSLOT * N, P128]),
                ap_of(z, zoff["CROUT"], [[1, NSLOT * N]]),
            )
            nc.sync.dma_start(
                ap_of(dsum_d, 0, [[1, NSLOT]], lead=[NSLOT, P128]),
                ap_of(z, zoff["DSUM"], [[1, NSLOT]]),
            )

    nc.compile()
    return nc


_NC_CACHE = {}


def get_nc(bounds):
    key = tuple(bounds)
    if key not in _NC_CACHE:
        _NC_CACHE[key] = build_nc(key)
    return _NC_CACHE[key]


def plan(len_array):
    """Sort sentences by length desc, deal round-robin to cores, slot into
    4 groups of 128 per core. Returns (order, bounds) where order[r] is the
    original sentence index of global sorted rank r and bounds[q] is the
    width bound of slot q (same for every core by round-robin construction).
    """
    ln = np.asarray(len_array).astype(np.int64)
    order = np.argsort(-ln, kind="stable")
    bounds = [int(ln[order[min(1024 * q, len(ln) - 1)]])
              for q in range(NSLOT)]
    bounds = [max(b, 1) for b in bounds]
    return order, bounds


def make_in_maps(trans_scores, dec_scores, len_array):
    t = np.asarray(trans_scores, dtype=np.float32)
    dec = np.asarray(dec_scores, dtype=np.float32)
    B = t.shape[0]
    order, bounds = plan(len_array)
    lay = _layout(bounds)
    go = dec[..., 0]                        # [B, n, dir, dv]
    # per-sentence linear pre-shift: each arc factor carries exp(-c0), so a
    # width-w entry is scaled exp(-c0*w); undone on the host at the end.
    tm = np.where(t < -1e8, -np.inf, t).max(axis=3)
    with np.errstate(invalid="ignore"):
        colmax = tm.max(axis=1)             # [B, n] best arc into each child
        proxy = np.nanmean(
            np.where(np.isfinite(colmax), colmax, np.nan)[:, 1:], axis=-1)
    c0 = (proxy + 0.5).astype(np.float32)
    c0 = np.clip(np.nan_to_num(c0), -20.0, 20.0)
    # one exp over trans (NEG -> 0 underflow is intended), then gather diags
    with np.errstate(under="ignore"):
        E = np.exp(t - c0[:, None, None, None])      # [B, n, n, 2]
        ego = np.exp(go)                             # [B, n, 2, 2]
    d_idx, i_idx = np.meshgrid(np.arange(N), np.arange(N), indexing="ij")
    j_idx = np.minimum(i_idx + d_idx, N - 1)
    valid = ((i_idx + d_idx) <= N - 1)[None].astype(np.float32)
    ea = E[:, i_idx, j_idx, :]              # [B, n, n, 2]  trans[i, i+d, v]
    eb = E[:, j_idx, i_idx, :]              # [B, n, n, 2]  trans[i+d, i, v]
    a1 = ea[..., 1] * ego[:, :, 1, 1][:, i_idx] * valid   # [B, d, i]
    a0 = ea[..., 0] * ego[:, :, 1, 0][:, i_idx] * valid
    b1 = eb[..., 1] * ego[:, :, 0, 1][:, j_idx] * valid
    b0 = eb[..., 0] * ego[:, :, 0, 0][:, j_idx] * valid
    est = np.exp(dec[..., 1])               # [B, n, dir, dv]

    # sentence index per (core, slot, partition)
    sent = order.reshape(-1, NCORES).T.reshape(NCORES, NSLOT, P128)

    consts = np.empty((NCORES, P128, lay["const_total"]), dtype=np.float32)
    for q in range(NSLOT):
        sq = sent[:, q, :]                  # [NCORES, P128]
        for w in range(1, bounds[q] + 1):
            s = bounds[q] + 1 - w
            o = lay["coff"][q][w]
            consts[:, :, o:o + s] = a1[sq, w, :s]
            consts[:, :, o + s:o + 2 * s] = b1[sq, w, :s]
            consts[:, :, o + 2 * s:o + 3 * s] = a0[sq, w, :s]
            consts[:, :, o + 3 * s:o + 4 * s] = b0[sq, w, :s]
    stops = np.empty((NCORES, P128, lay["stop_total"]), dtype=np.float32)
    for q in range(NSLOT):
        o = 4 * q * N
        sq = sent[:, q, :]
        stops[:, :, o:o + N] = est[sq][:, :, :, 1, 1]          # SRHAS
        stops[:, :, o + N:o + 2 * N] = est[sq][:, :, :, 0, 1]  # SLHAS
        stops[:, :, o + 2 * N:o + 3 * N] = est[sq][:, :, :, 1, 0]  # SRNO
        stops[:, :, o + 3 * N:o + 4 * N] = est[sq][:, :, :, 0, 0]  # SLNO
    in_maps = [{"consts": consts[c], "stops": stops[c]}
               for c in range(NCORES)]
    aux = dict(c0=c0, order=order, bounds=bounds, sent=sent)
    return in_maps, aux


def assemble(results, len_array, aux):
    ln = np.asarray(len_array).astype(np.int64)
    c0 = np.asarray(aux["c0"]).astype(np.float64)
    sent = aux["sent"]
    out = np.empty(len(ln), dtype=np.float32)
    for c, res in enumerate(results):
        ecr = res["ecr"].reshape(P128, NSLOT * N).astype(np.float64)
        dsum = res["dsum"].reshape(P128, NSLOT).astype(np.float64)
        for q in range(NSLOT):
            idx = sent[c, q]                # original sentence ids [P128]
            lc = ln[idx]
            with np.errstate(divide="ignore"):
                out[idx] = (
                    np.log(ecr[np.arange(P128), q * N + lc])
                    + dsum[:, q] * LN2 * lc + c0[idx] * lc
                ).astype(np.float32)
    return out


def kernel(trans_scores, dec_scores, len_array):
    from concourse.bass_utils import run_bass_kernel_spmd

    in_maps, aux = make_in_maps(trans_scores, dec_scores, len_array)
    nc = get_nc(aux["bounds"])
    res = run_bass_kernel_spmd(nc, in_maps, core_ids=list(range(NCORES)))
    return assemble(res.results, len_array, aux)


# revision 19
# speedup vs baseline: 2.4444x; 1.0493x over previous
"""DMV inside algorithm (Eisner chart DP, logsumexp semiring) on Trainium2.

Strategy
--------
Data parallel over the batch: 4096 sentences -> 8 cores x 512, with
length-aware packing: sentences are sorted by length (desc) and dealt
round-robin to cores, then within a core split into 4 slots of 128
(one sentence per SBUF partition per slot). Slot q only runs chart
widths w <= L[q] (its max length), cutting DP work ~2.4x vs running
all sentences to width 40.

The DP runs in the *exp domain* (tables hold exp(score)); each width-w
update is a fused strided multiply + segmented reduce. Multiplies are
split between VectorE (DVE) and GpSimd (Pool) to use both engines;
segmented reduces only exist on DVE.

Per slot q the 6 diag-packed tables [R=L+1 rows x 41] live at stride
D=41*R in the order [KR, CR, IL, IR, CL, KL], chosen so every fused
operand pair is adjacent: opA in0={KR,CR}, in1={CL,KL}; opB in0={IR,CL},
in1={CR,IL}; outputs {IR,IL}, {KR,KL}, {CR,CL} all constant-stride.
IR/IL row r holds width r+1; IL/opB patterns are column-shifted so all
gathers are constant-stride (as in the classic diagonal packing).

The valence edge case (closest-child GO) is handled by excluding the
edge split (t=0 for right arcs, t=w-1 for left arcs) from the reduce --
the fused AP absorbs the per-slot offset difference into the pair
stride -- and adding edge*{A0,B0} separately. Per-arc constants
{A1,B1,A0,B0} are host-precomputed per width row (triangular pack).

Numerics: scale composes linearly in span width, so slots with L >= 25
renormalize once at w=20: row d of every table is multiplied by an
exact power of two 2^(-k*d) (k integer per sentence), k accumulated in
dsum and undone on the host: LL = log(CR[0,len]) + k*ln2*len + c0*len.
"""

import os

os.environ.setdefault("JAX_PLATFORMS", "cpu")

import numpy as np

import concourse.bass as bass  # noqa: F401  (registers engine classes)
import concourse.tile as tile
import bass_rust
from concourse import bacc, mybir

F32 = mybir.dt.float32
AF = mybir.ActivationFunctionType
OP = mybir.AluOpType
AX = mybir.AxisListType

N = 41              # fake_len (ROOT at 0)
NCORES = 8
NSLOT = 4
P128 = 128
B_CORE = NSLOT * P128
RENORM_W = 20       # renorm width (slots with L >= RENORM_MIN_L)
RENORM_MIN_L = 25

# table order within a slot (pairs used by the fused ops are adjacent)
T_KR, T_CR, T_IL, T_IR, T_CL, T_KL = range(6)

LN2 = float(np.log(2.0))


def ap_of(t, offset, dims, lead=None):
    """Build a raw AP on tile/dram ap `t`: [lead or t.ap[0]] + dims."""
    ap = t.copy()
    first = list(t.ap[0]) if lead is None else list(lead)
    ap.ap = bass_rust.VecI64Pair([first] + [list(d) for d in dims])
    ap.offset = offset
    return ap


def _layout(bounds):
    """Compute per-slot offsets for banks / consts / stops / scratch."""
    L = list(bounds)
    R = [l + 1 for l in L]
    D = [N * r for r in R]
    bank_base, acc = [], 0
    for q in range(NSLOT):
        bank_base.append(acc)
        acc += 6 * D[q]
    bank_total = acc
    # consts, two triangles: tri1 per (q,w) = [A1(s), B1(s)] (renorm-rescaled)
    # and tri2 per (q,w) = [RA(s), RB(s)] (scale-free ratios)
    cbase, coff, acc = [], [], 0
    for q in range(NSLOT):
        cbase.append(acc)
        offs = {}
        for w in range(1, L[q] + 1):
            offs[w] = acc
            acc += 2 * (R[q] - w)
        coff.append(offs)
    cbase.append(acc)
    c2base, c2off = [], []
    for q in range(NSLOT):
        c2base.append(acc)
        offs = {}
        for w in range(1, L[q] + 1):
            offs[w] = acc
            acc += 2 * (R[q] - w)
        c2off.append(offs)
    c2base.append(acc)
    const_total = acc
    stop_total = NSLOT * 4 * N  # per q: [SRHAS, SLHAS, SRNO, SLNO] x 41
    # z scratch
    z = {}
    zacc = 0
    for q in range(NSLOT):
        pmax = max((2 * (R[q] - w) * w for w in range(1, L[q] + 1)),
                   default=2)
        z[("P", q)] = zacc; zacc += pmax
        z[("SS", q)] = zacc; zacc += 2 * N
        z[("T1", q)] = zacc; zacc += 2 * N
        z[("T2", q)] = zacc; zacc += 2 * N
    z["CROUT"] = zacc; zacc += NSLOT * N
    z["DSUM"] = zacc; zacc += NSLOT
    z["M2"] = zacc; zacc += 8
    z["MU"] = zacc; zacc += 2
    z["LM"] = zacc; zacc += 2
    z["M"] = zacc; zacc += NSLOT * (N + 1)  # renorm multiplier rows [q, 42]
    z_total = zacc
    return dict(L=L, R=R, D=D, bank_base=bank_base, bank_total=bank_total,
                cbase=cbase, coff=coff, c2base=c2base, c2off=c2off,
                const_total=const_total,
                stop_total=stop_total, z=z, z_total=z_total)


def build_nc(bounds):
    lay = _layout(bounds)
    L, R, D = lay["L"], lay["R"], lay["D"]
    bb = lay["bank_base"]
    coff = lay["coff"]
    c2off = lay["c2off"]
    zoff = lay["z"]
    renorm_qs = [q for q in range(NSLOT) if L[q] >= RENORM_MIN_L]
    # bounds are sorted desc, so renorm slots are the prefix [0..nq) and
    # dsum[j] lines up with slot j
    assert renorm_qs == list(range(len(renorm_qs)))

    nc = bacc.Bacc("TRN2", target_bir_lowering=False, debug=False,
                   num_devices=1)
    consts_in = nc.dram_tensor(
        "consts", [P128, lay["const_total"]], F32, kind="ExternalInput").ap()
    stops_in = nc.dram_tensor(
        "stops", [P128, lay["stop_total"]], F32, kind="ExternalInput").ap()
    ecr_d = nc.dram_tensor(
        "ecr", [P128, NSLOT * N], F32, kind="ExternalOutput").ap()
    dsum_d = nc.dram_tensor(
        "dsum", [P128, NSLOT], F32, kind="ExternalOutput").ap()

    with tile.TileContext(nc) as tc:
        with tc.tile_pool(name="p", bufs=1) as pool:
            banks_t = pool.tile([P128, lay["bank_total"]], F32)
            consts_t = pool.tile([P128, lay["const_total"]], F32)
            stops_t = pool.tile([P128, lay["stop_total"]], F32)
            z_t = pool.tile([P128, lay["z_total"]], F32)
            banks = banks_t[:]
            consts = consts_t[:]
            stops = stops_t[:]
            z = z_t[:]

            v = nc.vector
            g = nc.gpsimd
            sc = nc.scalar

            # ---- input DMA ----
            nc.sync.dma_start(
                ap_of(stops, 0, [[1, lay["stop_total"]]]),
                ap_of(stops_in, 0, [[1, lay["stop_total"]]],
                      lead=[lay["stop_total"], P128]),
            )
            # consts split at row 6 so the DP can start before the bulk lands
            for q in range(NSLOT):
                for base_list, off_list in ((lay["cbase"], coff),
                                            (lay["c2base"], c2off)):
                    lo = off_list[q][1]
                    mid = off_list[q][7] if L[q] >= 7 else base_list[q + 1]
                    hi = base_list[q + 1]
                    nc.sync.dma_start(
                        ap_of(consts, lo, [[1, mid - lo]]),
                        ap_of(consts_in, lo, [[1, mid - lo]],
                              lead=[lay["const_total"], P128]),
                    )
                    if hi > mid:
                        nc.sync.dma_start(
                            ap_of(consts, mid, [[1, hi - mid]]),
                            ap_of(consts_in, mid, [[1, hi - mid]],
                                  lead=[lay["const_total"], P128]),
                        )

            # ---- init ----
            # zero rows 1..RENORM_W of all 6 tables for renorm slots (the
            # in-place rescale reads full 41-wide rows); staged in 3 chunks
            # so early DP widths don't wait on the full memset.
            for q in renorm_qs:
                for r0, r1 in ((0, 3), (3, 7), (7, RENORM_W + 1)):
                    g.memset(
                        ap_of(banks, bb[q] + r0 * N,
                              [[D[q], 6], [1, (r1 - r0) * N]]), 0.0)
            v.memset(ap_of(z, zoff["DSUM"], [[1, NSLOT]]), 0.0)
            v.memset(ap_of(z, zoff["CROUT"], [[1, NSLOT * N]]), 0.0)
            for q in range(NSLOT):
                # CR[0,i] = exp(stop[i,R,NO]); CL[0,i] = exp(stop[i,L,NO])
                v.tensor_copy(
                    ap_of(banks, bb[q] + T_CR * D[q],
                          [[3 * D[q], 2], [1, N]]),
                    ap_of(stops, (4 * q + 2) * N, [[N, 2], [1, N]]),
                )

            # ---- chart DP ----
            # KR/KL row 0 hold the valence edge RATIOS (RA=a0/a1 at KR[0,i],
            # RB=b0/b1 at KL[0,i+w]), rewritten per width by the otherwise
            # idle ACT engine. opA's natural w-term gather then covers both
            # valence edge cases exactly once IR is scaled by {A1,B1}.
            def emit_ratcopy(w, q):
                s = R[q] - w
                Dq, base = D[q], bb[q]
                sc.activation(
                    ap_of(banks, base + T_KR * Dq, [[5 * Dq + w, 2], [1, s]]),
                    ap_of(consts, c2off[q][w], [[s, 2], [1, s]]),
                    AF.Copy)

            def emit_opA(w, q, eng):
                """P[2,t,i] = {KR[t,i], CR[t,i]}
                            * {CL[w-1-t,i+t+1], KL[w-1-t,i+t+1]}"""
                s = R[q] - w
                Dq, base = D[q], bb[q]
                out = (ap_of(z, zoff[("SS", q)], [[s, 2], [1, 1], [1, s]])
                       if w == 1 else
                       ap_of(z, zoff[("P", q)], [[w * s, 2], [s, w], [1, s]]))
                eng.tensor_tensor(
                    out,
                    ap_of(banks, base + T_KR * Dq, [[Dq, 2], [N, w], [1, s]]),
                    ap_of(banks, base + T_CL * Dq + (w - 1) * N + 1,
                          [[Dq, 2], [-(N - 1), w], [1, s]]),
                    OP.mult)

            def emit_Ared_smalls(w, q):
                s = R[q] - w
                Dq, base = D[q], bb[q]
                irout = ap_of(banks, base + T_IR * Dq + (w - 1) * N,
                              [[-Dq + 1, 2], [1, s]])
                ss = ap_of(z, zoff[("SS", q)], [[s, 2], [1, s]])
                if w > 1:
                    v.reduce_sum(
                        ss,
                        ap_of(z, zoff[("P", q)],
                              [[w * s, 2], [1, s], [s, w]]),
                        axis=AX.X)
                # IR[w-1,i], IL[w-1,i+1] = SS * {A1,B1}
                v.tensor_tensor(
                    irout, ss, ap_of(consts, coff[q][w], [[s, 2], [1, s]]),
                    OP.mult)

            def emit_opB(w, q, eng):
                """P[2,t,i] = {IR[t,i], CL[t,i]}
                            * {CR[w-1-t,i+t+1], IL[w-1-t,i+t+1]}"""
                s = R[q] - w
                Dq, base = D[q], bb[q]
                eng.tensor_tensor(
                    ap_of(z, zoff[("P", q)], [[w * s, 2], [s, w], [1, s]]),
                    ap_of(banks, base + T_IR * Dq, [[Dq, 2], [N, w], [1, s]]),
                    ap_of(banks, base + T_CR * Dq + (w - 1) * N + 1,
                          [[Dq, 2], [-(N - 1), w], [1, s]]),
                    OP.mult)

            def emit_Bred_crcl(w, q):
                s = R[q] - w
                Dq, base = D[q], bb[q]
                krout = ap_of(banks, base + T_KR * Dq + w * N,
                              [[5 * Dq, 2], [1, s]])
                if w == 1:
                    v.tensor_tensor(
                        krout,
                        ap_of(banks, base + T_IR * Dq, [[Dq, 2], [1, s]]),
                        ap_of(banks, base + T_CR * Dq + 1,
                              [[Dq, 2], [1, s]]),
                        OP.mult)
                else:
                    v.reduce_sum(
                        krout,
                        ap_of(z, zoff[("P", q)],
                              [[w * s, 2], [1, s], [s, w]]),
                        axis=AX.X)
                # {CR[w,i], CL[w,i]} = {KR[w,i]*sRhas[i], KL[w,i]*sLhas[i+w]}
                v.tensor_tensor(
                    ap_of(banks, base + T_CR * Dq + w * N,
                          [[3 * Dq, 2], [1, s]]),
                    ap_of(banks, base + T_KR * Dq + w * N,
                          [[5 * Dq, 2], [1, s]]),
                    ap_of(stops, 4 * q * N, [[N + w, 2], [1, s]]),
                    OP.mult)

            def emit_chain(w, q, eng):
                """Full width-w update for one slot on one mult engine."""
                emit_opA(w, q, eng)
                if w < L[q]:
                    emit_ratcopy(w + 1, q)
                emit_Ared_smalls(w, q)
                if w >= 2:
                    emit_opB(w, q, eng)
                emit_Bred_crcl(w, q)

            def emit_renorm():
                # per renorm slot: k = round(log2(max KR/KL at w) / w);
                # rescale row d by exact 2^(-k*d); accumulate k into dsum.
                w = RENORM_W
                nq = len(renorm_qs)
                if not nq:
                    return
                for j, q in enumerate(renorm_qs):
                    s = R[q] - w
                    v.reduce_max(
                        ap_of(z, zoff["M2"] + 2 * j, [[1, 2]]),
                        ap_of(banks, bb[q] + T_KR * D[q] + w * N,
                              [[5 * D[q], 2], [1, s]]),
                        axis=AX.X)
                # mu[j] = max over the {KR,KL} pair
                v.tensor_tensor(
                    ap_of(z, zoff["MU"], [[1, nq]]),
                    ap_of(z, zoff["M2"], [[2, nq]]),
                    ap_of(z, zoff["M2"] + 1, [[2, nq]]),
                    OP.max)
                # Ln range on ACT is +-2^64: compute via mu*2^-32
                v.tensor_scalar_mul(
                    ap_of(z, zoff["MU"], [[1, nq]]),
                    ap_of(z, zoff["MU"], [[1, nq]]), 2.0 ** -32)
                v.tensor_scalar_max(
                    ap_of(z, zoff["MU"], [[1, nq]]),
                    ap_of(z, zoff["MU"], [[1, nq]]), 1e-36)
                sc.activation(
                    ap_of(z, zoff["LM"], [[1, nq]]),
                    ap_of(z, zoff["MU"], [[1, nq]]), AF.Ln)
                # k = round((ln(mu*2^-32) + 32 ln2)/(w ln2)); round via the
                # 1.5*2^23 trick so every factor is an exact power of two
                v.tensor_scalar(
                    ap_of(z, zoff["LM"], [[1, nq]]),
                    ap_of(z, zoff["LM"], [[1, nq]]),
                    32.0 * LN2, 1.0 / (w * LN2), OP.add, OP.mult)
                v.tensor_scalar(
                    ap_of(z, zoff["LM"], [[1, nq]]),
                    ap_of(z, zoff["LM"], [[1, nq]]),
                    12582912.0, 12582912.0, OP.add, OP.subtract)
                # dsum[q] += k  (host undoes k*ln2*len)
                v.tensor_tensor(
                    ap_of(z, zoff["DSUM"], [[1, nq]]),
                    ap_of(z, zoff["DSUM"], [[1, nq]]),
                    ap_of(z, zoff["LM"], [[1, nq]]),
                    OP.add)
                # scale2 = 2^-k via exponent bits: (127 - k) << 23
                v.tensor_scalar(
                    ap_of(z, zoff["M2"], [[1, nq]]),
                    ap_of(z, zoff["LM"], [[1, nq]]),
                    -1.0, 127.0, OP.mult, OP.add)
                zi = z.bitcast(mybir.dt.int32)
                v.tensor_copy(
                    ap_of(zi, zoff["M2"] + 4, [[1, nq]]),
                    ap_of(z, zoff["M2"], [[1, nq]]))
                v.tensor_scalar(
                    ap_of(zi, zoff["M2"] + 4, [[1, nq]]),
                    ap_of(zi, zoff["M2"] + 4, [[1, nq]]),
                    23, None, OP.arith_shift_left)
                # M[j, d] = 2^(-k*d): d=0 -> 1, then multiplicative scan
                v.memset(ap_of(z, zoff["M"], [[N + 1, nq], [1, 1]]), 1.0)
                for j in range(nq):
                    sca = ap_of(z, zoff["M2"] + 4 + j, [[0, N]])
                    v.tensor_tensor_scan(
                        ap_of(z, zoff["M"] + j * (N + 1) + 1, [[1, N]]),
                        sca, sca, 1.0, OP.mult, OP.bypass)
                for j, q in enumerate(renorm_qs):
                    Dq, base = D[q], bb[q]
                    mrow = zoff["M"] + j * (N + 1)
                    eng = g if q == 0 else v
                    # natural tables rows d<=w: *2^(-k*d)
                    for tb in (T_KR, T_CL):
                        eng.tensor_tensor(
                            ap_of(banks, base + tb * Dq,
                                  [[Dq, 2], [N, w + 1], [1, N]]),
                            ap_of(banks, base + tb * Dq,
                                  [[Dq, 2], [N, w + 1], [1, N]]),
                            ap_of(z, mrow, [[0, 2], [1, w + 1], [0, N]]),
                            OP.mult)
                    # IR/IL rows r<=w-1 hold width r+1: *2^(-k*(r+1))
                    eng.tensor_tensor(
                        ap_of(banks, base + T_IL * Dq,
                              [[Dq, 2], [N, w], [1, N]]),
                        ap_of(banks, base + T_IL * Dq,
                              [[Dq, 2], [N, w], [1, N]]),
                        ap_of(z, mrow + 1, [[0, 2], [1, w], [0, N]]),
                        OP.mult)
                    # const rows > w: one extra arc factor 2^(-k); the RA/RB
                    # ratio triangle is scale-free and must stay untouched
                    lo = coff[q][w + 1]
                    hi = lay["cbase"][q + 1]
                    v.tensor_tensor(
                        ap_of(consts, lo, [[1, hi - lo]]),
                        ap_of(consts, lo, [[1, hi - lo]]),
                        ap_of(z, mrow + 1, [[0, hi - lo]]),
                        OP.mult)

            for q in range(NSLOT):
                emit_ratcopy(1, q)

            # Pipelined schedule per width: Pool takes q0 and q1-opB mults;
            # DVE does the rest, ordered so each engine's in-order stream has
            # independent work to chew on while the other engine's result for
            # the same width is still in flight.
            for w in range(1, max(L) + 1):
                a0 = w <= L[0]
                a1 = w <= L[1]
                if a1:
                    emit_opA(w, 1, v)
                if a0:
                    emit_opA(w, 0, g)
                    if w < L[0]:
                        emit_ratcopy(w + 1, 0)
                if a1 and w < L[1]:
                    emit_ratcopy(w + 1, 1)
                if w <= L[2]:
                    emit_chain(w, 2, v)
                if a1:
                    emit_Ared_smalls(w, 1)
                    if w >= 2:
                        emit_opB(w, 1, g)
                if a0:
                    emit_Ared_smalls(w, 0)
                    if w >= 2:
                        emit_opB(w, 0, g)
                if w <= L[3]:
                    emit_chain(w, 3, v)
                if a1:
                    emit_Bred_crcl(w, 1)
                if a0:
                    emit_Bred_crcl(w, 0)
                for q in range(NSLOT):
                    if w == L[q] + 1 or (w == max(L) and L[q] == max(L)):
                        # extract exp CR[0, 0..L] (stride-41 diag gather)
                        v.tensor_copy(
                            ap_of(z, zoff["CROUT"] + q * N, [[1, R[q]]]),
                            ap_of(banks, bb[q] + T_CR * D[q], [[N, R[q]]]))
                if w == RENORM_W:
                    emit_renorm()

            # ---- outputs ----
            nc.sync.dma_start(
                ap_of(ecr_d, 0, [[1, NSLOT * N]], lead=[NSLOT * N, P128]),
                ap_of(z, zoff["CROUT"], [[1, NSLOT * N]]),
            )
            nc.sync.dma_start(
                ap_of(dsum_d, 0, [[1, NSLOT]], lead=[NSLOT, P128]),
                ap_of(z, zoff["DSUM"], [[1, NSLOT]]),
            )

    nc.compile()
    return nc


_NC_CACHE = {}


def get_nc(bounds):
    key = tuple(bounds)
    if key not in _NC_CACHE:
        _NC_CACHE[key] = build_nc(key)
    return _NC_CACHE[key]


def plan(len_array):
    """Sort sentences by length desc, deal round-robin to cores, slot into
    4 groups of 128 per core. Returns (order, bounds) where order[r] is the
    original sentence index of global sorted rank r and bounds[q] is the
    width bound of slot q (same for every core by round-robin construction).
    """
    ln = np.asarray(len_array).astype(np.int64)
    order = np.argsort(-ln, kind="stable")
    bounds = [int(ln[order[min(1024 * q, len(ln) - 1)]])
              for q in range(NSLOT)]
    bounds = [max(b, 1) for b in bounds]
    return order, bounds


def make_in_maps(trans_scores, dec_scores, len_array):
    t = np.asarray(trans_scores, dtype=np.float32)
    dec = np.asarray(dec_scores, dtype=np.float32)
    B = t.shape[0]
    order, bounds = plan(len_array)
    lay = _layout(bounds)
    go = dec[..., 0]                        # [B, n, dir, dv]
    # per-sentence linear pre-shift: each arc factor carries exp(-c0), so a
    # width-w entry is scaled exp(-c0*w); undone on the host at the end.
    tm = np.where(t < -1e8, -np.inf, t).max(axis=3)
    with np.errstate(invalid="ignore"):
        colmax = tm.max(axis=1)             # [B, n] best arc into each child
        proxy = np.nanmean(
            np.where(np.isfinite(colmax), colmax, np.nan)[:, 1:], axis=-1)
    c0 = (proxy + 0.5).astype(np.float32)
    c0 = np.clip(np.nan_to_num(c0), -20.0, 20.0)
    # one exp over trans (NEG -> 0 underflow is intended), then gather diags
    with np.errstate(under="ignore"):
        E = np.exp(t - c0[:, None, None, None])      # [B, n, n, 2]
        ego = np.exp(go)                             # [B, n, 2, 2]
    d_idx, i_idx = np.meshgrid(np.arange(N), np.arange(N), indexing="ij")
    j_idx = np.minimum(i_idx + d_idx, N - 1)
    valid = ((i_idx + d_idx) <= N - 1)[None].astype(np.float32)
    ea = E[:, i_idx, j_idx, :]              # [B, n, n, 2]  trans[i, i+d, v]
    eb = E[:, j_idx, i_idx, :]              # [B, n, n, 2]  trans[i+d, i, v]
    a1 = ea[..., 1] * ego[:, :, 1, 1][:, i_idx] * valid   # [B, d, i]
    a0 = ea[..., 0] * ego[:, :, 1, 0][:, i_idx] * valid
    b1 = eb[..., 1] * ego[:, :, 0, 1][:, j_idx] * valid
    b0 = eb[..., 0] * ego[:, :, 0, 0][:, j_idx] * valid
    est = np.exp(dec[..., 1])               # [B, n, dir, dv]

    # sentence index per (core, slot, partition)
    sent = order.reshape(-1, NCORES).T.reshape(NCORES, NSLOT, P128)

    # valence-edge ratios (every packed cell is a valid arc, so a1,b1 > 0)
    with np.errstate(divide="ignore", invalid="ignore"):
        ra = (a0.astype(np.float64) / a1).astype(np.float32)
        rb = (b0.astype(np.float64) / b1).astype(np.float32)
    consts = np.empty((NCORES, P128, lay["const_total"]), dtype=np.float32)
    for q in range(NSLOT):
        sq = sent[:, q, :]                  # [NCORES, P128]
        for w in range(1, bounds[q] + 1):
            s = bounds[q] + 1 - w
            o = lay["coff"][q][w]
            consts[:, :, o:o + s] = a1[sq, w, :s]
            consts[:, :, o + s:o + 2 * s] = b1[sq, w, :s]
            o2 = lay["c2off"][q][w]
            consts[:, :, o2:o2 + s] = ra[sq, w, :s]
            consts[:, :, o2 + s:o2 + 2 * s] = rb[sq, w, :s]
    stops = np.empty((NCORES, P128, lay["stop_total"]), dtype=np.float32)
    for q in range(NSLOT):
        o = 4 * q * N
        sq = sent[:, q, :]
        stops[:, :, o:o + N] = est[sq][:, :, :, 1, 1]          # SRHAS
        stops[:, :, o + N:o + 2 * N] = est[sq][:, :, :, 0, 1]  # SLHAS
        stops[:, :, o + 2 * N:o + 3 * N] = est[sq][:, :, :, 1, 0]  # SRNO
        stops[:, :, o + 3 * N:o + 4 * N] = est[sq][:, :, :, 0, 0]  # SLNO
    in_maps = [{"consts": consts[c], "stops": stops[c]}
               for c in range(NCORES)]
    aux = dict(c0=c0, order=order, bounds=bounds, sent=sent)
    return in_maps, aux


def assemble(results, len_array, aux):
    ln = np.asarray(len_array).astype(np.int64)
    c0 = np.asarray(aux["c0"]).astype(np.float64)
    sent = aux["sent"]
    out = np.empty(len(ln), dtype=np.float32)
    for c, res in enumerate(results):
        ecr = res["ecr"].reshape(P128, NSLOT * N).astype(np.float64)
        dsum = res["dsum"].reshape(P128, NSLOT).astype(np.float64)
        for q in range(NSLOT):
            idx = sent[c, q]                # original sentence ids [P128]
            lc = ln[idx]
            with np.errstate(divide="ignore"):
                out[idx] = (
                    np.log(ecr[np.arange(P128), q * N + lc])
                    + dsum[:, q] * LN2 * lc + c0[idx] * lc
                ).astype(np.float32)
    return out


def kernel(trans_scores, dec_scores, len_array):
    from concourse.bass_utils import run_bass_kernel_spmd

    in_maps, aux = make_in_maps(trans_scores, dec_scores, len_array)
    nc = get_nc(aux["bounds"])
    res = run_bass_kernel_spmd(nc, in_maps, core_ids=list(range(NCORES)))
    return assemble(res.results, len_array, aux)


# revision 33
# speedup vs baseline: 2.9485x; 1.2063x over previous
"""DMV inside algorithm (Eisner chart DP, logsumexp semiring) on Trainium2.

Strategy
--------
Data parallel over the batch: 4096 sentences -> 8 cores x 512, with
length-aware packing: sentences are sorted by length (desc) and dealt
round-robin to cores, then within a core split into 4 slots of 128
(one sentence per SBUF partition per slot). Slot q only runs chart
widths w <= L[q] (its max length), cutting DP work ~2.4x vs running
all sentences to width 40.

The DP runs in the *exp domain* (tables hold exp(score)); each width-w
update is a fused strided multiply + segmented reduce. Segmented
reduces only exist on VectorE (DVE); multiplies are split between DVE
and GpSimd (Pool); the ACT engine rewrites the valence-ratio rows.
All ops are built as an explicit dependency graph and ordered by a
critical-path list scheduler before emission, because every engine
executes its stream strictly in order (a semaphore wait at the head
blocks everything behind it).

Per slot q the 6 diag-packed tables [R=L+1 rows x 41] live at stride
D=41*R in the order [KR, CR, IL, IR, CL, KL], chosen so every fused
operand pair is adjacent: opA in0={KR,CR}, in1={CL,KL}; opB in0={IR,CL},
in1={CR,IL}; outputs {IR,IL}, {KR,KL}, {CR,CL} all constant-stride.
IR/IL row r holds width r+1; IL/opB patterns are column-shifted so all
gathers are constant-stride.

KR/KL row 0 hold the valence edge RATIOS (RA=a0/a1 at KR[0,i],
RB=b0/b1 at KL[0,i+w]), rewritten per width by the otherwise idle ACT
engine; opA's natural w-term gather then covers both valence edge
cases exactly, once IR/IL is scaled by {A1,B1}. Per-arc constants live
in two triangular packs: tri1={A1,B1} (renorm-rescaled), tri2={RA,RB}
(scale-free).

Numerics: scale composes linearly in span width, so slots with L >= 25
renormalize once at w=20: row d of every table is multiplied by an
exact power of two 2^(-k*d) (k integer per sentence), k accumulated in
dsum and undone on the host: LL = log(CR[0,len]) + k*ln2*len + c0*len.
"""

import os

os.environ.setdefault("JAX_PLATFORMS", "cpu")

import heapq

import numpy as np

import concourse.bass as bass  # noqa: F401  (registers engine classes)
import concourse.tile as tile
import bass_rust
from concourse import bacc, mybir

F32 = mybir.dt.float32
AF = mybir.ActivationFunctionType
OP = mybir.AluOpType
AX = mybir.AxisListType

N = 41              # fake_len (ROOT at 0)
NCORES = 8
NSLOT = 4
P128 = 128
B_CORE = NSLOT * P128
RENORM_W = 20       # renorm width (slots with L >= RENORM_MIN_L)
RENORM_MIN_L = 25

# table order within a slot (pairs used by the fused ops are adjacent)
T_KR, T_CR, T_IL, T_IR, T_CL, T_KL = range(6)

LN2 = float(np.log(2.0))

# mult-engine assignment per slot: (opA engine, opB engine), 'v'=DVE 'g'=Pool
MULT_CFG = {0: ("v", "v"), 1: ("v", "g"), 2: ("v", "v"), 3: ("v", "v")}
# slots using the edge/main decomposition: the big mults' t∈[1,w-1) bulk
# only depends on width w-2, so it runs a width ahead on Pool while the
# dependency-carrying edge terms (t=0, t=w-1) are tiny [2,2,s] DVE ops.
EDGE_QS = (0,)


def ap_of(t, offset, dims, lead=None):
    """Build a raw AP on tile/dram ap `t`: [lead or t.ap[0]] + dims."""
    ap = t.copy()
    first = list(t.ap[0]) if lead is None else list(lead)
    ap.ap = bass_rust.VecI64Pair([first] + [list(d) for d in dims])
    ap.offset = offset
    return ap


def _layout(bounds):
    """Compute per-slot offsets for banks / consts / stops / scratch."""
    L = list(bounds)
    R = [l + 1 for l in L]
    D = [N * r for r in R]
    bank_base, acc = [], 0
    for q in range(NSLOT):
        bank_base.append(acc)
        acc += 6 * D[q]
    bank_total = acc
    # consts, two triangles: tri1 per (q,w) = [A1(s), B1(s)] (renorm-rescaled)
    # and tri2 per (q,w) = [RA(s), RB(s)] (scale-free ratios)
    cbase, coff, acc = [], [], 0
    for q in range(NSLOT):
        cbase.append(acc)
        offs = {}
        for w in range(1, L[q] + 1):
            offs[w] = acc
            acc += 2 * (R[q] - w)
        coff.append(offs)
    cbase.append(acc)
    c2base, c2off = [], []
    for q in range(NSLOT):
        c2base.append(acc)
        offs = {}
        for w in range(1, L[q] + 1):
            offs[w] = acc
            acc += 2 * (R[q] - w)
        c2off.append(offs)
    c2base.append(acc)
    const_total = acc
    stop_total = NSLOT * 4 * N  # per q: [SRHAS, SLHAS, SRNO, SLNO] x 41
    # z scratch
    z = {}
    zacc = 0
    for q in range(NSLOT):
        pmax = max((2 * (R[q] - w) * w for w in range(1, L[q] + 1)),
                   default=2)
        for pb in ("PA0", "PA1", "PB0", "PB1"):
            z[(pb, q)] = zacc; zacc += pmax
        z[("SS", q)] = zacc; zacc += 2 * N
        z[("T1", q)] = zacc; zacc += 2 * N
    z["CROUT"] = zacc; zacc += NSLOT * N
    z["DSUM"] = zacc; zacc += NSLOT
    z["M2"] = zacc; zacc += 8
    z["MU"] = zacc; zacc += 2
    z["LM"] = zacc; zacc += 2
    z["M"] = zacc; zacc += NSLOT * (N + 1)  # renorm multiplier rows [q, 42]
    z_total = zacc
    return dict(L=L, R=R, D=D, bank_base=bank_base, bank_total=bank_total,
                cbase=cbase, coff=coff, c2base=c2base, c2off=c2off,
                const_total=const_total,
                stop_total=stop_total, z=z, z_total=z_total)


class Graph:
    """Op graph + critical-path list scheduler for in-order engines."""

    EST = {"v": (1.0417, 107.0), "g": (1.984, 156.0), "a": (0.833, 217.0),
           "d": (1.544, 2500.0)}  # (ns/elem, fixed ns); d: ns/byte-ish
    XPEN = 80.0  # cross-engine semaphore latency

    def __init__(self):
        self.nodes = []          # dict(key, eng, est, deps, fn)
        self.byname = {}

    def add(self, key, eng, elems, deps, fn, fixed=None):
        slope, fix = self.EST[eng]
        est = elems * slope + (fix if fixed is None else fixed)
        n = dict(key=key, eng=eng, est=est, deps=[d for d in deps
                                                 if d is not None], fn=fn,
                 idx=len(self.nodes))
        self.nodes.append(n)
        self.byname[key] = n
        return key

    def schedule(self):
        """Earliest-start list scheduling with critical-path tie-break.
        Returns node indices in chosen global emission order."""
        nodes = self.nodes
        nn = len(nodes)
        succ = [[] for _ in range(nn)]
        npred = [0] * nn
        for n in nodes:
            for dk in n["deps"]:
                d = self.byname[dk]
                succ[d["idx"]].append(n["idx"])
                npred[n["idx"]] += 1
        # critical-path priority (longest path to sink)
        prio = [0.0] * nn
        for i in reversed(range(nn)):  # nodes added roughly topologically;
            pass
        order_topo = []
        tmp_pred = npred[:]
        stack = [i for i in range(nn) if tmp_pred[i] == 0]
        while stack:
            i = stack.pop()
            order_topo.append(i)
            for j in succ[i]:
                tmp_pred[j] -= 1
                if tmp_pred[j] == 0:
                    stack.append(j)
        assert len(order_topo) == nn, "cycle in op graph"
        for i in reversed(order_topo):
            best = 0.0
            for j in succ[i]:
                if prio[j] > best:
                    best = prio[j]
            prio[i] = nodes[i]["est"] + best
        # event-driven greedy
        free = {"v": 0.0, "g": 0.0, "a": 0.0, "d": 0.0}
        finish = [0.0] * nn
        ready_t = [0.0] * nn
        npred2 = npred[:]
        ready = [i for i in range(nn) if npred2[i] == 0]
        out = []
        while ready:
            # candidate est-start for each ready node
            best_i, best_start, best_prio = None, None, None
            min_start = min(max(free[nodes[i]["eng"]]
                                if nodes[i]["eng"] != "d" else 0.0,
                                ready_t[i]) for i in ready)
            for i in ready:
                e = nodes[i]["eng"]
                st = max(free[e] if e != "d" else 0.0, ready_t[i])
                if st <= min_start + 100.0:
                    if best_prio is None or prio[i] > best_prio:
                        best_i, best_start, best_prio = i, st, prio[i]
            i = best_i
            ready.remove(i)
            n = nodes[i]
            e = n["eng"]
            st = best_start
            fin = st + n["est"]
            finish[i] = fin
            if e != "d":
                free[e] = fin
            out.append((st, i))
            for j in succ[i]:
                pen = self.XPEN if nodes[j]["eng"] != e else 0.0
                ready_t[j] = max(ready_t[j], fin + pen)
                npred2[j] -= 1
                if npred2[j] == 0:
                    ready.append(j)
        assert len(out) == nn
        self.makespan = max(finish) if nn else 0.0
        out.sort(key=lambda t: (t[0], t[1]))
        return [i for _, i in out]

    def emit(self):
        order = self.schedule()
        global LAST_MAKESPAN
        LAST_MAKESPAN = self.makespan
        for i in order:
            self.nodes[i]["fn"]()


LAST_MAKESPAN = None


def build_nc(bounds):
    lay = _layout(bounds)
    L, R, D = lay["L"], lay["R"], lay["D"]
    bb = lay["bank_base"]
    coff = lay["coff"]
    c2off = lay["c2off"]
    zoff = lay["z"]
    renorm_qs = [q for q in range(NSLOT) if L[q] >= RENORM_MIN_L]
    # bounds are sorted desc, so renorm slots are the prefix [0..nq) and
    # dsum[j] lines up with slot j
    assert renorm_qs == list(range(len(renorm_qs)))
    nq = len(renorm_qs)

    nc = bacc.Bacc("TRN2", target_bir_lowering=False, debug=False,
                   num_devices=1)
    consts_in = nc.dram_tensor(
        "consts", [P128, lay["const_total"]], F32, kind="ExternalInput").ap()
    stops_in = nc.dram_tensor(
        "stops", [P128, lay["stop_total"]], F32, kind="ExternalInput").ap()
    ecr_d = nc.dram_tensor(
        "ecr", [P128, NSLOT * N], F32, kind="ExternalOutput").ap()
    dsum_d = nc.dram_tensor(
        "dsum", [P128, NSLOT], F32, kind="ExternalOutput").ap()

    with tile.TileContext(nc) as tc:
        with tc.tile_pool(name="p", bufs=1) as pool:
            banks_t = pool.tile([P128, lay["bank_total"]], F32)
            consts_t = pool.tile([P128, lay["const_total"]], F32)
            stops_t = pool.tile([P128, lay["stop_total"]], F32)
            z_t = pool.tile([P128, lay["z_total"]], F32)
            banks = banks_t[:]
            consts = consts_t[:]
            stops = stops_t[:]
            z = z_t[:]

            v = nc.vector
            g = nc.gpsimd
            sc = nc.scalar
            eng_of = {"v": v, "g": g}
            G = Graph()

            # ---------------- input DMA nodes ----------------
            def dma_in(dst_off, src_off, size):
                def fn():
                    nc.sync.dma_start(
                        ap_of(consts, dst_off, [[1, size]]),
                        ap_of(consts_in, src_off, [[1, size]],
                              lead=[lay["const_total"], P128]))
                return fn

            G.add("dma_stops", "d", lay["stop_total"] * 4, [], lambda: (
                nc.sync.dma_start(
                    ap_of(stops, 0, [[1, lay["stop_total"]]]),
                    ap_of(stops_in, 0, [[1, lay["stop_total"]]],
                          lead=[lay["stop_total"], P128]))))
            dma_dep = {}   # (tri, q, part) -> key; part 0: rows<7, 1: rest
            for q in range(NSLOT):
                for tri, (bases, offs) in enumerate(
                        ((lay["cbase"], coff), (lay["c2base"], c2off))):
                    lo = offs[q][1]
                    mid = offs[q][7] if L[q] >= 7 else bases[q + 1]
                    hi = bases[q + 1]
                    dma_dep[(tri, q, 0)] = G.add(
                        f"dmac{tri}_{q}_0", "d", (mid - lo) * 4, [],
                        dma_in(lo, lo, mid - lo))
                    if hi > mid:
                        dma_dep[(tri, q, 1)] = G.add(
                            f"dmac{tri}_{q}_1", "d", (hi - mid) * 4, [],
                            dma_in(mid, mid, hi - mid))
                    else:
                        dma_dep[(tri, q, 1)] = dma_dep[(tri, q, 0)]

            def c1dep(q, w):
                return dma_dep[(0, q, 0 if w < 7 else 1)]

            def c2dep(q, w):
                return dma_dep[(1, q, 0 if w < 7 else 1)]

            # ---------------- init nodes ----------------
            ms_keys = {q: [] for q in range(NSLOT)}
            for q in renorm_qs:
                for ci, (r0, r1) in enumerate(
                        ((0, 3), (3, 7), (7, RENORM_W + 1))):
                    def mfn(q=q, r0=r0, r1=r1):
                        g.memset(
                            ap_of(banks, bb[q] + r0 * N,
                                  [[D[q], 6], [1, (r1 - r0) * N]]), 0.0)
                    ms_keys[q].append(G.add(
                        f"ms{q}_{ci}", "g", 6 * (r1 - r0) * N, [], mfn))
            G.add("ms_dsum", "v", NSLOT, [], lambda: v.memset(
                ap_of(z, zoff["DSUM"], [[1, NSLOT]]), 0.0))
            G.add("ms_crout", "v", NSLOT * N, [], lambda: v.memset(
                ap_of(z, zoff["CROUT"], [[1, NSLOT * N]]), 0.0))
            init_k = {}
            for q in range(NSLOT):
                def ifn(q=q):
                    v.tensor_copy(
                        ap_of(banks, bb[q] + T_CR * D[q],
                              [[3 * D[q], 2], [1, N]]),
                        ap_of(stops, (4 * q + 2) * N, [[N, 2], [1, N]]))
                init_k[q] = G.add(f"init{q}", "v", 2 * N,
                                  ["dma_stops"] + ms_keys[q][:1], ifn)

            # ---------------- DP op builders ----------------
            def fn_ratcopy(w, q):
                s = R[q] - w
                Dq, base = D[q], bb[q]
                def fn():
                    sc.activation(
                        ap_of(banks, base + T_KR * Dq,
                              [[5 * Dq + w, 2], [1, s]]),
                        ap_of(consts, c2off[q][w], [[s, 2], [1, s]]),
                        AF.Copy)
                return fn

            def fn_opA(w, q, e, pb, t0=0, t1=None):
                """t-slice [t0, t1) of the opA product into buffer pb."""
                s = R[q] - w
                Dq, base = D[q], bb[q]
                tn = (w if t1 is None else t1) - t0
                def fn():
                    out = (ap_of(z, zoff[("SS", q)],
                                 [[s, 2], [1, 1], [1, s]])
                           if w == 1 else
                           ap_of(z, zoff[(pb, q)] + t0 * s,
                                 [[w * s, 2], [s, tn], [1, s]]))
                    eng_of[e].tensor_tensor(
                        out,
                        ap_of(banks, base + T_KR * Dq + t0 * N,
                              [[Dq, 2], [N, tn], [1, s]]),
                        ap_of(banks,
                              base + T_CL * Dq + (w - 1 - t0) * N + 1 + t0,
                              [[Dq, 2], [-(N - 1), tn], [1, s]]),
                        OP.mult)
                return fn

            def fn_edge(w, q, tbl0, pb):
                """Fused edge mult: t in {0, w-1} of opA (tbl0=T_KR) or opB
                (tbl0=T_IR) into buffer pb."""
                s = R[q] - w
                Dq, base = D[q], bb[q]
                in1base = (T_CL if tbl0 == T_KR else T_CR)
                def fn():
                    v.tensor_tensor(
                        ap_of(z, zoff[(pb, q)],
                              [[w * s, 2], [(w - 1) * s, 2], [1, s]]),
                        ap_of(banks, base + tbl0 * Dq,
                              [[Dq, 2], [(w - 1) * N, 2], [1, s]]),
                        ap_of(banks, base + in1base * Dq + (w - 1) * N + 1,
                              [[Dq, 2], [-(w - 1) * (N - 1), 2], [1, s]]),
                        OP.mult)
                return fn

            def fn_Ared(w, q, pb):
                s = R[q] - w
                def fn():
                    v.reduce_sum(
                        ap_of(z, zoff[("SS", q)], [[s, 2], [1, s]]),
                        ap_of(z, zoff[(pb, q)],
                              [[w * s, 2], [1, s], [s, w]]),
                        axis=AX.X)
                return fn

            def fn_T1(w, q):
                s = R[q] - w
                Dq, base = D[q], bb[q]
                def fn():
                    v.tensor_tensor(
                        ap_of(banks, base + T_IR * Dq + (w - 1) * N,
                              [[-Dq + 1, 2], [1, s]]),
                        ap_of(z, zoff[("SS", q)], [[s, 2], [1, s]]),
                        ap_of(consts, coff[q][w], [[s, 2], [1, s]]),
                        OP.mult)
                return fn

            def fn_opB(w, q, e, pb, t0=0, t1=None):
                s = R[q] - w
                Dq, base = D[q], bb[q]
                tn = (w if t1 is None else t1) - t0
                def fn():
                    eng_of[e].tensor_tensor(
                        ap_of(z, zoff[(pb, q)] + t0 * s,
                              [[w * s, 2], [s, tn], [1, s]]),
                        ap_of(banks, base + T_IR * Dq + t0 * N,
                              [[Dq, 2], [N, tn], [1, s]]),
                        ap_of(banks,
                              base + T_CR * Dq + (w - 1 - t0) * N + 1 + t0,
                              [[Dq, 2], [-(N - 1), tn], [1, s]]),
                        OP.mult)
                return fn

            def fn_Bred(w, q, pb):
                s = R[q] - w
                Dq, base = D[q], bb[q]
                def fn():
                    krout = ap_of(banks, base + T_KR * Dq + w * N,
                                  [[5 * Dq, 2], [1, s]])
                    if w == 1:
                        v.tensor_tensor(
                            krout,
                            ap_of(banks, base + T_IR * Dq,
                                  [[Dq, 2], [1, s]]),
                            ap_of(banks, base + T_CR * Dq + 1,
                                  [[Dq, 2], [1, s]]),
                            OP.mult)
                    else:
                        v.reduce_sum(
                            krout,
                            ap_of(z, zoff[(pb, q)],
                                  [[w * s, 2], [1, s], [s, w]]),
                            axis=AX.X)
                return fn

            def fn_crcl(w, q):
                s = R[q] - w
                Dq, base = D[q], bb[q]
                def fn():
                    v.tensor_tensor(
                        ap_of(banks, base + T_CR * Dq + w * N,
                              [[3 * Dq, 2], [1, s]]),
                        ap_of(banks, base + T_KR * Dq + w * N,
                              [[5 * Dq, 2], [1, s]]),
                        ap_of(stops, 4 * q * N, [[N + w, 2], [1, s]]),
                        OP.mult)
                return fn

            # ---------------- DP graph ----------------
            for q in range(NSLOT):
                G.add(f"RC{q}_1", "a", 2 * (R[q] - 1),
                      [c2dep(q, 1), ms_keys[q][0] if ms_keys[q] else None],
                      fn_ratcopy(1, q))
            for q in range(NSLOT):
                prevC, prev2C = None, None
                kT1prev = None
                for w in range(1, L[q] + 1):
                    s = R[q] - w
                    ea, ebn = MULT_CFG[q]
                    edged = (q in EDGE_QS and w >= 2)
                    pa = ("PA1" if w % 2 else "PA0") if edged else "PA0"
                    pbuf = ("PB1" if w % 2 else "PB0") if edged else "PA0"
                    rsb = (f"RSB{q}" if q in renorm_qs
                           and RENORM_W + 1 <= w <= RENORM_W + 2 else None)
                    adeps = [f"RC{q}_{w}", prevC, rsb]
                    if w == 1:
                        adeps += [init_k[q]]
                        if ms_keys[q]:
                            adeps += [ms_keys[q][0]]
                    if edged:
                        kAs = [G.add(f"AE{q}_{w}", "v", 4 * s, adeps,
                                     fn_edge(w, q, T_KR, pa))]
                        if w >= 3:
                            kAs.append(G.add(
                                f"A{q}_{w}", "g", 2 * (w - 2) * s,
                                [prev2C, rsb],
                                fn_opA(w, q, "g", pa, 1, w - 1)))
                        rdr = kAs[0]
                    else:
                        kAs = [G.add(f"A{q}_{w}", ea, 2 * w * s, adeps,
                                     fn_opA(w, q, ea, pa))]
                        rdr = kAs[0]
                    if w < L[q]:
                        # WAR: only the edge (or whole) op reads RAT row 0
                        G.add(f"RC{q}_{w + 1}", "a", 2 * (s - 1),
                              [rdr, c2dep(q, w + 1)], fn_ratcopy(w + 1, q))
                    if w > 1:
                        kR1 = G.add(f"R1{q}_{w}", "v", 2 * w * s,
                                    kAs + [kT1prev], fn_Ared(w, q, pa))
                    else:
                        kR1 = kAs[0]
                    t1deps = [kR1, c1dep(q, w)]
                    if q in renorm_qs and w == RENORM_W + 1:
                        t1deps += [f"RSC{q}"]
                    kT1 = G.add(f"T1{q}_{w}", "v", 2 * s, t1deps,
                                fn_T1(w, q))
                    # staged memsets zero bank rows [3,7) and [7,21); the
                    # first DP write into each zone must wait for its chunk
                    r2deps = []
                    if ms_keys[q]:
                        if w == 3:
                            r2deps.append(ms_keys[q][1])
                        elif w == 7:
                            r2deps.append(ms_keys[q][2])
                    if w > 1:
                        if edged:
                            kBs = [G.add(f"BE{q}_{w}", "v", 4 * s,
                                         [kT1, prevC, rsb],
                                         fn_edge(w, q, T_IR, pbuf))]
                            if w >= 3:
                                kBs.append(G.add(
                                    f"B{q}_{w}", "g", 2 * (w - 2) * s,
                                    [kT1prev, prev2C, rsb],
                                    fn_opB(w, q, "g", pbuf, 1, w - 1)))
                        else:
                            kBs = [G.add(f"B{q}_{w}", ebn, 2 * w * s,
                                         [kT1], fn_opB(w, q, ebn, pbuf))]
                        kR2 = G.add(f"R2{q}_{w}", "v", 2 * w * s,
                                    kBs + r2deps, fn_Bred(w, q, pbuf))
                    else:
                        kR2 = G.add(f"R2{q}_{w}", "v", 2 * s,
                                    [kT1] + r2deps, fn_Bred(w, q, pbuf))
                    prev2C = prevC
                    prevC = G.add(f"C{q}_{w}", "v", 2 * s,
                                  [kR2, "dma_stops"], fn_crcl(w, q))
                    kT1prev = kT1
                def xfn(q=q):
                    v.tensor_copy(
                        ap_of(z, zoff["CROUT"] + q * N, [[1, R[q]]]),
                        ap_of(banks, bb[q] + T_CR * D[q], [[N, R[q]]]))
                G.add(f"X{q}", "v", R[q], [prevC, "ms_crout"], xfn)

            # ---------------- renorm nodes ----------------
            if nq:
                w = RENORM_W

                def fn_rmax():
                    for j, q in enumerate(renorm_qs):
                        s = R[q] - w
                        v.reduce_max(
                            ap_of(z, zoff["M2"] + 2 * j, [[1, 2]]),
                            ap_of(banks, bb[q] + T_KR * D[q] + w * N,
                                  [[5 * D[q], 2], [1, s]]),
                            axis=AX.X)
                    v.tensor_tensor(
                        ap_of(z, zoff["MU"], [[1, nq]]),
                        ap_of(z, zoff["M2"], [[2, nq]]),
                        ap_of(z, zoff["M2"] + 1, [[2, nq]]),
                        OP.max)
                    v.tensor_scalar_mul(
                        ap_of(z, zoff["MU"], [[1, nq]]),
                        ap_of(z, zoff["MU"], [[1, nq]]), 2.0 ** -32)
                    v.tensor_scalar_max(
                        ap_of(z, zoff["MU"], [[1, nq]]),
                        ap_of(z, zoff["MU"], [[1, nq]]), 1e-36)
                G.add("RMAX", "v", 100,
                      [f"R2{q}_{w}" for q in renorm_qs], fn_rmax,
                      fixed=6 * 107.0)

                G.add("RLN", "a", nq, ["RMAX"], lambda: sc.activation(
                    ap_of(z, zoff["LM"], [[1, nq]]),
                    ap_of(z, zoff["MU"], [[1, nq]]), AF.Ln))

                def fn_kchain():
                    # k = round((ln(mu*2^-32) + 32 ln2)/(w ln2)); round via
                    # 1.5*2^23 so every factor is an exact power of two
                    v.tensor_scalar(
                        ap_of(z, zoff["LM"], [[1, nq]]),
                        ap_of(z, zoff["LM"], [[1, nq]]),
                        32.0 * LN2, 1.0 / (w * LN2), OP.add, OP.mult)
                    v.tensor_scalar(
                        ap_of(z, zoff["LM"], [[1, nq]]),
                        ap_of(z, zoff["LM"], [[1, nq]]),
                        12582912.0, 12582912.0, OP.add, OP.subtract)
                    v.tensor_tensor(
                        ap_of(z, zoff["DSUM"], [[1, nq]]),
                        ap_of(z, zoff["DSUM"], [[1, nq]]),
                        ap_of(z, zoff["LM"], [[1, nq]]),
                        OP.add)
                    # scale2 = 2^-k via exponent bits: (127 - k) << 23
                    v.tensor_scalar(
                        ap_of(z, zoff["M2"], [[1, nq]]),
                        ap_of(z, zoff["LM"], [[1, nq]]),
                        -1.0, 127.0, OP.mult, OP.add)
                    zi = z.bitcast(mybir.dt.int32)
                    v.tensor_copy(
                        ap_of(zi, zoff["M2"] + 4, [[1, nq]]),
                        ap_of(z, zoff["M2"], [[1, nq]]))
                    v.tensor_scalar(
                        ap_of(zi, zoff["M2"] + 4, [[1, nq]]),
                        ap_of(zi, zoff["M2"] + 4, [[1, nq]]),
                        23, None, OP.arith_shift_left)
                    # M[j, d] = 2^(-k*d): d=0 -> 1, multiplicative scan
                    v.memset(ap_of(z, zoff["M"], [[N + 1, nq], [1, 1]]), 1.0)
                    for j in range(nq):
                        sca = ap_of(z, zoff["M2"] + 4 + j, [[0, N]])
                        v.tensor_tensor_scan(
                            ap_of(z, zoff["M"] + j * (N + 1) + 1, [[1, N]]),
                            sca, sca, 1.0, OP.mult, OP.bypass)
                G.add("KCH", "v", 60, ["RLN", "ms_dsum"], fn_kchain,
                      fixed=9 * 107.0)

                for j, q in enumerate(renorm_qs):
                    Dq, base = D[q], bb[q]
                    mrow = zoff["M"] + j * (N + 1)
                    e = "g" if q == 0 else "v"

                    def fn_rsb(q=q, Dq=Dq, base=base, mrow=mrow, e=e):
                        def fn():
                            for tb in (T_KR, T_CL):
                                eng_of[e].tensor_tensor(
                                    ap_of(banks, base + tb * Dq,
                                          [[Dq, 2], [N, w + 1], [1, N]]),
                                    ap_of(banks, base + tb * Dq,
                                          [[Dq, 2], [N, w + 1], [1, N]]),
                                    ap_of(z, mrow,
                                          [[0, 2], [1, w + 1], [0, N]]),
                                    OP.mult)
                            eng_of[e].tensor_tensor(
                                ap_of(banks, base + T_IL * Dq,
                                      [[Dq, 2], [N, w], [1, N]]),
                                ap_of(banks, base + T_IL * Dq,
                                      [[Dq, 2], [N, w], [1, N]]),
                                ap_of(z, mrow + 1,
                                      [[0, 2], [1, w], [0, N]]),
                                OP.mult)
                        return fn
                    G.add(f"RSB{q}", e, 6 * (w + 1) * N,
                          ["KCH", f"C{q}_{w}", f"RC{q}_{w + 1}"],
                          fn_rsb(), fixed=3 * G.EST[e][1])

                    def fn_rsc(q=q, mrow=mrow):
                        lo = coff[q][w + 1]
                        hi = lay["cbase"][q + 1]
                        def fn():
                            v.tensor_tensor(
                                ap_of(consts, lo, [[1, hi - lo]]),
                                ap_of(consts, lo, [[1, hi - lo]]),
                                ap_of(z, mrow + 1, [[0, hi - lo]]),
                                OP.mult)
                        return fn
                    G.add(f"RSC{q}", "v",
                          lay["cbase"][q + 1] - coff[q][w + 1],
                          ["KCH", c1dep(q, 40)], fn_rsc())

            # ---------------- output DMA ----------------
            G.add("dma_ecr", "d", NSLOT * N * 4,
                  [f"X{q}" for q in range(NSLOT)], lambda: (
                      nc.sync.dma_start(
                          ap_of(ecr_d, 0, [[1, NSLOT * N]],
                                lead=[NSLOT * N, P128]),
                          ap_of(z, zoff["CROUT"], [[1, NSLOT * N]]))))
            G.add("dma_dsum", "d", NSLOT * 4,
                  ["KCH" if nq else "ms_dsum"], lambda: (
                      nc.sync.dma_start(
                          ap_of(dsum_d, 0, [[1, NSLOT]],
                                lead=[NSLOT, P128]),
                          ap_of(z, zoff["DSUM"], [[1, NSLOT]]))))

            G.emit()

    nc.compile()
    return nc


_NC_CACHE = {}


def get_nc(bounds):
    key = tuple(bounds)
    if key not in _NC_CACHE:
        _NC_CACHE[key] = build_nc(key)
    return _NC_CACHE[key]


def plan(len_array):
    """Sort sentences by length desc, deal round-robin to cores, slot into
    4 groups of 128 per core. Returns (order, bounds) where order[r] is the
    original sentence index of global sorted rank r and bounds[q] is the
    width bound of slot q (same for every core by round-robin construction).
    """
    ln = np.asarray(len_array).astype(np.int64)
    order = np.argsort(-ln, kind="stable")
    bounds = [int(ln[order[min(1024 * q, len(ln) - 1)]])
              for q in range(NSLOT)]
    bounds = [max(b, 1) for b in bounds]
    return order, bounds


def make_in_maps(trans_scores, dec_scores, len_array):
    t = np.asarray(trans_scores, dtype=np.float32)
    dec = np.asarray(dec_scores, dtype=np.float32)
    B = t.shape[0]
    order, bounds = plan(len_array)
    lay = _layout(bounds)
    go = dec[..., 0]                        # [B, n, dir, dv]
    # per-sentence linear pre-shift: each arc factor carries exp(-c0), so a
    # width-w entry is scaled exp(-c0*w); undone on the host at the end.
    tm = np.where(t < -1e8, -np.inf, t).max(axis=3)
    with np.errstate(invalid="ignore"):
        colmax = tm.max(axis=1)             # [B, n] best arc into each child
        proxy = np.nanmean(
            np.where(np.isfinite(colmax), colmax, np.nan)[:, 1:], axis=-1)
    c0 = (proxy + 0.5).astype(np.float32)
    c0 = np.clip(np.nan_to_num(c0), -20.0, 20.0)
    # one exp over trans (NEG -> 0 underflow is intended), then gather diags
    with np.errstate(under="ignore"):
        E = np.exp(t - c0[:, None, None, None])      # [B, n, n, 2]
        ego = np.exp(go)                             # [B, n, 2, 2]
    d_idx, i_idx = np.meshgrid(np.arange(N), np.arange(N), indexing="ij")
    j_idx = np.minimum(i_idx + d_idx, N - 1)
    valid = ((i_idx + d_idx) <= N - 1)[None].astype(np.float32)
    ea = E[:, i_idx, j_idx, :]              # [B, n, n, 2]  trans[i, i+d, v]
    eb = E[:, j_idx, i_idx, :]              # [B, n, n, 2]  trans[i+d, i, v]
    a1 = ea[..., 1] * ego[:, :, 1, 1][:, i_idx] * valid   # [B, d, i]
    a0 = ea[..., 0] * ego[:, :, 1, 0][:, i_idx] * valid
    b1 = eb[..., 1] * ego[:, :, 0, 1][:, j_idx] * valid
    b0 = eb[..., 0] * ego[:, :, 0, 0][:, j_idx] * valid
    est = np.exp(dec[..., 1])               # [B, n, dir, dv]

    # sentence index per (core, slot, partition)
    sent = order.reshape(-1, NCORES).T.reshape(NCORES, NSLOT, P128)

    # valence-edge ratios (every packed cell is a valid arc, so a1,b1 > 0)
    with np.errstate(divide="ignore", invalid="ignore"):
        ra = (a0.astype(np.float64) / a1).astype(np.float32)
        rb = (b0.astype(np.float64) / b1).astype(np.float32)
    consts = np.empty((NCORES, P128, lay["const_total"]), dtype=np.float32)
    for q in range(NSLOT):
        sq = sent[:, q, :]                  # [NCORES, P128]
        for w in range(1, bounds[q] + 1):
            s = bounds[q] + 1 - w
            o = lay["coff"][q][w]
            consts[:, :, o:o + s] = a1[sq, w, :s]
            consts[:, :, o + s:o + 2 * s] = b1[sq, w, :s]
            o2 = lay["c2off"][q][w]
            consts[:, :, o2:o2 + s] = ra[sq, w, :s]
            consts[:, :, o2 + s:o2 + 2 * s] = rb[sq, w, :s]
    stops = np.empty((NCORES, P128, lay["stop_total"]), dtype=np.float32)
    for q in range(NSLOT):
        o = 4 * q * N
        sq = sent[:, q, :]
        stops[:, :, o:o + N] = est[sq][:, :, :, 1, 1]          # SRHAS
        stops[:, :, o + N:o + 2 * N] = est[sq][:, :, :, 0, 1]  # SLHAS
        stops[:, :, o + 2 * N:o + 3 * N] = est[sq][:, :, :, 1, 0]  # SRNO
        stops[:, :, o + 3 * N:o + 4 * N] = est[sq][:, :, :, 0, 0]  # SLNO
    in_maps = [{"consts": consts[c], "stops": stops[c]}
               for c in range(NCORES)]
    aux = dict(c0=c0, order=order, bounds=bounds, sent=sent)
    return in_maps, aux


def assemble(results, len_array, aux):
    ln = np.asarray(len_array).astype(np.int64)
    c0 = np.asarray(aux["c0"]).astype(np.float64)
    sent = aux["sent"]
    out = np.empty(len(ln), dtype=np.float32)
    for c, res in enumerate(results):
        ecr = res["ecr"].reshape(P128, NSLOT * N).astype(np.float64)
        dsum = res["dsum"].reshape(P128, NSLOT).astype(np.float64)
        for q in range(NSLOT):
            idx = sent[c, q]                # original sentence ids [P128]
            lc = ln[idx]
            with np.errstate(divide="ignore"):
                out[idx] = (
                    np.log(ecr[np.arange(P128), q * N + lc])
                    + dsum[:, q] * LN2 * lc + c0[idx] * lc
                ).astype(np.float32)
    return out


def kernel(trans_scores, dec_scores, len_array):
    from concourse.bass_utils import run_bass_kernel_spmd

    in_maps, aux = make_in_maps(trans_scores, dec_scores, len_array)
    nc = get_nc(aux["bounds"])
    res = run_bass_kernel_spmd(nc, in_maps, core_ids=list(range(NCORES)))
    return assemble(res.results, len_array, aux)


# revision 50
# speedup vs baseline: 2.9893x; 1.0138x over previous
"""DMV inside algorithm (Eisner chart DP, logsumexp semiring) on Trainium2.

Strategy
--------
Data parallel over the batch: 4096 sentences -> 8 cores x 512, with
length-aware packing: sentences are sorted by length (desc) and dealt
round-robin to cores, then within a core split into 4 slots of 128
(one sentence per SBUF partition per slot). Slot q only runs chart
widths w <= L[q] (its max length), cutting DP work ~2.4x vs running
all sentences to width 40.

The DP runs in the *exp domain* (tables hold exp(score)); each width-w
update is a fused strided multiply + segmented reduce. Segmented
reduces only exist on VectorE (DVE); multiplies are split between DVE
and GpSimd (Pool); the ACT engine rewrites the valence-ratio rows.
All ops are built as an explicit dependency graph and ordered by a
critical-path list scheduler before emission, because every engine
executes its stream strictly in order (a semaphore wait at the head
blocks everything behind it).

Per slot q the 6 diag-packed tables [R=L+1 rows x 41] live at stride
D=41*R in the order [KR, CR, IL, IR, CL, KL], chosen so every fused
operand pair is adjacent: opA in0={KR,CR}, in1={CL,KL}; opB in0={IR,CL},
in1={CR,IL}; outputs {IR,IL}, {KR,KL}, {CR,CL} all constant-stride.
IR/IL row r holds width r+1; IL/opB patterns are column-shifted so all
gathers are constant-stride.

KR/KL row 0 hold the valence edge RATIOS (RA=a0/a1 at KR[0,i],
RB=b0/b1 at KL[0,i+w]), rewritten per width by the otherwise idle ACT
engine; opA's natural w-term gather then covers both valence edge
cases exactly, once IR/IL is scaled by {A1,B1}. Per-arc constants live
in two triangular packs: tri1={A1,B1} (renorm-rescaled), tri2={RA,RB}
(scale-free).

Numerics: scale composes linearly in span width, so slots with L >= 25
renormalize once at w=20: row d of every table is multiplied by an
exact power of two 2^(-k*d) (k integer per sentence), k accumulated in
dsum and undone on the host: LL = log(CR[0,len]) + k*ln2*len + c0*len.
"""

import os

os.environ.setdefault("JAX_PLATFORMS", "cpu")

import heapq

import numpy as np

import concourse.bass as bass  # noqa: F401  (registers engine classes)
import concourse.tile as tile
import bass_rust
from concourse import bacc, mybir

F32 = mybir.dt.float32
AF = mybir.ActivationFunctionType
OP = mybir.AluOpType
AX = mybir.AxisListType

N = 41              # fake_len (ROOT at 0)
NCORES = 8
NSLOT = 4
P128 = 128
B_CORE = NSLOT * P128
RENORM_W = 20       # renorm width (slots with L >= RENORM_MIN_L)
RENORM_MIN_L = 25
HEAD_W = 6          # consts rows <= HEAD_W form the startup DMA region
EDGE_MAX_W = 34     # widths above this run the whole chain on DVE

# table order within a slot (pairs used by the fused ops are adjacent)
T_KR, T_CR, T_IL, T_IR, T_CL, T_KL = range(6)

LN2 = float(np.log(2.0))

# mult-engine assignment per slot: (opA engine, opB engine), 'v'=DVE 'g'=Pool
MULT_CFG = {0: ("v", "v"), 1: ("v", "g"), 2: ("v", "v"), 3: ("v", "v")}
# slots using the edge/main decomposition: the big mults' t∈[1,w-1) bulk
# only depends on width w-2, so it runs a width ahead on Pool while the
# dependency-carrying edge terms (t=0, t=w-1) are tiny [2,2,s] DVE ops.
EDGE_QS = (0,)


def ap_of(t, offset, dims, lead=None):
    """Build a raw AP on tile/dram ap `t`: [lead or t.ap[0]] + dims."""
    ap = t.copy()
    first = list(t.ap[0]) if lead is None else list(lead)
    ap.ap = bass_rust.VecI64Pair([first] + [list(d) for d in dims])
    ap.offset = offset
    return ap


def _layout(bounds):
    """Compute per-slot offsets for banks / consts / stops / scratch."""
    L = list(bounds)
    R = [l + 1 for l in L]
    D = [N * r for r in R]
    bank_base, acc = [], 0
    for q in range(NSLOT):
        bank_base.append(acc)
        acc += 6 * D[q]
    bank_total = acc
    # consts, two triangles per slot: tri1 per (q,w) = [A1(s), B1(s)]
    # (renorm-rescaled) and tri2 per (q,w) = [RA(s), RB(s)] (scale-free
    # ratios). Rows w <= HEAD_W live in a contiguous head region so ONE
    # DMA covers everything the first DP widths need; the rest is one
    # tail DMA. Within each region blocks are (q, tri, w)-ordered, so any
    # (q, tri, w-range) stays contiguous (the renorm tail rescale relies
    # on rows >= RENORM_W+1 of a slot's tri1 being contiguous).
    coff = [dict() for _ in range(NSLOT)]
    c2off = [dict() for _ in range(NSLOT)]
    # stop vectors lead the head region: per q [SRHAS, SLHAS, SRNO, SLNO] x 41
    stop_total = NSLOT * 4 * N
    acc = stop_total
    for wlo, whi in ((1, HEAD_W), (HEAD_W + 1, N)):
        for q in range(NSLOT):
            for offs in (coff, c2off):
                for w in range(wlo, min(whi, L[q]) + 1):
                    offs[q][w] = acc
                    acc += 2 * (R[q] - w)
        if wlo == 1:
            head_end = acc
    const_total = acc
    # z scratch
    z = {}
    zacc = 0
    for q in range(NSLOT):
        pmax = max((2 * (R[q] - w) * w for w in range(1, L[q] + 1)),
                   default=2)
        for pb in ("PA0", "PA1", "PB0", "PB1"):
            z[(pb, q)] = zacc; zacc += pmax
        z[("SS", q)] = zacc; zacc += 2 * N
        z[("T1", q)] = zacc; zacc += 2 * N
    z["CROUT"] = zacc; zacc += NSLOT * N
    z["DSUM"] = zacc; zacc += NSLOT
    z["M2"] = zacc; zacc += 8
    z["MU"] = zacc; zacc += 2
    z["LM"] = zacc; zacc += 2
    z["M"] = zacc; zacc += NSLOT * (N + 1)  # renorm multiplier rows [q, 42]
    z_total = zacc
    return dict(L=L, R=R, D=D, bank_base=bank_base, bank_total=bank_total,
                coff=coff, c2off=c2off, head_end=head_end,
                const_total=const_total,
                stop_total=stop_total, z=z, z_total=z_total)


class Graph:
    """Op graph + critical-path list scheduler for in-order engines."""

    EST = {"v": (1.0417, 107.0), "g": (1.984, 156.0), "a": (0.833, 217.0),
           "d": (0.386, 1600.0)}  # (ns/elem, fixed ns); d: per byte
    XPEN = 80.0  # cross-engine semaphore latency

    def __init__(self):
        self.nodes = []          # dict(key, eng, est, deps, fn)
        self.byname = {}

    def add(self, key, eng, elems, deps, fn, fixed=None):
        slope, fix = self.EST[eng]
        est = elems * slope + (fix if fixed is None else fixed)
        n = dict(key=key, eng=eng, est=est, deps=[d for d in deps
                                                 if d is not None], fn=fn,
                 idx=len(self.nodes))
        self.nodes.append(n)
        self.byname[key] = n
        return key

    def schedule(self):
        """Earliest-start list scheduling with critical-path tie-break.
        Returns node indices in chosen global emission order."""
        nodes = self.nodes
        nn = len(nodes)
        succ = [[] for _ in range(nn)]
        npred = [0] * nn
        for n in nodes:
            for dk in n["deps"]:
                d = self.byname[dk]
                succ[d["idx"]].append(n["idx"])
                npred[n["idx"]] += 1
        # critical-path priority (longest path to sink)
        prio = [0.0] * nn
        for i in reversed(range(nn)):  # nodes added roughly topologically;
            pass
        order_topo = []
        tmp_pred = npred[:]
        stack = [i for i in range(nn) if tmp_pred[i] == 0]
        while stack:
            i = stack.pop()
            order_topo.append(i)
            for j in succ[i]:
                tmp_pred[j] -= 1
                if tmp_pred[j] == 0:
                    stack.append(j)
        assert len(order_topo) == nn, "cycle in op graph"
        for i in reversed(order_topo):
            best = 0.0
            for j in succ[i]:
                if prio[j] > best:
                    best = prio[j]
            prio[i] = nodes[i]["est"] + best
        # event-driven greedy
        free = {"v": 0.0, "g": 0.0, "a": 0.0, "d": 0.0}
        finish = [0.0] * nn
        ready_t = [0.0] * nn
        npred2 = npred[:]
        ready = [i for i in range(nn) if npred2[i] == 0]
        out = []
        while ready:
            # candidate est-start for each ready node
            best_i, best_start, best_prio = None, None, None
            min_start = min(max(free[nodes[i]["eng"]], ready_t[i])
                            for i in ready)
            for i in ready:
                e = nodes[i]["eng"]
                st = max(free[e], ready_t[i])
                if st <= min_start + 100.0:
                    if best_prio is None or prio[i] > best_prio:
                        best_i, best_start, best_prio = i, st, prio[i]
            i = best_i
            ready.remove(i)
            n = nodes[i]
            e = n["eng"]
            st = best_start
            fin = st + n["est"]
            finish[i] = fin
            free[e] = fin
            out.append((st, i))
            for j in succ[i]:
                pen = self.XPEN if nodes[j]["eng"] != e else 0.0
                ready_t[j] = max(ready_t[j], fin + pen)
                npred2[j] -= 1
                if npred2[j] == 0:
                    ready.append(j)
        assert len(out) == nn
        self.makespan = max(finish) if nn else 0.0
        out.sort(key=lambda t: (t[0], t[1]))
        return [i for _, i in out]

    def emit(self):
        order = self.schedule()
        global LAST_MAKESPAN
        LAST_MAKESPAN = self.makespan
        for i in order:
            self.nodes[i]["fn"]()


LAST_MAKESPAN = None


def build_nc(bounds):
    lay = _layout(bounds)
    L, R, D = lay["L"], lay["R"], lay["D"]
    bb = lay["bank_base"]
    coff = lay["coff"]
    c2off = lay["c2off"]
    zoff = lay["z"]
    renorm_qs = [q for q in range(NSLOT) if L[q] >= RENORM_MIN_L]
    # bounds are sorted desc, so renorm slots are the prefix [0..nq) and
    # dsum[j] lines up with slot j
    assert renorm_qs == list(range(len(renorm_qs)))
    nq = len(renorm_qs)

    nc = bacc.Bacc("TRN2", target_bir_lowering=False, debug=False,
                   num_devices=1)
    consts_in = nc.dram_tensor(
        "consts", [P128, lay["const_total"]], F32, kind="ExternalInput").ap()
    ecr_d = nc.dram_tensor(
        "ecr", [P128, NSLOT * N], F32, kind="ExternalOutput").ap()
    dsum_d = nc.dram_tensor(
        "dsum", [P128, NSLOT], F32, kind="ExternalOutput").ap()

    with tile.TileContext(nc) as tc:
        with tc.tile_pool(name="p", bufs=1) as pool:
            banks_t = pool.tile([P128, lay["bank_total"]], F32)
            consts_t = pool.tile([P128, lay["const_total"]], F32)
            z_t = pool.tile([P128, lay["z_total"]], F32)
            banks = banks_t[:]
            consts = consts_t[:]
            stops = consts  # stop vectors lead the consts head region
            z = z_t[:]

            v = nc.vector
            g = nc.gpsimd
            sc = nc.scalar
            eng_of = {"v": v, "g": g}
            G = Graph()

            # ---------------- input DMA nodes ----------------
            def dma_in(dst_off, src_off, size):
                def fn():
                    nc.sync.dma_start(
                        ap_of(consts, dst_off, [[1, size]]),
                        ap_of(consts_in, src_off, [[1, size]],
                              lead=[lay["const_total"], P128]))
                return fn

            he = lay["head_end"]
            G.add("dma_head", "d", he * 4, [], dma_in(0, 0, he))
            G.add("dma_tail", "d", (lay["const_total"] - he) * 4, [],
                  dma_in(he, he, lay["const_total"] - he))
            G.byname["dma_stops"] = G.byname["dma_head"]

            def c1dep(q, w):
                return "dma_head" if w <= HEAD_W else "dma_tail"

            c2dep = c1dep

            # ---------------- init nodes ----------------
            ms_keys = {q: [] for q in range(NSLOT)}
            for q in renorm_qs:
                for ci, (r0, r1) in enumerate(
                        ((0, 3), (3, 7), (7, RENORM_W + 1))):
                    def mfn(q=q, r0=r0, r1=r1):
                        g.memset(
                            ap_of(banks, bb[q] + r0 * N,
                                  [[D[q], 6], [1, (r1 - r0) * N]]), 0.0)
                    ms_keys[q].append(G.add(
                        f"ms{q}_{ci}", "g", 6 * (r1 - r0) * N, [], mfn))
            G.add("ms_dsum", "a", NSLOT, [], lambda: sc.memzero(
                ap_of(z, zoff["DSUM"], [[1, NSLOT]])))
            G.add("ms_crout", "a", NSLOT * N, [], lambda: sc.memzero(
                ap_of(z, zoff["CROUT"], [[1, NSLOT * N]])))
            init_k = {}
            for q in range(NSLOT):
                def ifn(q=q):
                    v.tensor_copy(
                        ap_of(banks, bb[q] + T_CR * D[q],
                              [[3 * D[q], 2], [1, N]]),
                        ap_of(stops, (4 * q + 2) * N, [[N, 2], [1, N]]))
                init_k[q] = G.add(f"init{q}", "v", 2 * N,
                                  ["dma_stops"] + ms_keys[q][:1], ifn)

            # ---------------- DP op builders ----------------
            def fn_ratcopy(w, q):
                s = R[q] - w
                Dq, base = D[q], bb[q]
                def fn():
                    sc.activation(
                        ap_of(banks, base + T_KR * Dq,
                              [[5 * Dq + w, 2], [1, s]]),
                        ap_of(consts, c2off[q][w], [[s, 2], [1, s]]),
                        AF.Copy)
                return fn

            def fn_opA(w, q, e, pb, t0=0, t1=None):
                """t-slice [t0, t1) of the opA product into buffer pb."""
                s = R[q] - w
                Dq, base = D[q], bb[q]
                tn = (w if t1 is None else t1) - t0
                def fn():
                    out = (ap_of(z, zoff[("SS", q)],
                                 [[s, 2], [1, 1], [1, s]])
                           if w == 1 else
                           ap_of(z, zoff[(pb, q)] + t0 * s,
                                 [[w * s, 2], [s, tn], [1, s]]))
                    eng_of[e].tensor_tensor(
                        out,
                        ap_of(banks, base + T_KR * Dq + t0 * N,
                              [[Dq, 2], [N, tn], [1, s]]),
                        ap_of(banks,
                              base + T_CL * Dq + (w - 1 - t0) * N + 1 + t0,
                              [[Dq, 2], [-(N - 1), tn], [1, s]]),
                        OP.mult)
                return fn

            def fn_edge(w, q, tbl0, pb):
                """Fused edge mult: t in {0, w-1} of opA (tbl0=T_KR) or opB
                (tbl0=T_IR) into buffer pb."""
                s = R[q] - w
                Dq, base = D[q], bb[q]
                in1base = (T_CL if tbl0 == T_KR else T_CR)
                def fn():
                    v.tensor_tensor(
                        ap_of(z, zoff[(pb, q)],
                              [[w * s, 2], [(w - 1) * s, 2], [1, s]]),
                        ap_of(banks, base + tbl0 * Dq,
                              [[Dq, 2], [(w - 1) * N, 2], [1, s]]),
                        ap_of(banks, base + in1base * Dq + (w - 1) * N + 1,
                              [[Dq, 2], [-(w - 1) * (N - 1), 2], [1, s]]),
                        OP.mult)
                return fn

            def fn_Ared(w, q, pb):
                s = R[q] - w
                def fn():
                    v.reduce_sum(
                        ap_of(z, zoff[("SS", q)], [[s, 2], [1, s]]),
                        ap_of(z, zoff[(pb, q)],
                              [[w * s, 2], [1, s], [s, w]]),
                        axis=AX.X)
                return fn

            def fn_T1(w, q):
                s = R[q] - w
                Dq, base = D[q], bb[q]
                def fn():
                    v.tensor_tensor(
                        ap_of(banks, base + T_IR * Dq + (w - 1) * N,
                              [[-Dq + 1, 2], [1, s]]),
                        ap_of(z, zoff[("SS", q)], [[s, 2], [1, s]]),
                        ap_of(consts, coff[q][w], [[s, 2], [1, s]]),
                        OP.mult)
                return fn

            def fn_opB(w, q, e, pb, t0=0, t1=None):
                s = R[q] - w
                Dq, base = D[q], bb[q]
                tn = (w if t1 is None else t1) - t0
                def fn():
                    eng_of[e].tensor_tensor(
                        ap_of(z, zoff[(pb, q)] + t0 * s,
                              [[w * s, 2], [s, tn], [1, s]]),
                        ap_of(banks, base + T_IR * Dq + t0 * N,
                              [[Dq, 2], [N, tn], [1, s]]),
                        ap_of(banks,
                              base + T_CR * Dq + (w - 1 - t0) * N + 1 + t0,
                              [[Dq, 2], [-(N - 1), tn], [1, s]]),
                        OP.mult)
                return fn

            def fn_Bred(w, q, pb):
                s = R[q] - w
                Dq, base = D[q], bb[q]
                def fn():
                    krout = ap_of(banks, base + T_KR * Dq + w * N,
                                  [[5 * Dq, 2], [1, s]])
                    if w == 1:
                        v.tensor_tensor(
                            krout,
                            ap_of(banks, base + T_IR * Dq,
                                  [[Dq, 2], [1, s]]),
                            ap_of(banks, base + T_CR * Dq + 1,
                                  [[Dq, 2], [1, s]]),
                            OP.mult)
                    else:
                        v.reduce_sum(
                            krout,
                            ap_of(z, zoff[(pb, q)],
                                  [[w * s, 2], [1, s], [s, w]]),
                            axis=AX.X)
                return fn

            def fn_crcl(w, q):
                s = R[q] - w
                Dq, base = D[q], bb[q]
                def fn():
                    v.tensor_tensor(
                        ap_of(banks, base + T_CR * Dq + w * N,
                              [[3 * Dq, 2], [1, s]]),
                        ap_of(banks, base + T_KR * Dq + w * N,
                              [[5 * Dq, 2], [1, s]]),
                        ap_of(stops, 4 * q * N, [[N + w, 2], [1, s]]),
                        OP.mult)
                return fn

            # ---------------- DP graph ----------------
            for q in range(NSLOT):
                G.add(f"RC{q}_1", "a", 2 * (R[q] - 1),
                      [c2dep(q, 1), ms_keys[q][0] if ms_keys[q] else None],
                      fn_ratcopy(1, q))
            for q in range(NSLOT):
                prevC, prev2C = None, None
                kT1prev = None
                for w in range(1, L[q] + 1):
                    s = R[q] - w
                    ea, ebn = MULT_CFG[q]
                    edged = (q in EDGE_QS and 2 <= w <= EDGE_MAX_W)
                    pa = ("PA1" if w % 2 else "PA0") if edged else "PA0"
                    pbuf = ("PB1" if w % 2 else "PB0") if edged else "PA0"
                    rsb = (f"RSB{q}" if q in renorm_qs
                           and RENORM_W + 1 <= w <= RENORM_W + 2 else None)
                    adeps = [f"RC{q}_{w}", prevC, rsb]
                    if w == 1:
                        adeps += [init_k[q]]
                        if ms_keys[q]:
                            adeps += [ms_keys[q][0]]
                    if edged:
                        kAs = [G.add(f"AE{q}_{w}", "v", 4 * s, adeps,
                                     fn_edge(w, q, T_KR, pa))]
                        if w >= 3:
                            kAs.append(G.add(
                                f"A{q}_{w}", "g", 2 * (w - 2) * s,
                                [prev2C, rsb],
                                fn_opA(w, q, "g", pa, 1, w - 1)))
                        rdr = kAs[0]
                    else:
                        kAs = [G.add(f"A{q}_{w}", ea, 2 * w * s, adeps,
                                     fn_opA(w, q, ea, pa))]
                        rdr = kAs[0]
                    if w < L[q]:
                        # WAR: only the edge (or whole) op reads RAT row 0
                        G.add(f"RC{q}_{w + 1}", "a", 2 * (s - 1),
                              [rdr, c2dep(q, w + 1)], fn_ratcopy(w + 1, q))
                    if w > 1:
                        kR1 = G.add(f"R1{q}_{w}", "v", 2 * w * s,
                                    kAs + [kT1prev], fn_Ared(w, q, pa))
                    else:
                        kR1 = kAs[0]
                    t1deps = [kR1, c1dep(q, w)]
                    if q in renorm_qs and w == RENORM_W + 1:
                        t1deps += [f"RSC{q}"]
                    kT1 = G.add(f"T1{q}_{w}", "v", 2 * s, t1deps,
                                fn_T1(w, q))
                    # staged memsets zero bank rows [3,7) and [7,21); the
                    # first DP write into each zone must wait for its chunk
                    r2deps = []
                    if ms_keys[q]:
                        if w == 3:
                            r2deps.append(ms_keys[q][1])
                        elif w == 7:
                            r2deps.append(ms_keys[q][2])
                    if w > 1:
                        if edged:
                            kBs = [G.add(f"BE{q}_{w}", "v", 4 * s,
                                         [kT1, prevC, rsb],
                                         fn_edge(w, q, T_IR, pbuf))]
                            if w >= 3:
                                kBs.append(G.add(
                                    f"B{q}_{w}", "g", 2 * (w - 2) * s,
                                    [kT1prev, prev2C, rsb],
                                    fn_opB(w, q, "g", pbuf, 1, w - 1)))
                        else:
                            kBs = [G.add(f"B{q}_{w}", ebn, 2 * w * s,
                                         [kT1], fn_opB(w, q, ebn, pbuf))]
                        kR2 = G.add(f"R2{q}_{w}", "v", 2 * w * s,
                                    kBs + r2deps, fn_Bred(w, q, pbuf))
                    else:
                        kR2 = G.add(f"R2{q}_{w}", "v", 2 * s,
                                    [kT1] + r2deps, fn_Bred(w, q, pbuf))
                    prev2C = prevC
                    prevC = G.add(f"C{q}_{w}", "v", 2 * s,
                                  [kR2, "dma_stops"], fn_crcl(w, q))
                    kT1prev = kT1
                def xfn(q=q):
                    sc.activation(
                        ap_of(z, zoff["CROUT"] + q * N, [[1, R[q]]]),
                        ap_of(banks, bb[q] + T_CR * D[q], [[N, R[q]]]),
                        AF.Copy)
                G.add(f"X{q}", "a", R[q], [prevC, "ms_crout"], xfn)

            # ---------------- renorm nodes ----------------
            if nq:
                w = RENORM_W

                def fn_rmax():
                    for j, q in enumerate(renorm_qs):
                        s = R[q] - w
                        v.reduce_max(
                            ap_of(z, zoff["M2"] + 2 * j, [[1, 2]]),
                            ap_of(banks, bb[q] + T_KR * D[q] + w * N,
                                  [[5 * D[q], 2], [1, s]]),
                            axis=AX.X)
                    v.tensor_tensor(
                        ap_of(z, zoff["MU"], [[1, nq]]),
                        ap_of(z, zoff["M2"], [[2, nq]]),
                        ap_of(z, zoff["M2"] + 1, [[2, nq]]),
                        OP.max)
                    v.tensor_scalar_mul(
                        ap_of(z, zoff["MU"], [[1, nq]]),
                        ap_of(z, zoff["MU"], [[1, nq]]), 2.0 ** -32)
                    v.tensor_scalar_max(
                        ap_of(z, zoff["MU"], [[1, nq]]),
                        ap_of(z, zoff["MU"], [[1, nq]]), 1e-36)
                G.add("RMAX", "v", 100,
                      [f"R2{q}_{w}" for q in renorm_qs], fn_rmax,
                      fixed=6 * 107.0)

                G.add("RLN", "a", nq, ["RMAX"], lambda: sc.activation(
                    ap_of(z, zoff["LM"], [[1, nq]]),
                    ap_of(z, zoff["MU"], [[1, nq]]), AF.Ln))

                def fn_kchain():
                    # k = round((ln(mu*2^-32) + 32 ln2)/(w ln2)); round via
                    # 1.5*2^23 so every factor is an exact power of two
                    v.tensor_scalar(
                        ap_of(z, zoff["LM"], [[1, nq]]),
                        ap_of(z, zoff["LM"], [[1, nq]]),
                        32.0 * LN2, 1.0 / (w * LN2), OP.add, OP.mult)
                    v.tensor_scalar(
                        ap_of(z, zoff["LM"], [[1, nq]]),
                        ap_of(z, zoff["LM"], [[1, nq]]),
                        12582912.0, 12582912.0, OP.add, OP.subtract)
                    v.tensor_tensor(
                        ap_of(z, zoff["DSUM"], [[1, nq]]),
                        ap_of(z, zoff["DSUM"], [[1, nq]]),
                        ap_of(z, zoff["LM"], [[1, nq]]),
                        OP.add)
                    # scale2 = 2^-k via exponent bits: (127 - k) << 23
                    v.tensor_scalar(
                        ap_of(z, zoff["M2"], [[1, nq]]),
                        ap_of(z, zoff["LM"], [[1, nq]]),
                        -1.0, 127.0, OP.mult, OP.add)
                    zi = z.bitcast(mybir.dt.int32)
                    v.tensor_copy(
                        ap_of(zi, zoff["M2"] + 4, [[1, nq]]),
                        ap_of(z, zoff["M2"], [[1, nq]]))
                    v.tensor_scalar(
                        ap_of(zi, zoff["M2"] + 4, [[1, nq]]),
                        ap_of(zi, zoff["M2"] + 4, [[1, nq]]),
                        23, None, OP.arith_shift_left)
                    # M[j, d] = 2^(-k*d): d=0 -> 1, multiplicative scan
                    v.memset(ap_of(z, zoff["M"], [[N + 1, nq], [1, 1]]), 1.0)
                    for j in range(nq):
                        sca = ap_of(z, zoff["M2"] + 4 + j, [[0, N]])
                        v.tensor_tensor_scan(
                            ap_of(z, zoff["M"] + j * (N + 1) + 1, [[1, N]]),
                            sca, sca, 1.0, OP.mult, OP.bypass)
                G.add("KCH", "v", 60, ["RLN", "ms_dsum"], fn_kchain,
                      fixed=9 * 107.0)

                for j, q in enumerate(renorm_qs):
                    Dq, base = D[q], bb[q]
                    mrow = zoff["M"] + j * (N + 1)
                    e = "g" if q == 0 else "v"

                    def fn_rsb(q=q, Dq=Dq, base=base, mrow=mrow, e=e):
                        def fn():
                            for tb in (T_KR, T_CL):
                                eng_of[e].tensor_tensor(
                                    ap_of(banks, base + tb * Dq,
                                          [[Dq, 2], [N, w + 1], [1, N]]),
                                    ap_of(banks, base + tb * Dq,
                                          [[Dq, 2], [N, w + 1], [1, N]]),
                                    ap_of(z, mrow,
                                          [[0, 2], [1, w + 1], [0, N]]),
                                    OP.mult)
                            eng_of[e].tensor_tensor(
                                ap_of(banks, base + T_IL * Dq,
                                      [[Dq, 2], [N, w], [1, N]]),
                                ap_of(banks, base + T_IL * Dq,
                                      [[Dq, 2], [N, w], [1, N]]),
                                ap_of(z, mrow + 1,
                                      [[0, 2], [1, w], [0, N]]),
                                OP.mult)
                        return fn
                    G.add(f"RSB{q}", e, 6 * (w + 1) * N,
                          ["KCH", f"C{q}_{w}", f"RC{q}_{w + 1}"],
                          fn_rsb(), fixed=3 * G.EST[e][1])

                    def fn_rsc(q=q, mrow=mrow):
                        lo = coff[q][w + 1]
                        hi = coff[q][L[q]] + 2 * (R[q] - L[q])
                        def fn():
                            v.tensor_tensor(
                                ap_of(consts, lo, [[1, hi - lo]]),
                                ap_of(consts, lo, [[1, hi - lo]]),
                                ap_of(z, mrow + 1, [[0, hi - lo]]),
                                OP.mult)
                        return fn
                    G.add(f"RSC{q}", "v",
                          coff[q][L[q]] + 2 * (R[q] - L[q])
                          - coff[q][w + 1],
                          ["KCH", "dma_tail"], fn_rsc())

            # ---------------- output DMA (per slot, overlaps the rest) ----
            for q in range(NSLOT):
                def efn(q=q):
                    nc.sync.dma_start(
                        ap_of(ecr_d, q * N, [[1, N]],
                              lead=[NSLOT * N, P128]),
                        ap_of(z, zoff["CROUT"] + q * N, [[1, N]]))
                G.add(f"dma_ecr{q}", "d", N * 4, [f"X{q}"], efn)
            G.add("dma_dsum", "d", NSLOT * 4,
                  ["KCH" if nq else "ms_dsum"], lambda: (
                      nc.sync.dma_start(
                          ap_of(dsum_d, 0, [[1, NSLOT]],
                                lead=[NSLOT, P128]),
                          ap_of(z, zoff["DSUM"], [[1, NSLOT]]))))

            G.emit()

    nc.compile()
    return nc


_NC_CACHE = {}


def get_nc(bounds):
    key = tuple(bounds)
    if key not in _NC_CACHE:
        _NC_CACHE[key] = build_nc(key)
    return _NC_CACHE[key]


def plan(len_array):
    """Sort sentences by length desc, deal round-robin to cores, slot into
    4 groups of 128 per core. Returns (order, bounds) where order[r] is the
    original sentence index of global sorted rank r and bounds[q] is the
    width bound of slot q (same for every core by round-robin construction).
    """
    ln = np.asarray(len_array).astype(np.int64)
    order = np.argsort(-ln, kind="stable")
    bounds = [int(ln[order[min(1024 * q, len(ln) - 1)]])
              for q in range(NSLOT)]
    bounds = [max(b, 1) for b in bounds]
    return order, bounds


def make_in_maps(trans_scores, dec_scores, len_array):
    t = np.asarray(trans_scores, dtype=np.float32)
    dec = np.asarray(dec_scores, dtype=np.float32)
    B = t.shape[0]
    order, bounds = plan(len_array)
    lay = _layout(bounds)
    go = dec[..., 0]                        # [B, n, dir, dv]
    # per-sentence linear pre-shift: each arc factor carries exp(-c0), so a
    # width-w entry is scaled exp(-c0*w); undone on the host at the end.
    tm = np.where(t < -1e8, -np.inf, t).max(axis=3)
    with np.errstate(invalid="ignore"):
        colmax = tm.max(axis=1)             # [B, n] best arc into each child
        proxy = np.nanmean(
            np.where(np.isfinite(colmax), colmax, np.nan)[:, 1:], axis=-1)
    c0 = (proxy + 0.5).astype(np.float32)
    c0 = np.clip(np.nan_to_num(c0), -20.0, 20.0)
    # one exp over trans (NEG -> 0 underflow is intended), then gather diags
    with np.errstate(under="ignore"):
        E = np.exp(t - c0[:, None, None, None])      # [B, n, n, 2]
        ego = np.exp(go)                             # [B, n, 2, 2]
    d_idx, i_idx = np.meshgrid(np.arange(N), np.arange(N), indexing="ij")
    j_idx = np.minimum(i_idx + d_idx, N - 1)
    valid = ((i_idx + d_idx) <= N - 1)[None].astype(np.float32)
    ea = E[:, i_idx, j_idx, :]              # [B, n, n, 2]  trans[i, i+d, v]
    eb = E[:, j_idx, i_idx, :]              # [B, n, n, 2]  trans[i+d, i, v]
    a1 = ea[..., 1] * ego[:, :, 1, 1][:, i_idx] * valid   # [B, d, i]
    a0 = ea[..., 0] * ego[:, :, 1, 0][:, i_idx] * valid
    b1 = eb[..., 1] * ego[:, :, 0, 1][:, j_idx] * valid
    b0 = eb[..., 0] * ego[:, :, 0, 0][:, j_idx] * valid
    est = np.exp(dec[..., 1])               # [B, n, dir, dv]

    # sentence index per (core, slot, partition)
    sent = order.reshape(-1, NCORES).T.reshape(NCORES, NSLOT, P128)

    # valence-edge ratios (every packed cell is a valid arc, so a1,b1 > 0)
    with np.errstate(divide="ignore", invalid="ignore"):
        ra = (a0.astype(np.float64) / a1).astype(np.float32)
        rb = (b0.astype(np.float64) / b1).astype(np.float32)
    consts = np.empty((NCORES, P128, lay["const_total"]), dtype=np.float32)
    for q in range(NSLOT):
        sq = sent[:, q, :]                  # [NCORES, P128]
        for w in range(1, bounds[q] + 1):
            s = bounds[q] + 1 - w
            o = lay["coff"][q][w]
            consts[:, :, o:o + s] = a1[sq, w, :s]
            consts[:, :, o + s:o + 2 * s] = b1[sq, w, :s]
            o2 = lay["c2off"][q][w]
            consts[:, :, o2:o2 + s] = ra[sq, w, :s]
            consts[:, :, o2 + s:o2 + 2 * s] = rb[sq, w, :s]
    for q in range(NSLOT):
        o = 4 * q * N
        sq = sent[:, q, :]
        consts[:, :, o:o + N] = est[sq][:, :, :, 1, 1]          # SRHAS
        consts[:, :, o + N:o + 2 * N] = est[sq][:, :, :, 0, 1]  # SLHAS
        consts[:, :, o + 2 * N:o + 3 * N] = est[sq][:, :, :, 1, 0]  # SRNO
        consts[:, :, o + 3 * N:o + 4 * N] = est[sq][:, :, :, 0, 0]  # SLNO
    in_maps = [{"consts": consts[c]} for c in range(NCORES)]
    aux = dict(c0=c0, order=order, bounds=bounds, sent=sent)
    return in_maps, aux


def assemble(results, len_array, aux):
    ln = np.asarray(len_array).astype(np.int64)
    c0 = np.asarray(aux["c0"]).astype(np.float64)
    sent = aux["sent"]
    out = np.empty(len(ln), dtype=np.float32)
    for c, res in enumerate(results):
        ecr = res["ecr"].reshape(P128, NSLOT * N).astype(np.float64)
        dsum = res["dsum"].reshape(P128, NSLOT).astype(np.float64)
        for q in range(NSLOT):
            idx = sent[c, q]                # original sentence ids [P128]
            lc = ln[idx]
            with np.errstate(divide="ignore"):
                out[idx] = (
                    np.log(ecr[np.arange(P128), q * N + lc])
                    + dsum[:, q] * LN2 * lc + c0[idx] * lc
                ).astype(np.float32)
    return out


def kernel(trans_scores, dec_scores, len_array):
    from concourse.bass_utils import run_bass_kernel_spmd

    in_maps, aux = make_in_maps(trans_scores, dec_scores, len_array)
    nc = get_nc(aux["bounds"])
    res = run_bass_kernel_spmd(nc, in_maps, core_ids=list(range(NCORES)))
    return assemble(res.results, len_array, aux)


# revision 51
# speedup vs baseline: 3.0020x; 1.0043x over previous
"""DMV inside algorithm (Eisner chart DP, logsumexp semiring) on Trainium2.

Strategy
--------
Data parallel over the batch: 4096 sentences -> 8 cores x 512, with
length-aware packing: sentences are sorted by length (desc) and dealt
round-robin to cores, then within a core split into 4 slots of 128
(one sentence per SBUF partition per slot). Slot q only runs chart
widths w <= L[q] (its max length), cutting DP work ~2.4x vs running
all sentences to width 40.

The DP runs in the *exp domain* (tables hold exp(score)); each width-w
update is a fused strided multiply + segmented reduce. Segmented
reduces only exist on VectorE (DVE); multiplies are split between DVE
and GpSimd (Pool); the ACT engine rewrites the valence-ratio rows.
All ops are built as an explicit dependency graph and ordered by a
critical-path list scheduler before emission, because every engine
executes its stream strictly in order (a semaphore wait at the head
blocks everything behind it).

Per slot q the 6 diag-packed tables [R=L+1 rows x 41] live at stride
D=41*R in the order [KR, CR, IL, IR, CL, KL], chosen so every fused
operand pair is adjacent: opA in0={KR,CR}, in1={CL,KL}; opB in0={IR,CL},
in1={CR,IL}; outputs {IR,IL}, {KR,KL}, {CR,CL} all constant-stride.
IR/IL row r holds width r+1; IL/opB patterns are column-shifted so all
gathers are constant-stride.

KR/KL row 0 hold the valence edge RATIOS (RA=a0/a1 at KR[0,i],
RB=b0/b1 at KL[0,i+w]), rewritten per width by the otherwise idle ACT
engine; opA's natural w-term gather then covers both valence edge
cases exactly, once IR/IL is scaled by {A1,B1}. Per-arc constants live
in two triangular packs: tri1={A1,B1} (renorm-rescaled), tri2={RA,RB}
(scale-free).

Numerics: scale composes linearly in span width, so slots with L >= 25
renormalize once at w=20: row d of every table is multiplied by an
exact power of two 2^(-k*d) (k integer per sentence), k accumulated in
dsum and undone on the host: LL = log(CR[0,len]) + k*ln2*len + c0*len.
"""

import os

os.environ.setdefault("JAX_PLATFORMS", "cpu")

import heapq

import numpy as np

import concourse.bass as bass  # noqa: F401  (registers engine classes)
import concourse.tile as tile
import bass_rust
from concourse import bacc, mybir

F32 = mybir.dt.float32
AF = mybir.ActivationFunctionType
OP = mybir.AluOpType
AX = mybir.AxisListType

N = 41              # fake_len (ROOT at 0)
NCORES = 8
NSLOT = 4
P128 = 128
B_CORE = NSLOT * P128
RENORM_W = 20       # renorm width (slots with L >= RENORM_MIN_L)
RENORM_MIN_L = 25
HEAD_W = 6          # consts rows <= HEAD_W form the startup DMA region
EDGE_MAX_W = 36     # widths above this run the whole chain on DVE

# table order within a slot (pairs used by the fused ops are adjacent)
T_KR, T_CR, T_IL, T_IR, T_CL, T_KL = range(6)

LN2 = float(np.log(2.0))

# mult-engine assignment per slot: (opA engine, opB engine), 'v'=DVE 'g'=Pool
MULT_CFG = {0: ("v", "v"), 1: ("v", "g"), 2: ("v", "v"), 3: ("v", "v")}
# slots using the edge/main decomposition: the big mults' t∈[1,w-1) bulk
# only depends on width w-2, so it runs a width ahead on Pool while the
# dependency-carrying edge terms (t=0, t=w-1) are tiny [2,2,s] DVE ops.
EDGE_QS = (0,)


def ap_of(t, offset, dims, lead=None):
    """Build a raw AP on tile/dram ap `t`: [lead or t.ap[0]] + dims."""
    ap = t.copy()
    first = list(t.ap[0]) if lead is None else list(lead)
    ap.ap = bass_rust.VecI64Pair([first] + [list(d) for d in dims])
    ap.offset = offset
    return ap


def _layout(bounds):
    """Compute per-slot offsets for banks / consts / stops / scratch."""
    L = list(bounds)
    R = [l + 1 for l in L]
    D = [N * r for r in R]
    bank_base, acc = [], 0
    for q in range(NSLOT):
        bank_base.append(acc)
        acc += 6 * D[q]
    bank_total = acc
    # consts, two triangles per slot: tri1 per (q,w) = [A1(s), B1(s)]
    # (renorm-rescaled) and tri2 per (q,w) = [RA(s), RB(s)] (scale-free
    # ratios). Rows w <= HEAD_W live in a contiguous head region so ONE
    # DMA covers everything the first DP widths need; the rest is one
    # tail DMA. Within each region blocks are (q, tri, w)-ordered, so any
    # (q, tri, w-range) stays contiguous (the renorm tail rescale relies
    # on rows >= RENORM_W+1 of a slot's tri1 being contiguous).
    coff = [dict() for _ in range(NSLOT)]
    c2off = [dict() for _ in range(NSLOT)]
    # stop vectors lead the head region: per q [SRHAS, SLHAS, SRNO, SLNO] x 41
    stop_total = NSLOT * 4 * N
    acc = stop_total
    for wlo, whi in ((1, HEAD_W), (HEAD_W + 1, N)):
        for q in range(NSLOT):
            for offs in (coff, c2off):
                for w in range(wlo, min(whi, L[q]) + 1):
                    offs[q][w] = acc
                    acc += 2 * (R[q] - w)
        if wlo == 1:
            head_end = acc
    const_total = acc
    # z scratch
    z = {}
    zacc = 0
    for q in range(NSLOT):
        pmax = max((2 * (R[q] - w) * w for w in range(1, L[q] + 1)),
                   default=2)
        for pb in ("PA0", "PA1", "PB0", "PB1"):
            z[(pb, q)] = zacc; zacc += pmax
        z[("SS", q)] = zacc; zacc += 2 * N
        z[("T1", q)] = zacc; zacc += 2 * N
    z["CROUT"] = zacc; zacc += NSLOT * N
    z["DSUM"] = zacc; zacc += NSLOT
    z["M2"] = zacc; zacc += 8
    z["MU"] = zacc; zacc += 2
    z["LM"] = zacc; zacc += 2
    z["M"] = zacc; zacc += NSLOT * (N + 1)  # renorm multiplier rows [q, 42]
    z_total = zacc
    return dict(L=L, R=R, D=D, bank_base=bank_base, bank_total=bank_total,
                coff=coff, c2off=c2off, head_end=head_end,
                const_total=const_total,
                stop_total=stop_total, z=z, z_total=z_total)


class Graph:
    """Op graph + critical-path list scheduler for in-order engines."""

    EST = {"v": (1.0417, 107.0), "g": (1.984, 156.0), "a": (0.833, 217.0),
           "d": (0.386, 1600.0)}  # (ns/elem, fixed ns); d: per byte
    XPEN = 80.0  # cross-engine semaphore latency

    def __init__(self):
        self.nodes = []          # dict(key, eng, est, deps, fn)
        self.byname = {}

    def add(self, key, eng, elems, deps, fn, fixed=None):
        slope, fix = self.EST[eng]
        est = elems * slope + (fix if fixed is None else fixed)
        n = dict(key=key, eng=eng, est=est, deps=[d for d in deps
                                                 if d is not None], fn=fn,
                 idx=len(self.nodes))
        self.nodes.append(n)
        self.byname[key] = n
        return key

    def schedule(self):
        """Earliest-start list scheduling with critical-path tie-break.
        Returns node indices in chosen global emission order."""
        nodes = self.nodes
        nn = len(nodes)
        succ = [[] for _ in range(nn)]
        npred = [0] * nn
        for n in nodes:
            for dk in n["deps"]:
                d = self.byname[dk]
                succ[d["idx"]].append(n["idx"])
                npred[n["idx"]] += 1
        # critical-path priority (longest path to sink)
        prio = [0.0] * nn
        for i in reversed(range(nn)):  # nodes added roughly topologically;
            pass
        order_topo = []
        tmp_pred = npred[:]
        stack = [i for i in range(nn) if tmp_pred[i] == 0]
        while stack:
            i = stack.pop()
            order_topo.append(i)
            for j in succ[i]:
                tmp_pred[j] -= 1
                if tmp_pred[j] == 0:
                    stack.append(j)
        assert len(order_topo) == nn, "cycle in op graph"
        for i in reversed(order_topo):
            best = 0.0
            for j in succ[i]:
                if prio[j] > best:
                    best = prio[j]
            prio[i] = nodes[i]["est"] + best
        # event-driven greedy
        free = {"v": 0.0, "g": 0.0, "a": 0.0, "d": 0.0}
        finish = [0.0] * nn
        ready_t = [0.0] * nn
        npred2 = npred[:]
        ready = [i for i in range(nn) if npred2[i] == 0]
        out = []
        while ready:
            # candidate est-start for each ready node
            best_i, best_start, best_prio = None, None, None
            min_start = min(max(free[nodes[i]["eng"]], ready_t[i])
                            for i in ready)
            for i in ready:
                e = nodes[i]["eng"]
                st = max(free[e], ready_t[i])
                if st <= min_start + 100.0:
                    if best_prio is None or prio[i] > best_prio:
                        best_i, best_start, best_prio = i, st, prio[i]
            i = best_i
            ready.remove(i)
            n = nodes[i]
            e = n["eng"]
            st = best_start
            fin = st + n["est"]
            finish[i] = fin
            free[e] = fin
            out.append((st, i))
            for j in succ[i]:
                pen = self.XPEN if nodes[j]["eng"] != e else 0.0
                ready_t[j] = max(ready_t[j], fin + pen)
                npred2[j] -= 1
                if npred2[j] == 0:
                    ready.append(j)
        assert len(out) == nn
        self.makespan = max(finish) if nn else 0.0
        out.sort(key=lambda t: (t[0], t[1]))
        return [i for _, i in out]

    def emit(self):
        order = self.schedule()
        global LAST_MAKESPAN
        LAST_MAKESPAN = self.makespan
        for i in order:
            self.nodes[i]["fn"]()


LAST_MAKESPAN = None


def build_nc(bounds):
    lay = _layout(bounds)
    L, R, D = lay["L"], lay["R"], lay["D"]
    bb = lay["bank_base"]
    coff = lay["coff"]
    c2off = lay["c2off"]
    zoff = lay["z"]
    renorm_qs = [q for q in range(NSLOT) if L[q] >= RENORM_MIN_L]
    # bounds are sorted desc, so renorm slots are the prefix [0..nq) and
    # dsum[j] lines up with slot j
    assert renorm_qs == list(range(len(renorm_qs)))
    nq = len(renorm_qs)

    nc = bacc.Bacc("TRN2", target_bir_lowering=False, debug=False,
                   num_devices=1)
    consts_in = nc.dram_tensor(
        "consts", [P128, lay["const_total"]], F32, kind="ExternalInput").ap()
    ecr_d = nc.dram_tensor(
        "ecr", [P128, NSLOT * N], F32, kind="ExternalOutput").ap()
    dsum_d = nc.dram_tensor(
        "dsum", [P128, NSLOT], F32, kind="ExternalOutput").ap()

    with tile.TileContext(nc) as tc:
        with tc.tile_pool(name="p", bufs=1) as pool:
            banks_t = pool.tile([P128, lay["bank_total"]], F32)
            consts_t = pool.tile([P128, lay["const_total"]], F32)
            z_t = pool.tile([P128, lay["z_total"]], F32)
            banks = banks_t[:]
            consts = consts_t[:]
            stops = consts  # stop vectors lead the consts head region
            z = z_t[:]

            v = nc.vector
            g = nc.gpsimd
            sc = nc.scalar
            eng_of = {"v": v, "g": g}
            G = Graph()

            # ---------------- input DMA nodes ----------------
            def dma_in(dst_off, src_off, size):
                def fn():
                    nc.sync.dma_start(
                        ap_of(consts, dst_off, [[1, size]]),
                        ap_of(consts_in, src_off, [[1, size]],
                              lead=[lay["const_total"], P128]))
                return fn

            he = lay["head_end"]
            G.add("dma_head", "d", he * 4, [], dma_in(0, 0, he))
            G.add("dma_tail", "d", (lay["const_total"] - he) * 4, [],
                  dma_in(he, he, lay["const_total"] - he))
            G.byname["dma_stops"] = G.byname["dma_head"]

            def c1dep(q, w):
                return "dma_head" if w <= HEAD_W else "dma_tail"

            c2dep = c1dep

            # ---------------- init nodes ----------------
            ms_keys = {q: [] for q in range(NSLOT)}
            for q in renorm_qs:
                for ci, (r0, r1) in enumerate(
                        ((0, 3), (3, 7), (7, RENORM_W + 1))):
                    def mfn(q=q, r0=r0, r1=r1):
                        g.memset(
                            ap_of(banks, bb[q] + r0 * N,
                                  [[D[q], 6], [1, (r1 - r0) * N]]), 0.0)
                    ms_keys[q].append(G.add(
                        f"ms{q}_{ci}", "g", 6 * (r1 - r0) * N, [], mfn))
            G.add("ms_dsum", "a", NSLOT, [], lambda: sc.memzero(
                ap_of(z, zoff["DSUM"], [[1, NSLOT]])))
            G.add("ms_crout", "a", NSLOT * N, [], lambda: sc.memzero(
                ap_of(z, zoff["CROUT"], [[1, NSLOT * N]])))
            init_k = {}
            for q in range(NSLOT):
                def ifn(q=q):
                    v.tensor_copy(
                        ap_of(banks, bb[q] + T_CR * D[q],
                              [[3 * D[q], 2], [1, N]]),
                        ap_of(stops, (4 * q + 2) * N, [[N, 2], [1, N]]))
                init_k[q] = G.add(f"init{q}", "v", 2 * N,
                                  ["dma_stops"] + ms_keys[q][:1], ifn)

            # ---------------- DP op builders ----------------
            def fn_ratcopy(w, q):
                s = R[q] - w
                Dq, base = D[q], bb[q]
                def fn():
                    sc.activation(
                        ap_of(banks, base + T_KR * Dq,
                              [[5 * Dq + w, 2], [1, s]]),
                        ap_of(consts, c2off[q][w], [[s, 2], [1, s]]),
                        AF.Copy)
                return fn

            def fn_opA(w, q, e, pb, t0=0, t1=None):
                """t-slice [t0, t1) of the opA product into buffer pb."""
                s = R[q] - w
                Dq, base = D[q], bb[q]
                tn = (w if t1 is None else t1) - t0
                def fn():
                    out = (ap_of(z, zoff[("SS", q)],
                                 [[s, 2], [1, 1], [1, s]])
                           if w == 1 else
                           ap_of(z, zoff[(pb, q)] + t0 * s,
                                 [[w * s, 2], [s, tn], [1, s]]))
                    eng_of[e].tensor_tensor(
                        out,
                        ap_of(banks, base + T_KR * Dq + t0 * N,
                              [[Dq, 2], [N, tn], [1, s]]),
                        ap_of(banks,
                              base + T_CL * Dq + (w - 1 - t0) * N + 1 + t0,
                              [[Dq, 2], [-(N - 1), tn], [1, s]]),
                        OP.mult)
                return fn

            def fn_edge(w, q, tbl0, pb):
                """Fused edge mult: t in {0, w-1} of opA (tbl0=T_KR) or opB
                (tbl0=T_IR) into buffer pb."""
                s = R[q] - w
                Dq, base = D[q], bb[q]
                in1base = (T_CL if tbl0 == T_KR else T_CR)
                def fn():
                    v.tensor_tensor(
                        ap_of(z, zoff[(pb, q)],
                              [[w * s, 2], [(w - 1) * s, 2], [1, s]]),
                        ap_of(banks, base + tbl0 * Dq,
                              [[Dq, 2], [(w - 1) * N, 2], [1, s]]),
                        ap_of(banks, base + in1base * Dq + (w - 1) * N + 1,
                              [[Dq, 2], [-(w - 1) * (N - 1), 2], [1, s]]),
                        OP.mult)
                return fn

            def fn_Ared(w, q, pb):
                s = R[q] - w
                def fn():
                    v.reduce_sum(
                        ap_of(z, zoff[("SS", q)], [[s, 2], [1, s]]),
                        ap_of(z, zoff[(pb, q)],
                              [[w * s, 2], [1, s], [s, w]]),
                        axis=AX.X)
                return fn

            def fn_T1(w, q):
                s = R[q] - w
                Dq, base = D[q], bb[q]
                def fn():
                    v.tensor_tensor(
                        ap_of(banks, base + T_IR * Dq + (w - 1) * N,
                              [[-Dq + 1, 2], [1, s]]),
                        ap_of(z, zoff[("SS", q)], [[s, 2], [1, s]]),
                        ap_of(consts, coff[q][w], [[s, 2], [1, s]]),
                        OP.mult)
                return fn

            def fn_opB(w, q, e, pb, t0=0, t1=None):
                s = R[q] - w
                Dq, base = D[q], bb[q]
                tn = (w if t1 is None else t1) - t0
                def fn():
                    eng_of[e].tensor_tensor(
                        ap_of(z, zoff[(pb, q)] + t0 * s,
                              [[w * s, 2], [s, tn], [1, s]]),
                        ap_of(banks, base + T_IR * Dq + t0 * N,
                              [[Dq, 2], [N, tn], [1, s]]),
                        ap_of(banks,
                              base + T_CR * Dq + (w - 1 - t0) * N + 1 + t0,
                              [[Dq, 2], [-(N - 1), tn], [1, s]]),
                        OP.mult)
                return fn

            def fn_Bred(w, q, pb):
                s = R[q] - w
                Dq, base = D[q], bb[q]
                def fn():
                    krout = ap_of(banks, base + T_KR * Dq + w * N,
                                  [[5 * Dq, 2], [1, s]])
                    if w == 1:
                        v.tensor_tensor(
                            krout,
                            ap_of(banks, base + T_IR * Dq,
                                  [[Dq, 2], [1, s]]),
                            ap_of(banks, base + T_CR * Dq + 1,
                                  [[Dq, 2], [1, s]]),
                            OP.mult)
                    else:
                        v.reduce_sum(
                            krout,
                            ap_of(z, zoff[(pb, q)],
                                  [[w * s, 2], [1, s], [s, w]]),
                            axis=AX.X)
                return fn

            def fn_crcl(w, q):
                s = R[q] - w
                Dq, base = D[q], bb[q]
                def fn():
                    v.tensor_tensor(
                        ap_of(banks, base + T_CR * Dq + w * N,
                              [[3 * Dq, 2], [1, s]]),
                        ap_of(banks, base + T_KR * Dq + w * N,
                              [[5 * Dq, 2], [1, s]]),
                        ap_of(stops, 4 * q * N, [[N + w, 2], [1, s]]),
                        OP.mult)
                return fn

            # ---------------- DP graph ----------------
            for q in range(NSLOT):
                G.add(f"RC{q}_1", "a", 2 * (R[q] - 1),
                      [c2dep(q, 1), ms_keys[q][0] if ms_keys[q] else None],
                      fn_ratcopy(1, q))
            for q in range(NSLOT):
                prevC, prev2C = None, None
                kT1prev = None
                for w in range(1, L[q] + 1):
                    s = R[q] - w
                    ea, ebn = MULT_CFG[q]
                    edged = (q in EDGE_QS and 2 <= w <= EDGE_MAX_W)
                    pa = ("PA1" if w % 2 else "PA0") if edged else "PA0"
                    pbuf = ("PB1" if w % 2 else "PB0") if edged else "PA0"
                    rsb = (f"RSB{q}" if q in renorm_qs
                           and RENORM_W + 1 <= w <= RENORM_W + 2 else None)
                    adeps = [f"RC{q}_{w}", prevC, rsb]
                    if w == 1:
                        adeps += [init_k[q]]
                        if ms_keys[q]:
                            adeps += [ms_keys[q][0]]
                    if edged:
                        kAs = [G.add(f"AE{q}_{w}", "v", 4 * s, adeps,
                                     fn_edge(w, q, T_KR, pa))]
                        if w >= 3:
                            kAs.append(G.add(
                                f"A{q}_{w}", "g", 2 * (w - 2) * s,
                                [prev2C, rsb],
                                fn_opA(w, q, "g", pa, 1, w - 1)))
                        rdr = kAs[0]
                    else:
                        kAs = [G.add(f"A{q}_{w}", ea, 2 * w * s, adeps,
                                     fn_opA(w, q, ea, pa))]
                        rdr = kAs[0]
                    if w < L[q]:
                        # WAR: only the edge (or whole) op reads RAT row 0
                        G.add(f"RC{q}_{w + 1}", "a", 2 * (s - 1),
                              [rdr, c2dep(q, w + 1)], fn_ratcopy(w + 1, q))
                    if w > 1:
                        kR1 = G.add(f"R1{q}_{w}", "v", 2 * w * s,
                                    kAs + [kT1prev], fn_Ared(w, q, pa))
                    else:
                        kR1 = kAs[0]
                    t1deps = [kR1, c1dep(q, w)]
                    if q in renorm_qs and w == RENORM_W + 1:
                        t1deps += [f"RSC{q}"]
                    kT1 = G.add(f"T1{q}_{w}", "v", 2 * s, t1deps,
                                fn_T1(w, q))
                    # staged memsets zero bank rows [3,7) and [7,21); the
                    # first DP write into each zone must wait for its chunk
                    r2deps = []
                    if ms_keys[q]:
                        if w == 3:
                            r2deps.append(ms_keys[q][1])
                        elif w == 7:
                            r2deps.append(ms_keys[q][2])
                    if w > 1:
                        if edged:
                            kBs = [G.add(f"BE{q}_{w}", "v", 4 * s,
                                         [kT1, prevC, rsb],
                                         fn_edge(w, q, T_IR, pbuf))]
                            if w >= 3:
                                kBs.append(G.add(
                                    f"B{q}_{w}", "g", 2 * (w - 2) * s,
                                    [kT1prev, prev2C, rsb],
                                    fn_opB(w, q, "g", pbuf, 1, w - 1)))
                        else:
                            kBs = [G.add(f"B{q}_{w}", ebn, 2 * w * s,
                                         [kT1], fn_opB(w, q, ebn, pbuf))]
                        kR2 = G.add(f"R2{q}_{w}", "v", 2 * w * s,
                                    kBs + r2deps, fn_Bred(w, q, pbuf))
                    else:
                        kR2 = G.add(f"R2{q}_{w}", "v", 2 * s,
                                    [kT1] + r2deps, fn_Bred(w, q, pbuf))
                    prev2C = prevC
                    prevC = G.add(f"C{q}_{w}", "v", 2 * s,
                                  [kR2, "dma_stops"], fn_crcl(w, q))
                    kT1prev = kT1
                def xfn(q=q):
                    sc.activation(
                        ap_of(z, zoff["CROUT"] + q * N, [[1, R[q]]]),
                        ap_of(banks, bb[q] + T_CR * D[q], [[N, R[q]]]),
                        AF.Copy)
                G.add(f"X{q}", "a", R[q], [prevC, "ms_crout"], xfn)

            # ---------------- renorm nodes ----------------
            if nq:
                w = RENORM_W

                def fn_rmax():
                    for j, q in enumerate(renorm_qs):
                        s = R[q] - w
                        v.reduce_max(
                            ap_of(z, zoff["M2"] + 2 * j, [[1, 2]]),
                            ap_of(banks, bb[q] + T_KR * D[q] + w * N,
                                  [[5 * D[q], 2], [1, s]]),
                            axis=AX.X)
                    v.tensor_tensor(
                        ap_of(z, zoff["MU"], [[1, nq]]),
                        ap_of(z, zoff["M2"], [[2, nq]]),
                        ap_of(z, zoff["M2"] + 1, [[2, nq]]),
                        OP.max)
                    v.tensor_scalar_mul(
                        ap_of(z, zoff["MU"], [[1, nq]]),
                        ap_of(z, zoff["MU"], [[1, nq]]), 2.0 ** -32)
                    v.tensor_scalar_max(
                        ap_of(z, zoff["MU"], [[1, nq]]),
                        ap_of(z, zoff["MU"], [[1, nq]]), 1e-36)
                G.add("RMAX", "v", 100,
                      [f"R2{q}_{w}" for q in renorm_qs], fn_rmax,
                      fixed=6 * 107.0)

                G.add("RLN", "a", nq, ["RMAX"], lambda: sc.activation(
                    ap_of(z, zoff["LM"], [[1, nq]]),
                    ap_of(z, zoff["MU"], [[1, nq]]), AF.Ln))

                def fn_kchain():
                    # k = round((ln(mu*2^-32) + 32 ln2)/(w ln2)); round via
                    # 1.5*2^23 so every factor is an exact power of two
                    v.tensor_scalar(
                        ap_of(z, zoff["LM"], [[1, nq]]),
                        ap_of(z, zoff["LM"], [[1, nq]]),
                        32.0 * LN2, 1.0 / (w * LN2), OP.add, OP.mult)
                    v.tensor_scalar(
                        ap_of(z, zoff["LM"], [[1, nq]]),
                        ap_of(z, zoff["LM"], [[1, nq]]),
                        12582912.0, 12582912.0, OP.add, OP.subtract)
                    v.tensor_tensor(
                        ap_of(z, zoff["DSUM"], [[1, nq]]),
                        ap_of(z, zoff["DSUM"], [[1, nq]]),
                        ap_of(z, zoff["LM"], [[1, nq]]),
                        OP.add)
                    # scale2 = 2^-k via exponent bits: (127 - k) << 23
                    v.tensor_scalar(
                        ap_of(z, zoff["M2"], [[1, nq]]),
                        ap_of(z, zoff["LM"], [[1, nq]]),
                        -1.0, 127.0, OP.mult, OP.add)
                    zi = z.bitcast(mybir.dt.int32)
                    v.tensor_copy(
                        ap_of(zi, zoff["M2"] + 4, [[1, nq]]),
                        ap_of(z, zoff["M2"], [[1, nq]]))
                    v.tensor_scalar(
                        ap_of(zi, zoff["M2"] + 4, [[1, nq]]),
                        ap_of(zi, zoff["M2"] + 4, [[1, nq]]),
                        23, None, OP.arith_shift_left)
                    # M[j, d] = 2^(-k*d): d=0 -> 1, multiplicative scan
                    v.memset(ap_of(z, zoff["M"], [[N + 1, nq], [1, 1]]), 1.0)
                    for j in range(nq):
                        sca = ap_of(z, zoff["M2"] + 4 + j, [[0, N]])
                        v.tensor_tensor_scan(
                            ap_of(z, zoff["M"] + j * (N + 1) + 1, [[1, N]]),
                            sca, sca, 1.0, OP.mult, OP.bypass)
                G.add("KCH", "v", 60, ["RLN", "ms_dsum"], fn_kchain,
                      fixed=9 * 107.0)

                for j, q in enumerate(renorm_qs):
                    Dq, base = D[q], bb[q]
                    mrow = zoff["M"] + j * (N + 1)
                    e = "g" if q == 0 else "v"

                    def fn_rsb(q=q, Dq=Dq, base=base, mrow=mrow, e=e):
                        def fn():
                            for tb in (T_KR, T_CL):
                                eng_of[e].tensor_tensor(
                                    ap_of(banks, base + tb * Dq,
                                          [[Dq, 2], [N, w + 1], [1, N]]),
                                    ap_of(banks, base + tb * Dq,
                                          [[Dq, 2], [N, w + 1], [1, N]]),
                                    ap_of(z, mrow,
                                          [[0, 2], [1, w + 1], [0, N]]),
                                    OP.mult)
                            eng_of[e].tensor_tensor(
                                ap_of(banks, base + T_IL * Dq,
                                      [[Dq, 2], [N, w], [1, N]]),
                                ap_of(banks, base + T_IL * Dq,
                                      [[Dq, 2], [N, w], [1, N]]),
                                ap_of(z, mrow + 1,
                                      [[0, 2], [1, w], [0, N]]),
                                OP.mult)
                        return fn
                    G.add(f"RSB{q}", e, 6 * (w + 1) * N,
                          ["KCH", f"C{q}_{w}", f"RC{q}_{w + 1}"],
                          fn_rsb(), fixed=3 * G.EST[e][1])

                    def fn_rsc(q=q, mrow=mrow):
                        lo = coff[q][w + 1]
                        hi = coff[q][L[q]] + 2 * (R[q] - L[q])
                        def fn():
                            v.tensor_tensor(
                                ap_of(consts, lo, [[1, hi - lo]]),
                                ap_of(consts, lo, [[1, hi - lo]]),
                                ap_of(z, mrow + 1, [[0, hi - lo]]),
                                OP.mult)
                        return fn
                    G.add(f"RSC{q}", "v",
                          coff[q][L[q]] + 2 * (R[q] - L[q])
                          - coff[q][w + 1],
                          ["KCH", "dma_tail"], fn_rsc())

            # ---------------- output DMA (per slot, overlaps the rest) ----
            for q in range(NSLOT):
                def efn(q=q):
                    nc.sync.dma_start(
                        ap_of(ecr_d, q * N, [[1, N]],
                              lead=[NSLOT * N, P128]),
                        ap_of(z, zoff["CROUT"] + q * N, [[1, N]]))
                G.add(f"dma_ecr{q}", "d", N * 4, [f"X{q}"], efn)
            G.add("dma_dsum", "d", NSLOT * 4,
                  ["KCH" if nq else "ms_dsum"], lambda: (
                      nc.sync.dma_start(
                          ap_of(dsum_d, 0, [[1, NSLOT]],
                                lead=[NSLOT, P128]),
                          ap_of(z, zoff["DSUM"], [[1, NSLOT]]))))

            G.emit()

    nc.compile()
    return nc


_NC_CACHE = {}


def get_nc(bounds):
    key = tuple(bounds)
    if key not in _NC_CACHE:
        _NC_CACHE[key] = build_nc(key)
    return _NC_CACHE[key]


def plan(len_array):
    """Sort sentences by length desc, deal round-robin to cores, slot into
    4 groups of 128 per core. Returns (order, bounds) where order[r] is the
    original sentence index of global sorted rank r and bounds[q] is the
    width bound of slot q (same for every core by round-robin construction).
    """
    ln = np.asarray(len_array).astype(np.int64)
    order = np.argsort(-ln, kind="stable")
    bounds = [int(ln[order[min(1024 * q, len(ln) - 1)]])
              for q in range(NSLOT)]
    bounds = [max(b, 1) for b in bounds]
    return order, bounds


def make_in_maps(trans_scores, dec_scores, len_array):
    t = np.asarray(trans_scores, dtype=np.float32)
    dec = np.asarray(dec_scores, dtype=np.float32)
    B = t.shape[0]
    order, bounds = plan(len_array)
    lay = _layout(bounds)
    go = dec[..., 0]                        # [B, n, dir, dv]
    # per-sentence linear pre-shift: each arc factor carries exp(-c0), so a
    # width-w entry is scaled exp(-c0*w); undone on the host at the end.
    tm = np.where(t < -1e8, -np.inf, t).max(axis=3)
    with np.errstate(invalid="ignore"):
        colmax = tm.max(axis=1)             # [B, n] best arc into each child
        proxy = np.nanmean(
            np.where(np.isfinite(colmax), colmax, np.nan)[:, 1:], axis=-1)
    c0 = (proxy + 0.5).astype(np.float32)
    c0 = np.clip(np.nan_to_num(c0), -20.0, 20.0)
    # one exp over trans (NEG -> 0 underflow is intended), then gather diags
    with np.errstate(under="ignore"):
        E = np.exp(t - c0[:, None, None, None])      # [B, n, n, 2]
        ego = np.exp(go)                             # [B, n, 2, 2]
    d_idx, i_idx = np.meshgrid(np.arange(N), np.arange(N), indexing="ij")
    j_idx = np.minimum(i_idx + d_idx, N - 1)
    valid = ((i_idx + d_idx) <= N - 1)[None].astype(np.float32)
    ea = E[:, i_idx, j_idx, :]              # [B, n, n, 2]  trans[i, i+d, v]
    eb = E[:, j_idx, i_idx, :]              # [B, n, n, 2]  trans[i+d, i, v]
    a1 = ea[..., 1] * ego[:, :, 1, 1][:, i_idx] * valid   # [B, d, i]
    a0 = ea[..., 0] * ego[:, :, 1, 0][:, i_idx] * valid
    b1 = eb[..., 1] * ego[:, :, 0, 1][:, j_idx] * valid
    b0 = eb[..., 0] * ego[:, :, 0, 0][:, j_idx] * valid
    est = np.exp(dec[..., 1])               # [B, n, dir, dv]

    # sentence index per (core, slot, partition)
    sent = order.reshape(-1, NCORES).T.reshape(NCORES, NSLOT, P128)

    # valence-edge ratios (every packed cell is a valid arc, so a1,b1 > 0)
    with np.errstate(divide="ignore", invalid="ignore"):
        ra = (a0.astype(np.float64) / a1).astype(np.float32)
        rb = (b0.astype(np.float64) / b1).astype(np.float32)
    consts = np.empty((NCORES, P128, lay["const_total"]), dtype=np.float32)
    for q in range(NSLOT):
        sq = sent[:, q, :]                  # [NCORES, P128]
        for w in range(1, bounds[q] + 1):
            s = bounds[q] + 1 - w
            o = lay["coff"][q][w]
            consts[:, :, o:o + s] = a1[sq, w, :s]
            consts[:, :, o + s:o + 2 * s] = b1[sq, w, :s]
            o2 = lay["c2off"][q][w]
            consts[:, :, o2:o2 + s] = ra[sq, w, :s]
            consts[:, :, o2 + s:o2 + 2 * s] = rb[sq, w, :s]
    for q in range(NSLOT):
        o = 4 * q * N
        sq = sent[:, q, :]
        consts[:, :, o:o + N] = est[sq][:, :, :, 1, 1]          # SRHAS
        consts[:, :, o + N:o + 2 * N] = est[sq][:, :, :, 0, 1]  # SLHAS
        consts[:, :, o + 2 * N:o + 3 * N] = est[sq][:, :, :, 1, 0]  # SRNO
        consts[:, :, o + 3 * N:o + 4 * N] = est[sq][:, :, :, 0, 0]  # SLNO
    in_maps = [{"consts": consts[c]} for c in range(NCORES)]
    aux = dict(c0=c0, order=order, bounds=bounds, sent=sent)
    return in_maps, aux


def assemble(results, len_array, aux):
    ln = np.asarray(len_array).astype(np.int64)
    c0 = np.asarray(aux["c0"]).astype(np.float64)
    sent = aux["sent"]
    out = np.empty(len(ln), dtype=np.float32)
    for c, res in enumerate(results):
        ecr = res["ecr"].reshape(P128, NSLOT * N).astype(np.float64)
        dsum = res["dsum"].reshape(P128, NSLOT).astype(np.float64)
        for q in range(NSLOT):
            idx = sent[c, q]                # original sentence ids [P128]
            lc = ln[idx]
            with np.errstate(divide="ignore"):
                out[idx] = (
                    np.log(ecr[np.arange(P128), q * N + lc])
                    + dsum[:, q] * LN2 * lc + c0[idx] * lc
                ).astype(np.float32)
    return out


def kernel(trans_scores, dec_scores, len_array):
    from concourse.bass_utils import run_bass_kernel_spmd

    in_maps, aux = make_in_maps(trans_scores, dec_scores, len_array)
    nc = get_nc(aux["bounds"])
    res = run_bass_kernel_spmd(nc, in_maps, core_ids=list(range(NCORES)))
    return assemble(res.results, len_array, aux)


# revision 54
# speedup vs baseline: 3.0339x; 1.0106x over previous
"""DMV inside algorithm (Eisner chart DP, logsumexp semiring) on Trainium2.

Strategy
--------
Data parallel over the batch: 4096 sentences -> 8 cores x 512, with
length-aware packing: sentences are sorted by length (desc) and dealt
round-robin to cores, then within a core split into 4 slots of 128
(one sentence per SBUF partition per slot). Slot q only runs chart
widths w <= L[q] (its max length), cutting DP work ~2.4x vs running
all sentences to width 40.

The DP runs in the *exp domain* (tables hold exp(score)); each width-w
update is a fused strided multiply + segmented reduce. Segmented
reduces only exist on VectorE (DVE); multiplies are split between DVE
and GpSimd (Pool); the ACT engine rewrites the valence-ratio rows.
All ops are built as an explicit dependency graph and ordered by a
critical-path list scheduler before emission, because every engine
executes its stream strictly in order (a semaphore wait at the head
blocks everything behind it).

Per slot q the 6 diag-packed tables [R=L+1 rows x 41] live at stride
D=41*R in the order [KR, CR, IL, IR, CL, KL], chosen so every fused
operand pair is adjacent: opA in0={KR,CR}, in1={CL,KL}; opB in0={IR,CL},
in1={CR,IL}; outputs {IR,IL}, {KR,KL}, {CR,CL} all constant-stride.
IR/IL row r holds width r+1; IL/opB patterns are column-shifted so all
gathers are constant-stride.

KR/KL row 0 hold the valence edge RATIOS (RA=a0/a1 at KR[0,i],
RB=b0/b1 at KL[0,i+w]), rewritten per width by the otherwise idle ACT
engine; opA's natural w-term gather then covers both valence edge
cases exactly, once IR/IL is scaled by {A1,B1}. Per-arc constants live
in two triangular packs: tri1={A1,B1} (renorm-rescaled), tri2={RA,RB}
(scale-free).

Numerics: scale composes linearly in span width, so slots with L >= 25
renormalize once at w=20: row d of every table is multiplied by an
exact power of two 2^(-k*d) (k integer per sentence), k accumulated in
dsum and undone on the host: LL = log(CR[0,len]) + k*ln2*len + c0*len.
"""

import os

os.environ.setdefault("JAX_PLATFORMS", "cpu")

import heapq

import numpy as np

import concourse.bass as bass  # noqa: F401  (registers engine classes)
import concourse.tile as tile
import bass_rust
from concourse import bacc, mybir

F32 = mybir.dt.float32
AF = mybir.ActivationFunctionType
OP = mybir.AluOpType
AX = mybir.AxisListType

N = 41              # fake_len (ROOT at 0)
NCORES = 8
NSLOT = 4
P128 = 128
B_CORE = NSLOT * P128
RENORM_W = 20       # renorm width (slots with L >= RENORM_MIN_L)
RENORM_MIN_L = 25
HEAD_W = 6          # consts rows <= HEAD_W form the startup DMA region
EDGE_MAX_W = 36     # widths above this run the whole chain on DVE

# table order within a slot (pairs used by the fused ops are adjacent)
T_KR, T_CR, T_IL, T_IR, T_CL, T_KL = range(6)

LN2 = float(np.log(2.0))

# mult-engine assignment per slot: (opA engine, opB engine), 'v'=DVE 'g'=Pool
MULT_CFG = {0: ("v", "v"), 1: ("v", "g"), 2: ("v", "v"), 3: ("v", "v")}
# slots using the edge/main decomposition: the big mults' t∈[1,w-1) bulk
# only depends on width w-2, so it runs a width ahead on Pool while the
# dependency-carrying edge terms (t=0, t=w-1) are tiny [2,2,s] DVE ops.
EDGE_QS = (0,)


def ap_of(t, offset, dims, lead=None):
    """Build a raw AP on tile/dram ap `t`: [lead or t.ap[0]] + dims."""
    ap = t.copy()
    first = list(t.ap[0]) if lead is None else list(lead)
    ap.ap = bass_rust.VecI64Pair([first] + [list(d) for d in dims])
    ap.offset = offset
    return ap


def _layout(bounds):
    """Compute per-slot offsets for banks / consts / stops / scratch."""
    L = list(bounds)
    R = [l + 1 for l in L]
    D = [N * r for r in R]
    bank_base, acc = [], 0
    for q in range(NSLOT):
        bank_base.append(acc)
        acc += 6 * D[q]
    bank_total = acc
    # consts, two triangles per slot: tri1 per (q,w) = [A1(s), B1(s)]
    # (renorm-rescaled) and tri2 per (q,w) = [RA(s), RB(s)] (scale-free
    # ratios). Rows w <= HEAD_W live in a contiguous head region so ONE
    # DMA covers everything the first DP widths need; the rest is one
    # tail DMA. Within each region blocks are (q, tri, w)-ordered, so any
    # (q, tri, w-range) stays contiguous (the renorm tail rescale relies
    # on rows >= RENORM_W+1 of a slot's tri1 being contiguous).
    coff = [dict() for _ in range(NSLOT)]
    c2off = [dict() for _ in range(NSLOT)]
    # stop vectors lead the head region: per q [SRHAS, SLHAS, SRNO, SLNO] x 41
    stop_total = NSLOT * 4 * N
    acc = stop_total
    for wlo, whi in ((1, HEAD_W), (HEAD_W + 1, N)):
        for q in range(NSLOT):
            for offs in (coff, c2off):
                for w in range(wlo, min(whi, L[q]) + 1):
                    offs[q][w] = acc
                    acc += 2 * (R[q] - w)
        if wlo == 1:
            head_end = acc
    const_total = acc
    # z scratch
    z = {}
    zacc = 0
    for q in range(NSLOT):
        pmax = max((2 * (R[q] - w) * w for w in range(1, L[q] + 1)),
                   default=2)
        for pb in ("PA0", "PA1", "PB0", "PB1"):
            z[(pb, q)] = zacc; zacc += pmax
        z[("SS", q)] = zacc; zacc += 2 * N
        z[("T1", q)] = zacc; zacc += 2 * N
    z["CROUT"] = zacc; zacc += NSLOT * N
    z["DSUM"] = zacc; zacc += NSLOT
    z["M2"] = zacc; zacc += 8
    z["MU"] = zacc; zacc += 2
    z["LM"] = zacc; zacc += 2
    z["M"] = zacc; zacc += NSLOT * (N + 1)  # renorm multiplier rows [q, 42]
    z_total = zacc
    return dict(L=L, R=R, D=D, bank_base=bank_base, bank_total=bank_total,
                coff=coff, c2off=c2off, head_end=head_end,
                const_total=const_total,
                stop_total=stop_total, z=z, z_total=z_total)


class Graph:
    """Op graph + critical-path list scheduler for in-order engines."""

    EST = {"v": (1.0417, 107.0), "g": (1.984, 156.0), "a": (0.833, 217.0),
           "d": (0.386, 1600.0)}  # (ns/elem, fixed ns); d: per byte
    XPEN = 100.0  # cross-engine semaphore latency
    SLACK = 100.0  # earliest-start tolerance when picking by priority

    def __init__(self):
        self.nodes = []          # dict(key, eng, est, deps, fn)
        self.byname = {}

    def add(self, key, eng, elems, deps, fn, fixed=None):
        slope, fix = self.EST[eng]
        est = elems * slope + (fix if fixed is None else fixed)
        n = dict(key=key, eng=eng, est=est, deps=[d for d in deps
                                                 if d is not None], fn=fn,
                 idx=len(self.nodes))
        self.nodes.append(n)
        self.byname[key] = n
        return key

    def schedule(self):
        """Earliest-start list scheduling with critical-path tie-break.
        Returns node indices in chosen global emission order."""
        nodes = self.nodes
        nn = len(nodes)
        succ = [[] for _ in range(nn)]
        npred = [0] * nn
        for n in nodes:
            for dk in n["deps"]:
                d = self.byname[dk]
                succ[d["idx"]].append(n["idx"])
                npred[n["idx"]] += 1
        # critical-path priority (longest path to sink)
        prio = [0.0] * nn
        for i in reversed(range(nn)):  # nodes added roughly topologically;
            pass
        order_topo = []
        tmp_pred = npred[:]
        stack = [i for i in range(nn) if tmp_pred[i] == 0]
        while stack:
            i = stack.pop()
            order_topo.append(i)
            for j in succ[i]:
                tmp_pred[j] -= 1
                if tmp_pred[j] == 0:
                    stack.append(j)
        assert len(order_topo) == nn, "cycle in op graph"
        for i in reversed(order_topo):
            best = 0.0
            for j in succ[i]:
                if prio[j] > best:
                    best = prio[j]
            prio[i] = nodes[i]["est"] + best
        # event-driven greedy
        free = {"v": 0.0, "g": 0.0, "a": 0.0, "d": 0.0}
        finish = [0.0] * nn
        ready_t = [0.0] * nn
        npred2 = npred[:]
        ready = [i for i in range(nn) if npred2[i] == 0]
        out = []
        while ready:
            # candidate est-start for each ready node
            best_i, best_start, best_prio = None, None, None
            min_start = min(max(free[nodes[i]["eng"]], ready_t[i])
                            for i in ready)
            for i in ready:
                e = nodes[i]["eng"]
                st = max(free[e], ready_t[i])
                if st <= min_start + self.SLACK:
                    if best_prio is None or prio[i] > best_prio:
                        best_i, best_start, best_prio = i, st, prio[i]
            i = best_i
            ready.remove(i)
            n = nodes[i]
            e = n["eng"]
            st = best_start
            fin = st + n["est"]
            finish[i] = fin
            free[e] = fin
            out.append((st, i))
            for j in succ[i]:
                pen = self.XPEN if nodes[j]["eng"] != e else 0.0
                ready_t[j] = max(ready_t[j], fin + pen)
                npred2[j] -= 1
                if npred2[j] == 0:
                    ready.append(j)
        assert len(out) == nn
        self.makespan = max(finish) if nn else 0.0
        out.sort(key=lambda t: (t[0], t[1]))
        return [i for _, i in out]

    def emit(self):
        order = self.schedule()
        global LAST_MAKESPAN
        LAST_MAKESPAN = self.makespan
        for i in order:
            self.nodes[i]["fn"]()


LAST_MAKESPAN = None


def build_nc(bounds):
    lay = _layout(bounds)
    L, R, D = lay["L"], lay["R"], lay["D"]
    bb = lay["bank_base"]
    coff = lay["coff"]
    c2off = lay["c2off"]
    zoff = lay["z"]
    renorm_qs = [q for q in range(NSLOT) if L[q] >= RENORM_MIN_L]
    # bounds are sorted desc, so renorm slots are the prefix [0..nq) and
    # dsum[j] lines up with slot j
    assert renorm_qs == list(range(len(renorm_qs)))
    nq = len(renorm_qs)

    nc = bacc.Bacc("TRN2", target_bir_lowering=False, debug=False,
                   num_devices=1)
    consts_in = nc.dram_tensor(
        "consts", [P128, lay["const_total"]], F32, kind="ExternalInput").ap()
    ecr_d = nc.dram_tensor(
        "ecr", [P128, NSLOT * N], F32, kind="ExternalOutput").ap()
    dsum_d = nc.dram_tensor(
        "dsum", [P128, NSLOT], F32, kind="ExternalOutput").ap()

    with tile.TileContext(nc) as tc:
        with tc.tile_pool(name="p", bufs=1) as pool:
            banks_t = pool.tile([P128, lay["bank_total"]], F32)
            consts_t = pool.tile([P128, lay["const_total"]], F32)
            z_t = pool.tile([P128, lay["z_total"]], F32)
            banks = banks_t[:]
            consts = consts_t[:]
            stops = consts  # stop vectors lead the consts head region
            z = z_t[:]

            v = nc.vector
            g = nc.gpsimd
            sc = nc.scalar
            eng_of = {"v": v, "g": g}
            G = Graph()

            # ---------------- input DMA nodes ----------------
            def dma_in(dst_off, src_off, size):
                def fn():
                    nc.sync.dma_start(
                        ap_of(consts, dst_off, [[1, size]]),
                        ap_of(consts_in, src_off, [[1, size]],
                              lead=[lay["const_total"], P128]))
                return fn

            he = lay["head_end"]
            G.add("dma_head", "d", he * 4, [], dma_in(0, 0, he))
            G.add("dma_tail", "d", (lay["const_total"] - he) * 4, [],
                  dma_in(he, he, lay["const_total"] - he))
            G.byname["dma_stops"] = G.byname["dma_head"]

            def c1dep(q, w):
                return "dma_head" if w <= HEAD_W else "dma_tail"

            c2dep = c1dep

            # ---------------- init nodes ----------------
            ms_keys = {q: [] for q in range(NSLOT)}
            for q in renorm_qs:
                for ci, (r0, r1) in enumerate(
                        ((0, 3), (3, 7), (7, RENORM_W + 1))):
                    def mfn(q=q, r0=r0, r1=r1):
                        g.memset(
                            ap_of(banks, bb[q] + r0 * N,
                                  [[D[q], 6], [1, (r1 - r0) * N]]), 0.0)
                    ms_keys[q].append(G.add(
                        f"ms{q}_{ci}", "g", 6 * (r1 - r0) * N, [], mfn))
            G.add("ms_dsum", "a", NSLOT, [], lambda: sc.memzero(
                ap_of(z, zoff["DSUM"], [[1, NSLOT]])))
            G.add("ms_crout", "a", NSLOT * N, [], lambda: sc.memzero(
                ap_of(z, zoff["CROUT"], [[1, NSLOT * N]])))
            init_k = {}
            for q in range(NSLOT):
                def ifn(q=q):
                    v.tensor_copy(
                        ap_of(banks, bb[q] + T_CR * D[q],
                              [[3 * D[q], 2], [1, N]]),
                        ap_of(stops, (4 * q + 2) * N, [[N, 2], [1, N]]))
                init_k[q] = G.add(f"init{q}", "v", 2 * N,
                                  ["dma_stops"] + ms_keys[q][:1], ifn)

            # ---------------- DP op builders ----------------
            def fn_ratcopy(w, q):
                s = R[q] - w
                Dq, base = D[q], bb[q]
                def fn():
                    sc.activation(
                        ap_of(banks, base + T_KR * Dq,
                              [[5 * Dq + w, 2], [1, s]]),
                        ap_of(consts, c2off[q][w], [[s, 2], [1, s]]),
                        AF.Copy)
                return fn

            def fn_opA(w, q, e, pb, t0=0, t1=None):
                """t-slice [t0, t1) of the opA product into buffer pb."""
                s = R[q] - w
                Dq, base = D[q], bb[q]
                tn = (w if t1 is None else t1) - t0
                def fn():
                    out = (ap_of(z, zoff[("SS", q)],
                                 [[s, 2], [1, 1], [1, s]])
                           if w == 1 else
                           ap_of(z, zoff[(pb, q)] + t0 * s,
                                 [[w * s, 2], [s, tn], [1, s]]))
                    eng_of[e].tensor_tensor(
                        out,
                        ap_of(banks, base + T_KR * Dq + t0 * N,
                              [[Dq, 2], [N, tn], [1, s]]),
                        ap_of(banks,
                              base + T_CL * Dq + (w - 1 - t0) * N + 1 + t0,
                              [[Dq, 2], [-(N - 1), tn], [1, s]]),
                        OP.mult)
                return fn

            def fn_edge(w, q, tbl0, pb):
                """Fused edge mult: t in {0, w-1} of opA (tbl0=T_KR) or opB
                (tbl0=T_IR) into buffer pb."""
                s = R[q] - w
                Dq, base = D[q], bb[q]
                in1base = (T_CL if tbl0 == T_KR else T_CR)
                def fn():
                    v.tensor_tensor(
                        ap_of(z, zoff[(pb, q)],
                              [[w * s, 2], [(w - 1) * s, 2], [1, s]]),
                        ap_of(banks, base + tbl0 * Dq,
                              [[Dq, 2], [(w - 1) * N, 2], [1, s]]),
                        ap_of(banks, base + in1base * Dq + (w - 1) * N + 1,
                              [[Dq, 2], [-(w - 1) * (N - 1), 2], [1, s]]),
                        OP.mult)
                return fn

            def fn_Ared(w, q, pb):
                s = R[q] - w
                def fn():
                    v.reduce_sum(
                        ap_of(z, zoff[("SS", q)], [[s, 2], [1, s]]),
                        ap_of(z, zoff[(pb, q)],
                              [[w * s, 2], [1, s], [s, w]]),
                        axis=AX.X)
                return fn

            def fn_T1(w, q):
                s = R[q] - w
                Dq, base = D[q], bb[q]
                def fn():
                    v.tensor_tensor(
                        ap_of(banks, base + T_IR * Dq + (w - 1) * N,
                              [[-Dq + 1, 2], [1, s]]),
                        ap_of(z, zoff[("SS", q)], [[s, 2], [1, s]]),
                        ap_of(consts, coff[q][w], [[s, 2], [1, s]]),
                        OP.mult)
                return fn

            def fn_opB(w, q, e, pb, t0=0, t1=None):
                s = R[q] - w
                Dq, base = D[q], bb[q]
                tn = (w if t1 is None else t1) - t0
                def fn():
                    eng_of[e].tensor_tensor(
                        ap_of(z, zoff[(pb, q)] + t0 * s,
                              [[w * s, 2], [s, tn], [1, s]]),
                        ap_of(banks, base + T_IR * Dq + t0 * N,
                              [[Dq, 2], [N, tn], [1, s]]),
                        ap_of(banks,
                              base + T_CR * Dq + (w - 1 - t0) * N + 1 + t0,
                              [[Dq, 2], [-(N - 1), tn], [1, s]]),
                        OP.mult)
                return fn

            def fn_Bred(w, q, pb):
                s = R[q] - w
                Dq, base = D[q], bb[q]
                def fn():
                    krout = ap_of(banks, base + T_KR * Dq + w * N,
                                  [[5 * Dq, 2], [1, s]])
                    if w == 1:
                        v.tensor_tensor(
                            krout,
                            ap_of(banks, base + T_IR * Dq,
                                  [[Dq, 2], [1, s]]),
                            ap_of(banks, base + T_CR * Dq + 1,
                                  [[Dq, 2], [1, s]]),
                            OP.mult)
                    else:
                        v.reduce_sum(
                            krout,
                            ap_of(z, zoff[(pb, q)],
                                  [[w * s, 2], [1, s], [s, w]]),
                            axis=AX.X)
                return fn

            def fn_crcl(w, q):
                s = R[q] - w
                Dq, base = D[q], bb[q]
                def fn():
                    v.tensor_tensor(
                        ap_of(banks, base + T_CR * Dq + w * N,
                              [[3 * Dq, 2], [1, s]]),
                        ap_of(banks, base + T_KR * Dq + w * N,
                              [[5 * Dq, 2], [1, s]]),
                        ap_of(stops, 4 * q * N, [[N + w, 2], [1, s]]),
                        OP.mult)
                return fn

            # ---------------- DP graph ----------------
            for q in range(NSLOT):
                G.add(f"RC{q}_1", "a", 2 * (R[q] - 1),
                      [c2dep(q, 1), ms_keys[q][0] if ms_keys[q] else None],
                      fn_ratcopy(1, q))
            for q in range(NSLOT):
                prevC, prev2C = None, None
                kT1prev = None
                for w in range(1, L[q] + 1):
                    s = R[q] - w
                    ea, ebn = MULT_CFG[q]
                    edged = (q in EDGE_QS and 2 <= w <= EDGE_MAX_W)
                    pa = ("PA1" if w % 2 else "PA0") if edged else "PA0"
                    pbuf = ("PB1" if w % 2 else "PB0") if edged else "PA0"
                    rsb = (f"RSB{q}" if q in renorm_qs
                           and RENORM_W + 1 <= w <= RENORM_W + 2 else None)
                    adeps = [f"RC{q}_{w}", prevC, rsb]
                    if w == 1:
                        adeps += [init_k[q]]
                        if ms_keys[q]:
                            adeps += [ms_keys[q][0]]
                    if edged:
                        kAs = [G.add(f"AE{q}_{w}", "v", 4 * s, adeps,
                                     fn_edge(w, q, T_KR, pa))]
                        if w >= 3:
                            kAs.append(G.add(
                                f"A{q}_{w}", "g", 2 * (w - 2) * s,
                                [prev2C, rsb],
                                fn_opA(w, q, "g", pa, 1, w - 1)))
                        rdr = kAs[0]
                    else:
                        kAs = [G.add(f"A{q}_{w}", ea, 2 * w * s, adeps,
                                     fn_opA(w, q, ea, pa))]
                        rdr = kAs[0]
                    if w < L[q]:
                        # WAR: only the edge (or whole) op reads RAT row 0
                        G.add(f"RC{q}_{w + 1}", "a", 2 * (s - 1),
                              [rdr, c2dep(q, w + 1)], fn_ratcopy(w + 1, q))
                    if w > 1:
                        kR1 = G.add(f"R1{q}_{w}", "v", 2 * w * s,
                                    kAs + [kT1prev], fn_Ared(w, q, pa))
                    else:
                        kR1 = kAs[0]
                    t1deps = [kR1, c1dep(q, w)]
                    if q in renorm_qs and w == RENORM_W + 1:
                        t1deps += [f"RSC{q}"]
                    kT1 = G.add(f"T1{q}_{w}", "v", 2 * s, t1deps,
                                fn_T1(w, q))
                    # staged memsets zero bank rows [3,7) and [7,21); the
                    # first DP write into each zone must wait for its chunk
                    r2deps = []
                    if ms_keys[q]:
                        if w == 3:
                            r2deps.append(ms_keys[q][1])
                        elif w == 7:
                            r2deps.append(ms_keys[q][2])
                    if w > 1:
                        if edged:
                            kBs = [G.add(f"BE{q}_{w}", "v", 4 * s,
                                         [kT1, prevC, rsb],
                                         fn_edge(w, q, T_IR, pbuf))]
                            if w >= 3:
                                kBs.append(G.add(
                                    f"B{q}_{w}", "g", 2 * (w - 2) * s,
                                    [kT1prev, prev2C, rsb],
                                    fn_opB(w, q, "g", pbuf, 1, w - 1)))
                        else:
                            kBs = [G.add(f"B{q}_{w}", ebn, 2 * w * s,
                                         [kT1], fn_opB(w, q, ebn, pbuf))]
                        kR2 = G.add(f"R2{q}_{w}", "v", 2 * w * s,
                                    kBs + r2deps, fn_Bred(w, q, pbuf))
                    else:
                        kR2 = G.add(f"R2{q}_{w}", "v", 2 * s,
                                    [kT1] + r2deps, fn_Bred(w, q, pbuf))
                    prev2C = prevC
                    prevC = G.add(f"C{q}_{w}", "v", 2 * s,
                                  [kR2, "dma_stops"], fn_crcl(w, q))
                    kT1prev = kT1
                def xfn(q=q):
                    sc.activation(
                        ap_of(z, zoff["CROUT"] + q * N, [[1, R[q]]]),
                        ap_of(banks, bb[q] + T_CR * D[q], [[N, R[q]]]),
                        AF.Copy)
                G.add(f"X{q}", "a", R[q], [prevC, "ms_crout"], xfn)

            # ---------------- renorm nodes ----------------
            if nq:
                w = RENORM_W

                def fn_rmax():
                    for j, q in enumerate(renorm_qs):
                        s = R[q] - w
                        v.reduce_max(
                            ap_of(z, zoff["M2"] + 2 * j, [[1, 2]]),
                            ap_of(banks, bb[q] + T_KR * D[q] + w * N,
                                  [[5 * D[q], 2], [1, s]]),
                            axis=AX.X)
                    v.tensor_tensor(
                        ap_of(z, zoff["MU"], [[1, nq]]),
                        ap_of(z, zoff["M2"], [[2, nq]]),
                        ap_of(z, zoff["M2"] + 1, [[2, nq]]),
                        OP.max)
                    v.tensor_scalar_mul(
                        ap_of(z, zoff["MU"], [[1, nq]]),
                        ap_of(z, zoff["MU"], [[1, nq]]), 2.0 ** -32)
                    v.tensor_scalar_max(
                        ap_of(z, zoff["MU"], [[1, nq]]),
                        ap_of(z, zoff["MU"], [[1, nq]]), 1e-36)
                G.add("RMAX", "v", 100,
                      [f"R2{q}_{w}" for q in renorm_qs], fn_rmax,
                      fixed=6 * 107.0)

                G.add("RLN", "a", nq, ["RMAX"], lambda: sc.activation(
                    ap_of(z, zoff["LM"], [[1, nq]]),
                    ap_of(z, zoff["MU"], [[1, nq]]), AF.Ln))

                def fn_kchain():
                    # k = round((ln(mu*2^-32) + 32 ln2)/(w ln2)); round via
                    # 1.5*2^23 so every factor is an exact power of two
                    v.tensor_scalar(
                        ap_of(z, zoff["LM"], [[1, nq]]),
                        ap_of(z, zoff["LM"], [[1, nq]]),
                        32.0 * LN2, 1.0 / (w * LN2), OP.add, OP.mult)
                    v.tensor_scalar(
                        ap_of(z, zoff["LM"], [[1, nq]]),
                        ap_of(z, zoff["LM"], [[1, nq]]),
                        12582912.0, 12582912.0, OP.add, OP.subtract)
                    v.tensor_tensor(
                        ap_of(z, zoff["DSUM"], [[1, nq]]),
                        ap_of(z, zoff["DSUM"], [[1, nq]]),
                        ap_of(z, zoff["LM"], [[1, nq]]),
                        OP.add)
                    # scale2 = 2^-k via exponent bits: (127 - k) << 23
                    v.tensor_scalar(
                        ap_of(z, zoff["M2"], [[1, nq]]),
                        ap_of(z, zoff["LM"], [[1, nq]]),
                        -1.0, 127.0, OP.mult, OP.add)
                    zi = z.bitcast(mybir.dt.int32)
                    v.tensor_copy(
                        ap_of(zi, zoff["M2"] + 4, [[1, nq]]),
                        ap_of(z, zoff["M2"], [[1, nq]]))
                    v.tensor_scalar(
                        ap_of(zi, zoff["M2"] + 4, [[1, nq]]),
                        ap_of(zi, zoff["M2"] + 4, [[1, nq]]),
                        23, None, OP.arith_shift_left)
                    # M[j, d] = 2^(-k*d): d=0 -> 1, multiplicative scan
                    v.memset(ap_of(z, zoff["M"], [[N + 1, nq], [1, 1]]), 1.0)
                    for j in range(nq):
                        sca = ap_of(z, zoff["M2"] + 4 + j, [[0, N]])
                        v.tensor_tensor_scan(
                            ap_of(z, zoff["M"] + j * (N + 1) + 1, [[1, N]]),
                            sca, sca, 1.0, OP.mult, OP.bypass)
                G.add("KCH", "v", 60, ["RLN", "ms_dsum"], fn_kchain,
                      fixed=9 * 107.0)

                for j, q in enumerate(renorm_qs):
                    Dq, base = D[q], bb[q]
                    mrow = zoff["M"] + j * (N + 1)
                    e = "g" if q == 0 else "v"

                    def fn_rsb(q=q, Dq=Dq, base=base, mrow=mrow, e=e):
                        def fn():
                            for tb in (T_KR, T_CL):
                                eng_of[e].tensor_tensor(
                                    ap_of(banks, base + tb * Dq,
                                          [[Dq, 2], [N, w + 1], [1, N]]),
                                    ap_of(banks, base + tb * Dq,
                                          [[Dq, 2], [N, w + 1], [1, N]]),
                                    ap_of(z, mrow,
                                          [[0, 2], [1, w + 1], [0, N]]),
                                    OP.mult)
                            eng_of[e].tensor_tensor(
                                ap_of(banks, base + T_IL * Dq,
                                      [[Dq, 2], [N, w], [1, N]]),
                                ap_of(banks, base + T_IL * Dq,
                                      [[Dq, 2], [N, w], [1, N]]),
                                ap_of(z, mrow + 1,
                                      [[0, 2], [1, w], [0, N]]),
                                OP.mult)
                        return fn
                    G.add(f"RSB{q}", e, 6 * (w + 1) * N,
                          ["KCH", f"C{q}_{w}", f"RC{q}_{w + 1}"],
                          fn_rsb(), fixed=3 * G.EST[e][1])

                    def fn_rsc(q=q, mrow=mrow):
                        lo = coff[q][w + 1]
                        hi = coff[q][L[q]] + 2 * (R[q] - L[q])
                        def fn():
                            v.tensor_tensor(
                                ap_of(consts, lo, [[1, hi - lo]]),
                                ap_of(consts, lo, [[1, hi - lo]]),
                                ap_of(z, mrow + 1, [[0, hi - lo]]),
                                OP.mult)
                        return fn
                    G.add(f"RSC{q}", "v",
                          coff[q][L[q]] + 2 * (R[q] - L[q])
                          - coff[q][w + 1],
                          ["KCH", "dma_tail"], fn_rsc())

            # ---------------- output DMA (per slot, overlaps the rest) ----
            for q in range(NSLOT):
                def efn(q=q):
                    nc.sync.dma_start(
                        ap_of(ecr_d, q * N, [[1, N]],
                              lead=[NSLOT * N, P128]),
                        ap_of(z, zoff["CROUT"] + q * N, [[1, N]]))
                G.add(f"dma_ecr{q}", "d", N * 4, [f"X{q}"], efn)
            G.add("dma_dsum", "d", NSLOT * 4,
                  ["KCH" if nq else "ms_dsum"], lambda: (
                      nc.sync.dma_start(
                          ap_of(dsum_d, 0, [[1, NSLOT]],
                                lead=[NSLOT, P128]),
                          ap_of(z, zoff["DSUM"], [[1, NSLOT]]))))

            G.emit()

    nc.compile()
    return nc


_NC_CACHE = {}


def get_nc(bounds):
    key = tuple(bounds)
    if key not in _NC_CACHE:
        _NC_CACHE[key] = build_nc(key)
    return _NC_CACHE[key]


def plan(len_array):
    """Sort sentences by length desc, deal round-robin to cores, slot into
    4 groups of 128 per core. Returns (order, bounds) where order[r] is the
    original sentence index of global sorted rank r and bounds[q] is the
    width bound of slot q (same for every core by round-robin construction).
    """
    ln = np.asarray(len_array).astype(np.int64)
    order = np.argsort(-ln, kind="stable")
    bounds = [int(ln[order[min(1024 * q, len(ln) - 1)]])
              for q in range(NSLOT)]
    bounds = [max(b, 1) for b in bounds]
    return order, bounds


def make_in_maps(trans_scores, dec_scores, len_array):
    t = np.asarray(trans_scores, dtype=np.float32)
    dec = np.asarray(dec_scores, dtype=np.float32)
    B = t.shape[0]
    order, bounds = plan(len_array)
    lay = _layout(bounds)
    go = dec[..., 0]                        # [B, n, dir, dv]
    # per-sentence linear pre-shift: each arc factor carries exp(-c0), so a
    # width-w entry is scaled exp(-c0*w); undone on the host at the end.
    tm = np.where(t < -1e8, -np.inf, t).max(axis=3)
    with np.errstate(invalid="ignore"):
        colmax = tm.max(axis=1)             # [B, n] best arc into each child
        proxy = np.nanmean(
            np.where(np.isfinite(colmax), colmax, np.nan)[:, 1:], axis=-1)
    c0 = (proxy + 0.5).astype(np.float32)
    c0 = np.clip(np.nan_to_num(c0), -20.0, 20.0)
    # one exp over trans (NEG -> 0 underflow is intended), then gather diags
    with np.errstate(under="ignore"):
        E = np.exp(t - c0[:, None, None, None])      # [B, n, n, 2]
        ego = np.exp(go)                             # [B, n, 2, 2]
    d_idx, i_idx = np.meshgrid(np.arange(N), np.arange(N), indexing="ij")
    j_idx = np.minimum(i_idx + d_idx, N - 1)
    valid = ((i_idx + d_idx) <= N - 1)[None].astype(np.float32)
    ea = E[:, i_idx, j_idx, :]              # [B, n, n, 2]  trans[i, i+d, v]
    eb = E[:, j_idx, i_idx, :]              # [B, n, n, 2]  trans[i+d, i, v]
    a1 = ea[..., 1] * ego[:, :, 1, 1][:, i_idx] * valid   # [B, d, i]
    a0 = ea[..., 0] * ego[:, :, 1, 0][:, i_idx] * valid
    b1 = eb[..., 1] * ego[:, :, 0, 1][:, j_idx] * valid
    b0 = eb[..., 0] * ego[:, :, 0, 0][:, j_idx] * valid
    est = np.exp(dec[..., 1])               # [B, n, dir, dv]

    # sentence index per (core, slot, partition)
    sent = order.reshape(-1, NCORES).T.reshape(NCORES, NSLOT, P128)

    # valence-edge ratios (every packed cell is a valid arc, so a1,b1 > 0)
    with np.errstate(divide="ignore", invalid="ignore"):
        ra = (a0.astype(np.float64) / a1).astype(np.float32)
        rb = (b0.astype(np.float64) / b1).astype(np.float32)
    consts = np.empty((NCORES, P128, lay["const_total"]), dtype=np.float32)
    for q in range(NSLOT):
        sq = sent[:, q, :]                  # [NCORES, P128]
        for w in range(1, bounds[q] + 1):
            s = bounds[q] + 1 - w
            o = lay["coff"][q][w]
            consts[:, :, o:o + s] = a1[sq, w, :s]
            consts[:, :, o + s:o + 2 * s] = b1[sq, w, :s]
            o2 = lay["c2off"][q][w]
            consts[:, :, o2:o2 + s] = ra[sq, w, :s]
            consts[:, :, o2 + s:o2 + 2 * s] = rb[sq, w, :s]
    for q in range(NSLOT):
        o = 4 * q * N
        sq = sent[:, q, :]
        consts[:, :, o:o + N] = est[sq][:, :, :, 1, 1]          # SRHAS
        consts[:, :, o + N:o + 2 * N] = est[sq][:, :, :, 0, 1]  # SLHAS
        consts[:, :, o + 2 * N:o + 3 * N] = est[sq][:, :, :, 1, 0]  # SRNO
        consts[:, :, o + 3 * N:o + 4 * N] = est[sq][:, :, :, 0, 0]  # SLNO
    in_maps = [{"consts": consts[c]} for c in range(NCORES)]
    aux = dict(c0=c0, order=order, bounds=bounds, sent=sent)
    return in_maps, aux


def assemble(results, len_array, aux):
    ln = np.asarray(len_array).astype(np.int64)
    c0 = np.asarray(aux["c0"]).astype(np.float64)
    sent = aux["sent"]
    out = np.empty(len(ln), dtype=np.float32)
    for c, res in enumerate(results):
        ecr = res["ecr"].reshape(P128, NSLOT * N).astype(np.float64)
        dsum = res["dsum"].reshape(P128, NSLOT).astype(np.float64)
        for q in range(NSLOT):
            idx = sent[c, q]                # original sentence ids [P128]
            lc = ln[idx]
            with np.errstate(divide="ignore"):
                out[idx] = (
                    np.log(ecr[np.arange(P128), q * N + lc])
                    + dsum[:, q] * LN2 * lc + c0[idx] * lc
                ).astype(np.float32)
    return out


def kernel(trans_scores, dec_scores, len_array):
    from concourse.bass_utils import run_bass_kernel_spmd

    in_maps, aux = make_in_maps(trans_scores, dec_scores, len_array)
    nc = get_nc(aux["bounds"])
    res = run_bass_kernel_spmd(nc, in_maps, core_ids=list(range(NCORES)))
    return assemble(res.results, len_array, aux)


# revision 55
# speedup vs baseline: 3.0380x; 1.0014x over previous
"""DMV inside algorithm (Eisner chart DP, logsumexp semiring) on Trainium2.

Strategy
--------
Data parallel over the batch: 4096 sentences -> 8 cores x 512, with
length-aware packing: sentences are sorted by length (desc) and dealt
round-robin to cores, then within a core split into 4 slots of 128
(one sentence per SBUF partition per slot). Slot q only runs chart
widths w <= L[q] (its max length), cutting DP work ~2.4x vs running
all sentences to width 40.

The DP runs in the *exp domain* (tables hold exp(score)); each width-w
update is a fused strided multiply + segmented reduce. Segmented
reduces only exist on VectorE (DVE); multiplies are split between DVE
and GpSimd (Pool); the ACT engine rewrites the valence-ratio rows.
All ops are built as an explicit dependency graph and ordered by a
critical-path list scheduler before emission, because every engine
executes its stream strictly in order (a semaphore wait at the head
blocks everything behind it).

Per slot q the 6 diag-packed tables [R=L+1 rows x 41] live at stride
D=41*R in the order [KR, CR, IL, IR, CL, KL], chosen so every fused
operand pair is adjacent: opA in0={KR,CR}, in1={CL,KL}; opB in0={IR,CL},
in1={CR,IL}; outputs {IR,IL}, {KR,KL}, {CR,CL} all constant-stride.
IR/IL row r holds width r+1; IL/opB patterns are column-shifted so all
gathers are constant-stride.

KR/KL row 0 hold the valence edge RATIOS (RA=a0/a1 at KR[0,i],
RB=b0/b1 at KL[0,i+w]), rewritten per width by the otherwise idle ACT
engine; opA's natural w-term gather then covers both valence edge
cases exactly, once IR/IL is scaled by {A1,B1}. Per-arc constants live
in two triangular packs: tri1={A1,B1} (renorm-rescaled), tri2={RA,RB}
(scale-free).

Numerics: scale composes linearly in span width, so slots with L >= 25
renormalize once at w=20: row d of every table is multiplied by an
exact power of two 2^(-k*d) (k integer per sentence), k accumulated in
dsum and undone on the host: LL = log(CR[0,len]) + k*ln2*len + c0*len.
"""

import os

os.environ.setdefault("JAX_PLATFORMS", "cpu")

import heapq

import numpy as np

import concourse.bass as bass  # noqa: F401  (registers engine classes)
import concourse.tile as tile
import bass_rust
from concourse import bacc, mybir

F32 = mybir.dt.float32
AF = mybir.ActivationFunctionType
OP = mybir.AluOpType
AX = mybir.AxisListType

N = 41              # fake_len (ROOT at 0)
NCORES = 8
NSLOT = 4
P128 = 128
B_CORE = NSLOT * P128
RENORM_W = 20       # renorm width (slots with L >= RENORM_MIN_L)
RENORM_MIN_L = 25
HEAD_W = 6          # consts rows <= HEAD_W form the startup DMA region
EDGE_MAX_W = 36     # widths above this run the whole chain on DVE

# table order within a slot (pairs used by the fused ops are adjacent)
T_KR, T_CR, T_IL, T_IR, T_CL, T_KL = range(6)

LN2 = float(np.log(2.0))

# mult-engine assignment per slot: (opA engine, opB engine), 'v'=DVE 'g'=Pool
MULT_CFG = {0: ("v", "v"), 1: ("v", "g"), 2: ("v", "v"), 3: ("v", "v")}
# slots using the edge/main decomposition: the big mults' t∈[1,w-1) bulk
# only depends on width w-2, so it runs a width ahead on Pool while the
# dependency-carrying edge terms (t=0, t=w-1) are tiny [2,2,s] DVE ops.
EDGE_QS = (0,)


def ap_of(t, offset, dims, lead=None):
    """Build a raw AP on tile/dram ap `t`: [lead or t.ap[0]] + dims."""
    ap = t.copy()
    first = list(t.ap[0]) if lead is None else list(lead)
    ap.ap = bass_rust.VecI64Pair([first] + [list(d) for d in dims])
    ap.offset = offset
    return ap


def _layout(bounds):
    """Compute per-slot offsets for banks / consts / stops / scratch."""
    L = list(bounds)
    R = [l + 1 for l in L]
    D = [N * r for r in R]
    bank_base, acc = [], 0
    for q in range(NSLOT):
        bank_base.append(acc)
        acc += 6 * D[q]
    bank_total = acc
    # consts, two triangles per slot: tri1 per (q,w) = [A1(s), B1(s)]
    # (renorm-rescaled) and tri2 per (q,w) = [RA(s), RB(s)] (scale-free
    # ratios). Rows w <= HEAD_W live in a contiguous head region so ONE
    # DMA covers everything the first DP widths need; the rest is one
    # tail DMA. Within each region blocks are (q, tri, w)-ordered, so any
    # (q, tri, w-range) stays contiguous (the renorm tail rescale relies
    # on rows >= RENORM_W+1 of a slot's tri1 being contiguous).
    coff = [dict() for _ in range(NSLOT)]
    c2off = [dict() for _ in range(NSLOT)]
    # stop vectors lead the head region: per q [SRHAS, SLHAS, SRNO, SLNO] x 41
    stop_total = NSLOT * 4 * N
    acc = stop_total
    for wlo, whi in ((1, HEAD_W), (HEAD_W + 1, N)):
        for q in range(NSLOT):
            for offs in (coff, c2off):
                for w in range(wlo, min(whi, L[q]) + 1):
                    offs[q][w] = acc
                    acc += 2 * (R[q] - w)
        if wlo == 1:
            head_end = acc
    const_total = acc
    # z scratch
    z = {}
    zacc = 0
    for q in range(NSLOT):
        pmax = max((2 * (R[q] - w) * w for w in range(1, L[q] + 1)),
                   default=2)
        for pb in ("PA0", "PA1", "PB0", "PB1"):
            z[(pb, q)] = zacc; zacc += pmax
        z[("SS", q)] = zacc; zacc += 2 * N
        z[("T1", q)] = zacc; zacc += 2 * N
    z["CROUT"] = zacc; zacc += NSLOT * N
    z["DSUM"] = zacc; zacc += NSLOT
    z["M2"] = zacc; zacc += 8
    z["MU"] = zacc; zacc += 2
    z["LM"] = zacc; zacc += 2
    z["M"] = zacc; zacc += NSLOT * (N + 1)  # renorm multiplier rows [q, 42]
    z_total = zacc
    return dict(L=L, R=R, D=D, bank_base=bank_base, bank_total=bank_total,
                coff=coff, c2off=c2off, head_end=head_end,
                const_total=const_total,
                stop_total=stop_total, z=z, z_total=z_total)


class Graph:
    """Op graph + critical-path list scheduler for in-order engines."""

    EST = {"v": (1.0417, 107.0), "g": (1.984, 156.0), "a": (0.833, 217.0),
           "d": (0.386, 1600.0)}  # (ns/elem, fixed ns); d: per byte
    XPEN = 100.0  # cross-engine semaphore latency
    SLACK = 100.0  # earliest-start tolerance when picking by priority

    def __init__(self):
        self.nodes = []          # dict(key, eng, est, deps, fn)
        self.byname = {}

    def add(self, key, eng, elems, deps, fn, fixed=None):
        slope, fix = self.EST[eng]
        est = elems * slope + (fix if fixed is None else fixed)
        n = dict(key=key, eng=eng, est=est, deps=[d for d in deps
                                                 if d is not None], fn=fn,
                 idx=len(self.nodes))
        self.nodes.append(n)
        self.byname[key] = n
        return key

    def schedule(self):
        """Earliest-start list scheduling with critical-path tie-break.
        Returns node indices in chosen global emission order."""
        nodes = self.nodes
        nn = len(nodes)
        succ = [[] for _ in range(nn)]
        npred = [0] * nn
        for n in nodes:
            for dk in n["deps"]:
                d = self.byname[dk]
                succ[d["idx"]].append(n["idx"])
                npred[n["idx"]] += 1
        # critical-path priority (longest path to sink)
        prio = [0.0] * nn
        for i in reversed(range(nn)):  # nodes added roughly topologically;
            pass
        order_topo = []
        tmp_pred = npred[:]
        stack = [i for i in range(nn) if tmp_pred[i] == 0]
        while stack:
            i = stack.pop()
            order_topo.append(i)
            for j in succ[i]:
                tmp_pred[j] -= 1
                if tmp_pred[j] == 0:
                    stack.append(j)
        assert len(order_topo) == nn, "cycle in op graph"
        for i in reversed(order_topo):
            best = 0.0
            for j in succ[i]:
                if prio[j] > best:
                    best = prio[j]
            prio[i] = nodes[i]["est"] + best
        # event-driven greedy
        free = {"v": 0.0, "g": 0.0, "a": 0.0, "d": 0.0}
        finish = [0.0] * nn
        ready_t = [0.0] * nn
        npred2 = npred[:]
        ready = [i for i in range(nn) if npred2[i] == 0]
        out = []
        while ready:
            # candidate est-start for each ready node
            best_i, best_start, best_prio = None, None, None
            min_start = min(max(free[nodes[i]["eng"]], ready_t[i])
                            for i in ready)
            for i in ready:
                e = nodes[i]["eng"]
                st = max(free[e], ready_t[i])
                if st <= min_start + self.SLACK:
                    if best_prio is None or prio[i] > best_prio:
                        best_i, best_start, best_prio = i, st, prio[i]
            i = best_i
            ready.remove(i)
            n = nodes[i]
            e = n["eng"]
            st = best_start
            fin = st + n["est"]
            finish[i] = fin
            free[e] = fin
            out.append((st, i))
            for j in succ[i]:
                pen = self.XPEN if nodes[j]["eng"] != e else 0.0
                ready_t[j] = max(ready_t[j], fin + pen)
                npred2[j] -= 1
                if npred2[j] == 0:
                    ready.append(j)
        assert len(out) == nn
        self.makespan = max(finish) if nn else 0.0
        out.sort(key=lambda t: (t[0], t[1]))
        return [i for _, i in out]

    def emit(self):
        order = self.schedule()
        global LAST_MAKESPAN
        LAST_MAKESPAN = self.makespan
        for i in order:
            self.nodes[i]["fn"]()


LAST_MAKESPAN = None


def build_nc(bounds):
    lay = _layout(bounds)
    L, R, D = lay["L"], lay["R"], lay["D"]
    bb = lay["bank_base"]
    coff = lay["coff"]
    c2off = lay["c2off"]
    zoff = lay["z"]
    renorm_qs = [q for q in range(NSLOT) if L[q] >= RENORM_MIN_L]
    # bounds are sorted desc, so renorm slots are the prefix [0..nq) and
    # dsum[j] lines up with slot j
    assert renorm_qs == list(range(len(renorm_qs)))
    nq = len(renorm_qs)

    nc = bacc.Bacc("TRN2", target_bir_lowering=False, debug=False,
                   num_devices=1)
    consts_in = nc.dram_tensor(
        "consts", [P128, lay["const_total"]], F32, kind="ExternalInput").ap()
    ecr_d = nc.dram_tensor(
        "ecr", [P128, NSLOT * N], F32, kind="ExternalOutput").ap()
    dsum_d = nc.dram_tensor(
        "dsum", [P128, NSLOT], F32, kind="ExternalOutput").ap()

    with tile.TileContext(nc) as tc:
        with tc.tile_pool(name="p", bufs=1) as pool:
            banks_t = pool.tile([P128, lay["bank_total"]], F32)
            consts_t = pool.tile([P128, lay["const_total"]], F32)
            z_t = pool.tile([P128, lay["z_total"]], F32)
            banks = banks_t[:]
            consts = consts_t[:]
            stops = consts  # stop vectors lead the consts head region
            z = z_t[:]

            v = nc.vector
            g = nc.gpsimd
            sc = nc.scalar
            eng_of = {"v": v, "g": g}
            G = Graph()

            # ---------------- input DMA nodes ----------------
            def dma_in(dst_off, src_off, size):
                def fn():
                    nc.sync.dma_start(
                        ap_of(consts, dst_off, [[1, size]]),
                        ap_of(consts_in, src_off, [[1, size]],
                              lead=[lay["const_total"], P128]))
                return fn

            he = lay["head_end"]
            G.add("dma_head", "d", he * 4, [], dma_in(0, 0, he))
            G.add("dma_tail", "d", (lay["const_total"] - he) * 4, [],
                  dma_in(he, he, lay["const_total"] - he))
            G.byname["dma_stops"] = G.byname["dma_head"]

            def c1dep(q, w):
                return "dma_head" if w <= HEAD_W else "dma_tail"

            c2dep = c1dep

            # ---------------- init nodes ----------------
            ms_keys = {q: [] for q in range(NSLOT)}
            for q in renorm_qs:
                for ci, (r0, r1) in enumerate(
                        ((0, 3), (3, 7), (7, RENORM_W + 1))):
                    def mfn(q=q, r0=r0, r1=r1):
                        g.memset(
                            ap_of(banks, bb[q] + r0 * N,
                                  [[D[q], 6], [1, (r1 - r0) * N]]), 0.0)
                    ms_keys[q].append(G.add(
                        f"ms{q}_{ci}", "g", 6 * (r1 - r0) * N, [], mfn))
            G.add("ms_dsum", "a", NSLOT, [], lambda: sc.memzero(
                ap_of(z, zoff["DSUM"], [[1, NSLOT]])))
            G.add("ms_crout", "a", NSLOT * N, [], lambda: sc.memzero(
                ap_of(z, zoff["CROUT"], [[1, NSLOT * N]])))
            init_k = {}
            for q in range(NSLOT):
                def ifn(q=q):
                    v.tensor_copy(
                        ap_of(banks, bb[q] + T_CR * D[q],
                              [[3 * D[q], 2], [1, N]]),
                        ap_of(stops, (4 * q + 2) * N, [[N, 2], [1, N]]))
                init_k[q] = G.add(f"init{q}", "v", 2 * N,
                                  ["dma_stops"] + ms_keys[q][:1], ifn)

            # ---------------- DP op builders ----------------
            def fn_ratcopy(w, q):
                s = R[q] - w
                Dq, base = D[q], bb[q]
                def fn():
                    sc.activation(
                        ap_of(banks, base + T_KR * Dq,
                              [[5 * Dq + w, 2], [1, s]]),
                        ap_of(consts, c2off[q][w], [[s, 2], [1, s]]),
                        AF.Copy)
                return fn

            def fn_opA(w, q, e, pb, t0=0, t1=None):
                """t-slice [t0, t1) of the opA product into buffer pb."""
                s = R[q] - w
                Dq, base = D[q], bb[q]
                tn = (w if t1 is None else t1) - t0
                def fn():
                    out = (ap_of(z, zoff[("SS", q)],
                                 [[s, 2], [1, 1], [1, s]])
                           if w == 1 else
                           ap_of(z, zoff[(pb, q)] + t0 * s,
                                 [[w * s, 2], [s, tn], [1, s]]))
                    eng_of[e].tensor_tensor(
                        out,
                        ap_of(banks, base + T_KR * Dq + t0 * N,
                              [[Dq, 2], [N, tn], [1, s]]),
                        ap_of(banks,
                              base + T_CL * Dq + (w - 1 - t0) * N + 1 + t0,
                              [[Dq, 2], [-(N - 1), tn], [1, s]]),
                        OP.mult)
                return fn

            def fn_edge(w, q, tbl0, pb):
                """Fused edge mult: t in {0, w-1} of opA (tbl0=T_KR) or opB
                (tbl0=T_IR) into buffer pb."""
                s = R[q] - w
                Dq, base = D[q], bb[q]
                in1base = (T_CL if tbl0 == T_KR else T_CR)
                def fn():
                    v.tensor_tensor(
                        ap_of(z, zoff[(pb, q)],
                              [[w * s, 2], [(w - 1) * s, 2], [1, s]]),
                        ap_of(banks, base + tbl0 * Dq,
                              [[Dq, 2], [(w - 1) * N, 2], [1, s]]),
                        ap_of(banks, base + in1base * Dq + (w - 1) * N + 1,
                              [[Dq, 2], [-(w - 1) * (N - 1), 2], [1, s]]),
                        OP.mult)
                return fn

            def fn_Ared(w, q, pb):
                s = R[q] - w
                def fn():
                    v.reduce_sum(
                        ap_of(z, zoff[("SS", q)], [[s, 2], [1, s]]),
                        ap_of(z, zoff[(pb, q)],
                              [[w * s, 2], [1, s], [s, w]]),
                        axis=AX.X)
                return fn

            def fn_T1(w, q):
                s = R[q] - w
                Dq, base = D[q], bb[q]
                def fn():
                    v.tensor_tensor(
                        ap_of(banks, base + T_IR * Dq + (w - 1) * N,
                              [[-Dq + 1, 2], [1, s]]),
                        ap_of(z, zoff[("SS", q)], [[s, 2], [1, s]]),
                        ap_of(consts, coff[q][w], [[s, 2], [1, s]]),
                        OP.mult)
                return fn

            def fn_opB(w, q, e, pb, t0=0, t1=None):
                s = R[q] - w
                Dq, base = D[q], bb[q]
                tn = (w if t1 is None else t1) - t0
                def fn():
                    eng_of[e].tensor_tensor(
                        ap_of(z, zoff[(pb, q)] + t0 * s,
                              [[w * s, 2], [s, tn], [1, s]]),
                        ap_of(banks, base + T_IR * Dq + t0 * N,
                              [[Dq, 2], [N, tn], [1, s]]),
                        ap_of(banks,
                              base + T_CR * Dq + (w - 1 - t0) * N + 1 + t0,
                              [[Dq, 2], [-(N - 1), tn], [1, s]]),
                        OP.mult)
                return fn

            def fn_Bred(w, q, pb):
                s = R[q] - w
                Dq, base = D[q], bb[q]
                def fn():
                    krout = ap_of(banks, base + T_KR * Dq + w * N,
                                  [[5 * Dq, 2], [1, s]])
                    if w == 1:
                        v.tensor_tensor(
                            krout,
                            ap_of(banks, base + T_IR * Dq,
                                  [[Dq, 2], [1, s]]),
                            ap_of(banks, base + T_CR * Dq + 1,
                                  [[Dq, 2], [1, s]]),
                            OP.mult)
                    else:
                        v.reduce_sum(
                            krout,
                            ap_of(z, zoff[(pb, q)],
                                  [[w * s, 2], [1, s], [s, w]]),
                            axis=AX.X)
                return fn

            def fn_crcl(w, q):
                s = R[q] - w
                Dq, base = D[q], bb[q]
                def fn():
                    v.tensor_tensor(
                        ap_of(banks, base + T_CR * Dq + w * N,
                              [[3 * Dq, 2], [1, s]]),
                        ap_of(banks, base + T_KR * Dq + w * N,
                              [[5 * Dq, 2], [1, s]]),
                        ap_of(stops, 4 * q * N, [[N + w, 2], [1, s]]),
                        OP.mult)
                return fn

            # ---------------- DP graph ----------------
            for q in range(NSLOT):
                G.add(f"RC{q}_1", "a", 2 * (R[q] - 1),
                      [c2dep(q, 1), ms_keys[q][0] if ms_keys[q] else None],
                      fn_ratcopy(1, q))
            for q in range(NSLOT):
                prevC, prev2C = None, None
                kT1prev = None
                for w in range(1, L[q] + 1):
                    s = R[q] - w
                    ea, ebn = MULT_CFG[q]
                    edged = (q in EDGE_QS and 2 <= w <= EDGE_MAX_W)
                    pa = ("PA1" if w % 2 else "PA0") if edged else "PA0"
                    pbuf = ("PB1" if w % 2 else "PB0") if edged else "PA0"
                    rsb = (f"RSB{q}" if q in renorm_qs
                           and RENORM_W + 1 <= w <= RENORM_W + 2 else None)
                    adeps = [f"RC{q}_{w}", prevC, rsb]
                    if w == 1:
                        adeps += [init_k[q]]
                        if ms_keys[q]:
                            adeps += [ms_keys[q][0]]
                    if edged:
                        kAs = [G.add(f"AE{q}_{w}", "v", 4 * s, adeps,
                                     fn_edge(w, q, T_KR, pa))]
                        if w >= 3:
                            kAs.append(G.add(
                                f"A{q}_{w}", "g", 2 * (w - 2) * s,
                                [prev2C, rsb],
                                fn_opA(w, q, "g", pa, 1, w - 1)))
                        rdr = kAs[0]
                    else:
                        kAs = [G.add(f"A{q}_{w}", ea, 2 * w * s, adeps,
                                     fn_opA(w, q, ea, pa))]
                        rdr = kAs[0]
                    if w < L[q]:
                        # WAR: only the edge (or whole) op reads RAT row 0
                        G.add(f"RC{q}_{w + 1}", "a", 2 * (s - 1),
                              [rdr, c2dep(q, w + 1)], fn_ratcopy(w + 1, q))
                    if w > 1:
                        kR1 = G.add(f"R1{q}_{w}", "v", 2 * w * s,
                                    kAs + [kT1prev], fn_Ared(w, q, pa))
                    else:
                        kR1 = kAs[0]
                    t1deps = [kR1, c1dep(q, w)]
                    if q in renorm_qs and w == RENORM_W + 1:
                        t1deps += [f"RSC{q}"]
                    kT1 = G.add(f"T1{q}_{w}", "v", 2 * s, t1deps,
                                fn_T1(w, q))
                    # staged memsets zero bank rows [3,7) and [7,21); the
                    # first DP write into each zone must wait for its chunk
                    r2deps = []
                    if ms_keys[q]:
                        if w == 3:
                            r2deps.append(ms_keys[q][1])
                        elif w == 7:
                            r2deps.append(ms_keys[q][2])
                    if w > 1:
                        if edged:
                            kBs = [G.add(f"BE{q}_{w}", "v", 4 * s,
                                         [kT1, prevC, rsb],
                                         fn_edge(w, q, T_IR, pbuf))]
                            if w >= 3:
                                kBs.append(G.add(
                                    f"B{q}_{w}", "g", 2 * (w - 2) * s,
                                    [kT1prev, prev2C, rsb],
                                    fn_opB(w, q, "g", pbuf, 1, w - 1)))
                        else:
                            kBs = [G.add(f"B{q}_{w}", ebn, 2 * w * s,
                                         [kT1], fn_opB(w, q, ebn, pbuf))]
                        kR2 = G.add(f"R2{q}_{w}", "v", 2 * w * s,
                                    kBs + r2deps, fn_Bred(w, q, pbuf))
                    else:
                        kR2 = G.add(f"R2{q}_{w}", "v", 2 * s,
                                    [kT1] + r2deps, fn_Bred(w, q, pbuf))
                    prev2C = prevC
                    prevC = G.add(f"C{q}_{w}", "v", 2 * s,
                                  [kR2, "dma_stops"], fn_crcl(w, q))
                    kT1prev = kT1
                # the longest slot's extract ends the kernel: keep it on DVE
                # to skip the ACT handoff; the others overlap on idle ACT
                xeng = "v" if L[q] == max(L) else "a"
                def xfn(q=q, xeng=xeng):
                    out = ap_of(z, zoff["CROUT"] + q * N, [[1, R[q]]])
                    src = ap_of(banks, bb[q] + T_CR * D[q], [[N, R[q]]])
                    if xeng == "v":
                        v.tensor_copy(out, src)
                    else:
                        sc.activation(out, src, AF.Copy)
                G.add(f"X{q}", xeng, R[q], [prevC, "ms_crout"], xfn)

            # ---------------- renorm nodes ----------------
            if nq:
                w = RENORM_W

                def fn_rmax():
                    for j, q in enumerate(renorm_qs):
                        s = R[q] - w
                        v.reduce_max(
                            ap_of(z, zoff["M2"] + 2 * j, [[1, 2]]),
                            ap_of(banks, bb[q] + T_KR * D[q] + w * N,
                                  [[5 * D[q], 2], [1, s]]),
                            axis=AX.X)
                    v.tensor_tensor(
                        ap_of(z, zoff["MU"], [[1, nq]]),
                        ap_of(z, zoff["M2"], [[2, nq]]),
                        ap_of(z, zoff["M2"] + 1, [[2, nq]]),
                        OP.max)
                    v.tensor_scalar_mul(
                        ap_of(z, zoff["MU"], [[1, nq]]),
                        ap_of(z, zoff["MU"], [[1, nq]]), 2.0 ** -32)
                    v.tensor_scalar_max(
                        ap_of(z, zoff["MU"], [[1, nq]]),
                        ap_of(z, zoff["MU"], [[1, nq]]), 1e-36)
                G.add("RMAX", "v", 100,
                      [f"R2{q}_{w}" for q in renorm_qs], fn_rmax,
                      fixed=6 * 107.0)

                G.add("RLN", "a", nq, ["RMAX"], lambda: sc.activation(
                    ap_of(z, zoff["LM"], [[1, nq]]),
                    ap_of(z, zoff["MU"], [[1, nq]]), AF.Ln))

                def fn_kchain():
                    # k = round((ln(mu*2^-32) + 32 ln2)/(w ln2)); round via
                    # 1.5*2^23 so every factor is an exact power of two
                    v.tensor_scalar(
                        ap_of(z, zoff["LM"], [[1, nq]]),
                        ap_of(z, zoff["LM"], [[1, nq]]),
                        32.0 * LN2, 1.0 / (w * LN2), OP.add, OP.mult)
                    v.tensor_scalar(
                        ap_of(z, zoff["LM"], [[1, nq]]),
                        ap_of(z, zoff["LM"], [[1, nq]]),
                        12582912.0, 12582912.0, OP.add, OP.subtract)
                    v.tensor_tensor(
                        ap_of(z, zoff["DSUM"], [[1, nq]]),
                        ap_of(z, zoff["DSUM"], [[1, nq]]),
                        ap_of(z, zoff["LM"], [[1, nq]]),
                        OP.add)
                    # scale2 = 2^-k via exponent bits: (127 - k) << 23
                    v.tensor_scalar(
                        ap_of(z, zoff["M2"], [[1, nq]]),
                        ap_of(z, zoff["LM"], [[1, nq]]),
                        -1.0, 127.0, OP.mult, OP.add)
                    zi = z.bitcast(mybir.dt.int32)
                    v.tensor_copy(
                        ap_of(zi, zoff["M2"] + 4, [[1, nq]]),
                        ap_of(z, zoff["M2"], [[1, nq]]))
                    v.tensor_scalar(
                        ap_of(zi, zoff["M2"] + 4, [[1, nq]]),
                        ap_of(zi, zoff["M2"] + 4, [[1, nq]]),
                        23, None, OP.arith_shift_left)
                    # M[j, d] = 2^(-k*d): d=0 -> 1, multiplicative scan
                    v.memset(ap_of(z, zoff["M"], [[N + 1, nq], [1, 1]]), 1.0)
                    for j in range(nq):
                        sca = ap_of(z, zoff["M2"] + 4 + j, [[0, N]])
                        v.tensor_tensor_scan(
                            ap_of(z, zoff["M"] + j * (N + 1) + 1, [[1, N]]),
                            sca, sca, 1.0, OP.mult, OP.bypass)
                G.add("KCH", "v", 60, ["RLN", "ms_dsum"], fn_kchain,
                      fixed=9 * 107.0)

                for j, q in enumerate(renorm_qs):
                    Dq, base = D[q], bb[q]
                    mrow = zoff["M"] + j * (N + 1)
                    e = "g" if q == 0 else "v"

                    def fn_rsb(q=q, Dq=Dq, base=base, mrow=mrow, e=e):
                        def fn():
                            for tb in (T_KR, T_CL):
                                eng_of[e].tensor_tensor(
                                    ap_of(banks, base + tb * Dq,
                                          [[Dq, 2], [N, w + 1], [1, N]]),
                                    ap_of(banks, base + tb * Dq,
                                          [[Dq, 2], [N, w + 1], [1, N]]),
                                    ap_of(z, mrow,
                                          [[0, 2], [1, w + 1], [0, N]]),
                                    OP.mult)
                            eng_of[e].tensor_tensor(
                                ap_of(banks, base + T_IL * Dq,
                                      [[Dq, 2], [N, w], [1, N]]),
                                ap_of(banks, base + T_IL * Dq,
                                      [[Dq, 2], [N, w], [1, N]]),
                                ap_of(z, mrow + 1,
                                      [[0, 2], [1, w], [0, N]]),
                                OP.mult)
                        return fn
                    G.add(f"RSB{q}", e, 6 * (w + 1) * N,
                          ["KCH", f"C{q}_{w}", f"RC{q}_{w + 1}"],
                          fn_rsb(), fixed=3 * G.EST[e][1])

                    def fn_rsc(q=q, mrow=mrow):
                        lo = coff[q][w + 1]
                        hi = coff[q][L[q]] + 2 * (R[q] - L[q])
                        def fn():
                            v.tensor_tensor(
                                ap_of(consts, lo, [[1, hi - lo]]),
                                ap_of(consts, lo, [[1, hi - lo]]),
                                ap_of(z, mrow + 1, [[0, hi - lo]]),
                                OP.mult)
                        return fn
                    G.add(f"RSC{q}", "v",
                          coff[q][L[q]] + 2 * (R[q] - L[q])
                          - coff[q][w + 1],
                          ["KCH", "dma_tail"], fn_rsc())

            # ---------------- output DMA (per slot, overlaps the rest) ----
            for q in range(NSLOT):
                def efn(q=q):
                    nc.sync.dma_start(
                        ap_of(ecr_d, q * N, [[1, N]],
                              lead=[NSLOT * N, P128]),
                        ap_of(z, zoff["CROUT"] + q * N, [[1, N]]))
                G.add(f"dma_ecr{q}", "d", N * 4, [f"X{q}"], efn)
            G.add("dma_dsum", "d", NSLOT * 4,
                  ["KCH" if nq else "ms_dsum"], lambda: (
                      nc.sync.dma_start(
                          ap_of(dsum_d, 0, [[1, NSLOT]],
                                lead=[NSLOT, P128]),
                          ap_of(z, zoff["DSUM"], [[1, NSLOT]]))))

            G.emit()

    nc.compile()
    return nc


_NC_CACHE = {}


def get_nc(bounds):
    key = tuple(bounds)
    if key not in _NC_CACHE:
        _NC_CACHE[key] = build_nc(key)
    return _NC_CACHE[key]


def plan(len_array):
    """Sort sentences by length desc, deal round-robin to cores, slot into
    4 groups of 128 per core. Returns (order, bounds) where order[r] is the
    original sentence index of global sorted rank r and bounds[q] is the
    width bound of slot q (same for every core by round-robin construction).
    """
    ln = np.asarray(len_array).astype(np.int64)
    order = np.argsort(-ln, kind="stable")
    bounds = [int(ln[order[min(1024 * q, len(ln) - 1)]])
              for q in range(NSLOT)]
    bounds = [max(b, 1) for b in bounds]
    return order, bounds


def make_in_maps(trans_scores, dec_scores, len_array):
    t = np.asarray(trans_scores, dtype=np.float32)
    dec = np.asarray(dec_scores, dtype=np.float32)
    B = t.shape[0]
    order, bounds = plan(len_array)
    lay = _layout(bounds)
    go = dec[..., 0]                        # [B, n, dir, dv]
    # per-sentence linear pre-shift: each arc factor carries exp(-c0), so a
    # width-w entry is scaled exp(-c0*w); undone on the host at the end.
    tm = np.where(t < -1e8, -np.inf, t).max(axis=3)
    with np.errstate(invalid="ignore"):
        colmax = tm.max(axis=1)             # [B, n] best arc into each child
        proxy = np.nanmean(
            np.where(np.isfinite(colmax), colmax, np.nan)[:, 1:], axis=-1)
    c0 = (proxy + 0.5).astype(np.float32)
    c0 = np.clip(np.nan_to_num(c0), -20.0, 20.0)
    # one exp over trans (NEG -> 0 underflow is intended), then gather diags
    with np.errstate(under="ignore"):
        E = np.exp(t - c0[:, None, None, None])      # [B, n, n, 2]
        ego = np.exp(go)                             # [B, n, 2, 2]
    d_idx, i_idx = np.meshgrid(np.arange(N), np.arange(N), indexing="ij")
    j_idx = np.minimum(i_idx + d_idx, N - 1)
    valid = ((i_idx + d_idx) <= N - 1)[None].astype(np.float32)
    ea = E[:, i_idx, j_idx, :]              # [B, n, n, 2]  trans[i, i+d, v]
    eb = E[:, j_idx, i_idx, :]              # [B, n, n, 2]  trans[i+d, i, v]
    a1 = ea[..., 1] * ego[:, :, 1, 1][:, i_idx] * valid   # [B, d, i]
    a0 = ea[..., 0] * ego[:, :, 1, 0][:, i_idx] * valid
    b1 = eb[..., 1] * ego[:, :, 0, 1][:, j_idx] * valid
    b0 = eb[..., 0] * ego[:, :, 0, 0][:, j_idx] * valid
    est = np.exp(dec[..., 1])               # [B, n, dir, dv]

    # sentence index per (core, slot, partition)
    sent = order.reshape(-1, NCORES).T.reshape(NCORES, NSLOT, P128)

    # valence-edge ratios (every packed cell is a valid arc, so a1,b1 > 0)
    with np.errstate(divide="ignore", invalid="ignore"):
        ra = (a0.astype(np.float64) / a1).astype(np.float32)
        rb = (b0.astype(np.float64) / b1).astype(np.float32)
    consts = np.empty((NCORES, P128, lay["const_total"]), dtype=np.float32)
    for q in range(NSLOT):
        sq = sent[:, q, :]                  # [NCORES, P128]
        for w in range(1, bounds[q] + 1):
            s = bounds[q] + 1 - w
            o = lay["coff"][q][w]
            consts[:, :, o:o + s] = a1[sq, w, :s]
            consts[:, :, o + s:o + 2 * s] = b1[sq, w, :s]
            o2 = lay["c2off"][q][w]
            consts[:, :, o2:o2 + s] = ra[sq, w, :s]
            consts[:, :, o2 + s:o2 + 2 * s] = rb[sq, w, :s]
    for q in range(NSLOT):
        o = 4 * q * N
        sq = sent[:, q, :]
        consts[:, :, o:o + N] = est[sq][:, :, :, 1, 1]          # SRHAS
        consts[:, :, o + N:o + 2 * N] = est[sq][:, :, :, 0, 1]  # SLHAS
        consts[:, :, o + 2 * N:o + 3 * N] = est[sq][:, :, :, 1, 0]  # SRNO
        consts[:, :, o + 3 * N:o + 4 * N] = est[sq][:, :, :, 0, 0]  # SLNO
    in_maps = [{"consts": consts[c]} for c in range(NCORES)]
    aux = dict(c0=c0, order=order, bounds=bounds, sent=sent)
    return in_maps, aux


def assemble(results, len_array, aux):
    ln = np.asarray(len_array).astype(np.int64)
    c0 = np.asarray(aux["c0"]).astype(np.float64)
    sent = aux["sent"]
    out = np.empty(len(ln), dtype=np.float32)
    for c, res in enumerate(results):
        ecr = res["ecr"].reshape(P128, NSLOT * N).astype(np.float64)
        dsum = res["dsum"].reshape(P128, NSLOT).astype(np.float64)
        for q in range(NSLOT):
            idx = sent[c, q]                # original sentence ids [P128]
            lc = ln[idx]
            with np.errstate(divide="ignore"):
                out[idx] = (
                    np.log(ecr[np.arange(P128), q * N + lc])
                    + dsum[:, q] * LN2 * lc + c0[idx] * lc
                ).astype(np.float32)
    return out


def kernel(trans_scores, dec_scores, len_array):
    from concourse.bass_utils import run_bass_kernel_spmd

    in_maps, aux = make_in_maps(trans_scores, dec_scores, len_array)
    nc = get_nc(aux["bounds"])
    res = run_bass_kernel_spmd(nc, in_maps, core_ids=list(range(NCORES)))
    return assemble(res.results, len_array, aux)
